# revision 1
# baseline (speedup 1.0000x reference)
"""Trainium2 Bass kernel for fused multi-head attention (CompositeMHA).

Reference computation (B=4, S=1024, E=2048, H=16, D=128), fp32:
    proj = x @ in_proj_weight.T + in_proj_bias        # [B,S,3E]
    q,k,v = split(proj); heads of D=128
    ctx = softmax(q k^T / sqrt(D)) v                   # per (b, head)
    out = ctx @ out_proj_weight.T + out_proj_bias      # [B,S,E]

Sharding (8 cores, no on-device collectives), per the tensor-parallel hint:
data-parallel over the 4 batches x tensor-parallel over head halves.
Core c handles batch c//2 and heads [hh*8, hh*8+8) where hh = c%2 —
sharding the corresponding 3E rows of in_proj_weight and columns of
out_proj_weight.  Each core emits a partial output [S, E]; the gather
step sums each batch's two partials (the TP reduction).

Exact algebraic simplifications (no accuracy cost):
  - K bias dropped: softmax over j of (q+bq)·(k_j+bk) is invariant to
    the j-constant term (q+bq)·bk, so k_j needs no bias.
  - V bias folded into the output bias: sum_j p_j = 1, so
    ctx = ctxU + bv and out = ctxU@Wo^T + (bo + Wo@bv).

Tiling: ALL matmuls use 256-wide moving tensors.  Measured on these
TRN2 cores, an N=256 bf16 matmul streams at ~110 ns (full 2.4 GHz with
the weight load hidden) in chains/singles/interleaved patterns, while
N=512 costs ~272 ns — so 2x256 beats 1x512 by ~20%.  Matmuls are
emitted as pairs of chains sharing each loaded lhsT (B=2 interleave).

On-core dataflow (bf16 matmuls into fp32 PSUM):
    xT   = X_b^T                      [E, S]
    K^T[h] = Wk^T-chunk^T @ xT        [D, S]   per head (no bias)
    Q^T[h] likewise + bq (DVE)        [D, S]   (1/sqrt(D) folded in)
    V      = xT-chunk^T @ Wv^T        [S, E/2] natural layout (no bias)
    scoresT[k,q] = K^T-chunk^T @ Q^T  -> exp on ACT -> P^T (bf16)
    sums[q] = ones^T @ P^T (PE chains); recip (DVE); replicate (GPSIMD)
    ctx^T[h] = V-chunk^T @ P^T accumulated; * recip -> bf16
    out_partial = ctx^T-chunk^T @ Wout^T (+ bo') -> fp32
"""

import numpy as np
import ml_dtypes

B, S, E, H = 4, 1024, 2048, 16
D = 128          # head dim == partition size
P = 128
HH = 8           # heads per core (head half)
EH = HH * D      # 1024: e-columns of this half
EC = E // P      # 16 e-chunks (contraction for in-proj)
OC = EH // P     # 8 e-chunks (contraction for out-proj)
NCORES = 8
NB = 256         # moving-tensor width for all matmuls
BF16 = ml_dtypes.bfloat16

_PROGRAM = None


def _build_program(bench_iters=None, phase="full", opts=None):
    opts = opts or {}
    import concourse.bass as bass  # noqa: F401
    import concourse.tile as tile
    from concourse import bacc, mybir
    from contextlib import ExitStack

    dt = mybir.dt
    AFT = mybir.ActivationFunctionType

    nc = bacc.Bacc("TRN2", target_bir_lowering=False, debug=False,
                   num_devices=NCORES)
    out_dt = dt.bfloat16 if opts.get("out_bf16", True) else dt.float32

    xT_d = nc.dram_tensor("xT", [E, S], dt.bfloat16, kind="ExternalInput").ap()
    wqT_d = nc.dram_tensor("wqT", [E, EH], dt.bfloat16, kind="ExternalInput").ap()
    wkT_d = nc.dram_tensor("wkT", [E, EH], dt.bfloat16, kind="ExternalInput").ap()
    wvT_d = nc.dram_tensor("wvT", [E, EH], dt.bfloat16, kind="ExternalInput").ap()
    woT_d = nc.dram_tensor("woT", [EH, E], dt.bfloat16, kind="ExternalInput").ap()
    bqT_d = nc.dram_tensor("bqT", [P, HH], dt.float32, kind="ExternalInput").ap()
    bo_d = nc.dram_tensor("bo", [1, E], dt.float32, kind="ExternalInput").ap()
    out_d = nc.dram_tensor("out", [S, E], out_dt,
                           kind="ExternalOutput").ap()

    sums_on = opts.get("sums_on", "pe")  # "pe" | "dve"

    with tile.TileContext(nc) as tc, ExitStack() as ctx:
        sb = ctx.enter_context(tc.tile_pool(name="persist", bufs=1))
        wp = ctx.enter_context(tc.tile_pool(name="wstream", bufs=2))
        ktp = ctx.enter_context(tc.tile_pool(name="ktp", bufs=8))
        qtp = ctx.enter_context(tc.tile_pool(name="qtp", bufs=8))
        ptp = ctx.enter_context(tc.tile_pool(name="ptp",
                                             bufs=opts.get("ptp_bufs", 24)))
        outp = ctx.enter_context(tc.tile_pool(name="outp", bufs=6))
        rowp = ctx.enter_context(tc.tile_pool(name="rowp", bufs=3))
        tp = ctx.enter_context(tc.tile_pool(name="tsum", bufs=4))
        # PSUM banks (each buf = 1 bank of [128,512] fp32; A/B pairs are
        # packed as halves of one bank): pp(4) + sp(2) + cp(1) + up(1) = 8.
        # More projection-bank runway won both paired A/Bs (pp2->3: -8us,
        # pp3cp2->pp4cp1: -6/-24us medians); sp=2 covers the lag-2
        # score->exp pipeline.
        ppp = ctx.enter_context(tc.tile_pool(name="ppsum",
                                             bufs=opts.get("pp_bufs", 4),
                                             space="PSUM"))
        spp = ctx.enter_context(tc.tile_pool(name="spsum",
                                             bufs=opts.get("sp_bufs", 2),
                                             space="PSUM"))
        cpp = ctx.enter_context(tc.tile_pool(name="cpsum",
                                             bufs=opts.get("cp_bufs", 1),
                                             space="PSUM"))
        upp = ctx.enter_context(tc.tile_pool(name="upsum", bufs=1,
                                             space="PSUM"))

        def emit():
            # ---- persistent loads ----
            xt = []
            for ec in range(EC):
                t = sb.tile([P, S], dt.bfloat16, name=f"xt{ec}", tag=f"xt{ec}")
                nc.sync.dma_start(t[:], xT_d[ec * P:(ec + 1) * P, :])
                xt.append(t)
            bqt = sb.tile([P, HH], dt.float32, name="bqt", tag="bqt")
            nc.sync.dma_start(bqt[:], bqT_d[:])
            bo_rep = sb.tile([P, E], dt.float32, name="bo_rep", tag="bo_rep")
            nc.sync.dma_start(bo_rep[:], bo_d.to_broadcast((P, E)))
            ones_col = sb.tile([P, 1], dt.bfloat16, name="ones_col",
                               tag="ones_col")
            nc.vector.memset(ones_col[:], 1.0)

            v_sb = []
            for sc in range(S // P):
                t = sb.tile([P, EH], dt.bfloat16, name=f"v{sc}", tag=f"v{sc}")
                v_sb.append(t)
            ctxT = []
            for h in range(HH):
                t = sb.tile([P, S], dt.bfloat16, name=f"ctxT{h}",
                            tag=f"ctxT{h}")
                ctxT.append(t)

            def load_w_tiles(dram, col0, label, nec=EC):
                tiles = []
                for ec in range(nec):
                    t = wp.tile([P, 512], dt.bfloat16,
                                name=f"{label}{ec}", tag=f"w{ec}")
                    nc.sync.dma_start(
                        t[:], dram[ec * P:(ec + 1) * P, col0:col0 + 512])
                    tiles.append(t)
                return tiles

            # ---- per-half pipeline: project 4 heads' K/Q/V, then their
            # ---- attention ----
            KC = S // P  # 8 key chunks
            kt = {}
            qt = {}
            for grp in range(2):
                # K^T for heads 4*grp .. 4*grp+3 (local head index)
                wk = load_w_tiles(wkT_d, grp * 512, f"wk{grp}")
                for hh4 in range(4):
                    h = grp * 4 + hh4
                    kth = ktp.tile([P, S], dt.bfloat16, name=f"kt{h}",
                                   tag="kt")
                    hsl = slice(hh4 * P, (hh4 + 1) * P)
                    for pr in range(2):
                        ps = ppp.tile([P, 512], dt.float32, name="kps",
                                      tag="pp")
                        c0 = pr * 512
                        for ec in range(EC):
                            nc.tensor.matmul(
                                ps[:, 0:NB], wk[ec][:, hsl],
                                xt[ec][:, c0:c0 + NB],
                                start=(ec == 0), stop=False)
                            nc.tensor.matmul(
                                ps[:, NB:2 * NB], wk[ec][:, hsl],
                                xt[ec][:, c0 + NB:c0 + 2 * NB],
                                start=False, stop=(ec == EC - 1))
                        nc.vector.tensor_copy(kth[:, c0:c0 + 512], ps[:])
                    kt[h] = kth

                # Q^T for the same heads (+ per-partition bias)
                wq = load_w_tiles(wqT_d, grp * 512, f"wq{grp}")
                for hh4 in range(4):
                    h = grp * 4 + hh4
                    qth = qtp.tile([P, S], dt.bfloat16, name=f"qt{h}",
                                   tag="qt")
                    hsl = slice(hh4 * P, (hh4 + 1) * P)
                    for pr in range(2):
                        ps = ppp.tile([P, 512], dt.float32, name="qps",
                                      tag="pp")
                        c0 = pr * 512
                        for ec in range(EC):
                            nc.tensor.matmul(
                                ps[:, 0:NB], wq[ec][:, hsl],
                                xt[ec][:, c0:c0 + NB],
                                start=(ec == 0), stop=False)
                            nc.tensor.matmul(
                                ps[:, NB:2 * NB], wq[ec][:, hsl],
                                xt[ec][:, c0 + NB:c0 + 2 * NB],
                                start=False, stop=(ec == EC - 1))
                        nc.vector.tensor_scalar_add(
                            qth[:, c0:c0 + 512], ps[:], bqt[:, h:h + 1])
                    qt[h] = qth

                # V columns for these 4 heads (natural [s, e] layout)
                fw = grp
                wv = load_w_tiles(wvT_d, fw * 512, f"wv{grp}")
                for sc in range(S // P):
                    ps = ppp.tile([P, 512], dt.float32, name="vps", tag="pp")
                    ssl = slice(sc * P, (sc + 1) * P)
                    for ec in range(EC):
                        nc.tensor.matmul(
                            ps[:, 0:NB], xt[ec][:, ssl], wv[ec][:, 0:NB],
                            start=(ec == 0), stop=False)
                        nc.tensor.matmul(
                            ps[:, NB:2 * NB], xt[ec][:, ssl],
                            wv[ec][:, NB:2 * NB],
                            start=False, stop=(ec == EC - 1))
                    nc.vector.tensor_copy(
                        v_sb[sc][:, fw * 512:(fw + 1) * 512], ps[:])

                # attention for heads 4*grp..4*grp+3, q in two 512-blocks
                # (each processed as an A/B pair of 256-wide chains)
                for hh4 in range(4 if phase != "proj" else 0):
                    h = grp * 4 + hh4
                    hsl = slice(h * P, (h + 1) * P)
                    for qb in range(S // 512):
                        qA = slice(qb * 512, qb * 512 + NB)
                        qB = slice(qb * 512 + NB, qb * 512 + 2 * NB)
                        cpt = cpp.tile([P, 512], dt.float32,
                                       name=f"ctx{h}_{qb}", tag="cp")
                        ctxA = cpt[:, 0:NB]
                        ctxB = cpt[:, NB:2 * NB]
                        # software-pipeline by `lag` k-chunks: PE queue is
                        # strict FIFO; ctx consumers of exp(kc) are emitted
                        # after score(kc+lag) to give ACT headroom
                        lag = opts.get("pipe_lag", 2)
                        pts = {}

                        def consume(kc):
                            pt = pts[kc]
                            nc.tensor.matmul(
                                ctxA, v_sb[kc][:, hsl], pt[:, 0:NB],
                                start=(kc == 0), stop=False)
                            nc.tensor.matmul(
                                ctxB, v_sb[kc][:, hsl], pt[:, NB:2 * NB],
                                start=False, stop=(kc == KC - 1))

                        for kc in range(KC):
                            ksl = slice(kc * P, (kc + 1) * P)
                            sp = spp.tile([P, 512], dt.float32, name="sp",
                                          tag="sp")
                            nc.tensor.matmul(sp[:, 0:NB], kt[h][:, ksl],
                                             qt[h][:, qA])
                            nc.tensor.matmul(sp[:, NB:2 * NB], kt[h][:, ksl],
                                             qt[h][:, qB])
                            pt = ptp.tile([P, 512], dt.bfloat16, name="pt",
                                          tag="pt")
                            nc.scalar.activation(pt[:], sp[:], AFT.Exp)
                            pts[kc] = pt
                            if kc >= lag:
                                consume(kc - lag)
                        for kc in range(KC - lag, KC):
                            consume(kc)

                        # row sums over k (128 partitions x 8 chunks)
                        su = upp.tile([1, 512], dt.float32,
                                      name=f"su{h}_{qb}", tag="su")
                        if sums_on == "pe":
                            for kc in range(KC):
                                pt = pts[kc]
                                nc.tensor.matmul(su[:, 0:NB], ones_col[:],
                                                 pt[:, 0:NB],
                                                 start=(kc == 0),
                                                 stop=False)
                                nc.tensor.matmul(su[:, NB:2 * NB],
                                                 ones_col[:],
                                                 pt[:, NB:2 * NB],
                                                 start=False,
                                                 stop=(kc == KC - 1))
                        else:
                            tacc = tp.tile([P, 512], dt.float32, name="tacc",
                                           tag="tacc")
                            nc.vector.tensor_add(tacc[:], pts[0][:],
                                                 pts[1][:])
                            for kc in range(2, KC):
                                nc.vector.tensor_add(tacc[:], tacc[:],
                                                     pts[kc][:])
                            t16 = tp.tile([P, 512], dt.bfloat16, name="t16",
                                          tag="t16")
                            nc.vector.tensor_copy(t16[:], tacc[:])
                            nc.tensor.matmul(su[:, 0:NB], ones_col[:],
                                             t16[:, 0:NB])
                            nc.tensor.matmul(su[:, NB:2 * NB], ones_col[:],
                                             t16[:, NB:2 * NB])
                        r_row = rowp.tile([1, 512], dt.float32, name="r_row",
                                          tag="rrow")
                        nc.vector.reciprocal(r_row[:], su[:])
                        rep = rowp.tile([P, 512], dt.float32, name="rep",
                                        tag="rep")
                        nc.gpsimd.partition_broadcast(rep[:], r_row[:])
                        nc.vector.tensor_mul(
                            ctxT[h][:, qb * 512:(qb + 1) * 512], cpt[:],
                            rep[:])

            # ---- output projection (partial: contracts this e-half) ----
            for fw in range(4 if phase == "full" else 0):
                wo = load_w_tiles(woT_d, fw * 512, f"wo{fw}", nec=OC)
                for qc in range(S // P):
                    qsl = slice(qc * P, (qc + 1) * P)
                    ps = ppp.tile([P, 512], dt.float32, name="ops", tag="pp")
                    for h in range(HH):
                        nc.tensor.matmul(
                            ps[:, 0:NB], ctxT[h][:, qsl], wo[h][:, 0:NB],
                            start=(h == 0), stop=False)
                        nc.tensor.matmul(
                            ps[:, NB:2 * NB], ctxT[h][:, qsl],
                            wo[h][:, NB:2 * NB],
                            start=False, stop=(h == HH - 1))
                    o0 = outp.tile([P, 512], out_dt, name="o0",
                                   tag="ot")
                    f0 = fw * 512
                    nc.vector.tensor_add(o0[:], ps[:],
                                         bo_rep[:, f0:f0 + 512])
                    nc.sync.dma_start(out_d[qsl, f0:f0 + 512], o0[:])

        if bench_iters is None:
            emit()
        else:
            with tc.For_i(0, bench_iters, 1):
                emit()

    nc.compile()
    return nc


def _get_program():
    global _PROGRAM
    if _PROGRAM is None:
        _PROGRAM = _build_program()
    return _PROGRAM


def make_in_maps(query, in_proj_weight, in_proj_bias, out_proj_weight,
                 out_proj_bias):
    """Host-side sharding: slice/transpose/cast per core. Pure layout prep."""
    x = np.asarray(query, dtype=np.float32)
    W = np.asarray(in_proj_weight, dtype=np.float32)
    b = np.asarray(in_proj_bias, dtype=np.float32)
    Wo = np.asarray(out_proj_weight, dtype=np.float32)
    bo = np.asarray(out_proj_bias, dtype=np.float32)

    sc = np.float32(1.0 / np.sqrt(D))
    wqT = np.ascontiguousarray((W[:E] * sc).T).astype(BF16)       # [E, E]
    wkT = np.ascontiguousarray(W[E:2 * E].T).astype(BF16)
    wvT = np.ascontiguousarray(W[2 * E:].T).astype(BF16)
    woT = np.ascontiguousarray(Wo.T).astype(BF16)                 # [E, E]
    bq_s = (b[:E] * sc).reshape(H, P)
    bv = b[2 * E:]                                                # [E]

    in_maps = []
    for c in range(NCORES):
        bi, hh = c // 2, c % 2
        esl = slice(hh * EH, (hh + 1) * EH)
        xT = np.ascontiguousarray(x[bi].T).astype(BF16)
        # fold this half's share of Wo@bv into the output bias; add bo
        # itself only on the hh==0 core (partials are summed)
        bo_half = Wo[:, esl] @ bv[esl]
        if hh == 0:
            bo_half = bo_half + bo
        in_maps.append({
            "xT": xT,
            "wqT": np.ascontiguousarray(wqT[:, esl]),
            "wkT": np.ascontiguousarray(wkT[:, esl]),
            "wvT": np.ascontiguousarray(wvT[:, esl]),
            "woT": np.ascontiguousarray(woT[esl, :]),
            "bqT": np.ascontiguousarray(bq_s[hh * HH:(hh + 1) * HH].T),
            "bo": np.ascontiguousarray(bo_half.reshape(1, E)
                                       .astype(np.float32)),
        })
    return in_maps


def assemble_out(results):
    """Gather: sum each batch's two tensor-parallel partial outputs."""
    out = np.empty((B, S, E), dtype=np.float32)
    for bi in range(B):
        out[bi] = (results[2 * bi]["out"].astype(np.float32)
                   + results[2 * bi + 1]["out"].astype(np.float32))
    return out


def kernel(query, in_proj_weight, in_proj_bias, out_proj_weight,
           out_proj_bias):
    from concourse import bass_utils
    nc = _get_program()
    in_maps = make_in_maps(query, in_proj_weight, in_proj_bias,
                           out_proj_weight, out_proj_bias)
    res = bass_utils.run_bass_kernel_spmd(nc, in_maps,
                                          core_ids=list(range(NCORES)))
    return assemble_out(res.results)



# revision 7
# speedup vs baseline: 1.1161x; 1.1161x over previous
"""Trainium2 Bass kernel for fused multi-head attention (CompositeMHA).

Reference computation (B=4, S=1024, E=2048, H=16, D=128), fp32:
    proj = x @ in_proj_weight.T + in_proj_bias        # [B,S,3E]
    q,k,v = split(proj); heads of D=128
    ctx = softmax(q k^T / sqrt(D)) v                   # per (b, head)
    out = ctx @ out_proj_weight.T + out_proj_bias      # [B,S,E]

Sharding (8 cores, no on-device collectives), per the tensor-parallel hint:
data-parallel over the 4 batches x tensor-parallel over head halves.
Core c handles batch c//2 and heads [hh*8, hh*8+8) where hh = c%2 —
sharding the corresponding 3E rows of in_proj_weight and columns of
out_proj_weight.  Each core emits a partial output [S, E]; the gather
step sums each batch's two partials (the TP reduction).

Exact algebraic simplifications (no accuracy cost):
  - K bias dropped: softmax over j of (q+bq)·(k_j+bk) is invariant to
    the j-constant term (q+bq)·bk, so k_j needs no bias.
  - V bias folded into the output bias: sum_j p_j = 1, so
    ctx = ctxU + bv and out = ctxU@Wo^T + (bo + Wo@bv).

Tiling: ALL matmuls use 256-wide moving tensors (~107 ns measured at
full rate; 512-wide costs ~272 ns, so 2x256 wins).  Matmuls are emitted
as pairs sharing each loaded lhsT.

Schedule v2 (default): the PE queue is kept saturated by interleaving
independent projection / out-projection chains ("filler") into the
attention stream, so cross-engine waits (PE->ACT exp->PE) hide behind
queued matmuls instead of stalling the PE head-of-line:
    P0: K0/Q0/V0 projections (first K chain pair ec-outer, paced by the
        interleaved xT+Wk DMA stream to shrink the startup stall)
    blocks qb0 h0..h3   + filler K1/Q1/V1 chains (forced done here)
    blocks qb0 h4..h7   (no filler available -- ACT-paced stretch)
    blocks qb1 h0..h7   + filler out-proj chains for q-rows of qb0
    tail: out-proj chains for qb1 (pure PE)
Softmax sums run off-PE: DVE accumulates exp chunks (bf16 tree) and
GPSIMD partition_all_reduce replicates the per-q sums; DVE reciprocal +
multiply normalize ctx.  PSUM: 2 proj banks + 4 score banks + 2 ctx
banks.

On-core dataflow (bf16 matmuls into fp32 PSUM):
    xT   = X_b^T                      [E, S]
    K^T[h] = Wk^T-chunk^T @ xT        [D, S]   per head (no bias)
    Q^T[h] likewise + bq (DVE)        [D, S]   (1/sqrt(D) folded in)
    V      = xT-chunk^T @ Wv^T        [S, E/2] natural layout (no bias)
    scoresT[k,q] = K^T-chunk^T @ Q^T  -> exp on ACT -> P^T (bf16)
    sums[q]: DVE add-tree over P^T chunks -> GPSIMD all-reduce -> recip
    ctx^T[h] = V-chunk^T @ P^T accumulated; * recip -> bf16
    out_partial = ctx^T-chunk^T @ Wout^T (+ bo') -> fp32
"""

import numpy as np
import ml_dtypes

B, S, E, H = 4, 1024, 2048, 16
D = 128          # head dim == partition size
P = 128
HH = 8           # heads per core (head half)
EH = HH * D      # 1024: e-columns of this half
EC = E // P      # 16 e-chunks (contraction for in-proj)
OC = EH // P     # 8 e-chunks (contraction for out-proj)
KC = S // P      # 8 key chunks
NCORES = 8
NB = 256         # moving-tensor width for all matmuls
BF16 = ml_dtypes.bfloat16

_PROGRAM = None


def _build_program(bench_iters=None, phase="full", opts=None):
    opts = opts or {}
    import concourse.bass as bass  # noqa: F401
    import concourse.tile as tile
    from concourse import bacc, bass_isa, mybir
    from contextlib import ExitStack

    dt = mybir.dt
    AFT = mybir.ActivationFunctionType
    sched = opts.get("sched", "v2")

    nc = bacc.Bacc("TRN2", target_bir_lowering=False, debug=False,
                   num_devices=NCORES)
    out_dt = dt.bfloat16 if opts.get("out_bf16", True) else dt.float32

    xT_d = nc.dram_tensor("xT", [E, S], dt.bfloat16, kind="ExternalInput").ap()
    wqT_d = nc.dram_tensor("wqT", [E, EH], dt.bfloat16, kind="ExternalInput").ap()
    wkT_d = nc.dram_tensor("wkT", [E, EH], dt.bfloat16, kind="ExternalInput").ap()
    wvT_d = nc.dram_tensor("wvT", [E, EH], dt.bfloat16, kind="ExternalInput").ap()
    woT_d = nc.dram_tensor("woT", [EH, E], dt.bfloat16, kind="ExternalInput").ap()
    bqT_d = nc.dram_tensor("bqT", [P, HH], dt.float32, kind="ExternalInput").ap()
    bo_d = nc.dram_tensor("bo", [1, E], dt.float32, kind="ExternalInput").ap()
    out_d = nc.dram_tensor("out", [S, E], out_dt,
                           kind="ExternalOutput").ap()

    sums_on = opts.get("sums_on", "pe")  # v1: "pe" | "dve"

    with tile.TileContext(nc) as tc, ExitStack() as ctx:
        sb = ctx.enter_context(tc.tile_pool(name="persist", bufs=1))
        wp = ctx.enter_context(tc.tile_pool(name="wstream",
                                            bufs=opts.get("wp_bufs", 2)))
        ktp = ctx.enter_context(tc.tile_pool(name="ktp", bufs=8))
        qtp = ctx.enter_context(tc.tile_pool(name="qtp", bufs=8))
        ptp = ctx.enter_context(tc.tile_pool(
            name="ptp", bufs=opts.get("ptp_bufs", 24 if sched == "v1" else 16)))
        outp = ctx.enter_context(tc.tile_pool(
            name="outp", bufs=6 if sched == "v1" else 16))
        rowp = ctx.enter_context(tc.tile_pool(
            name="rowp", bufs=4 if sched == "v1" else 2))
        tp = ctx.enter_context(tc.tile_pool(
            name="tsum", bufs=4 if sched == "v1" else 2))
        if sched == "v1":
            pp_n, sp_n, cp_n, up_n = (opts.get("pp_bufs", 4),
                                      opts.get("sp_bufs", 2),
                                      opts.get("cp_bufs", 1), 1)
        else:
            pp_n, sp_n, cp_n, up_n = (opts.get("pp_bufs", 2),
                                      opts.get("sp_bufs", 4),
                                      opts.get("cp_bufs", 2), 0)
        ppp = ctx.enter_context(tc.tile_pool(name="ppsum", bufs=pp_n,
                                             space="PSUM"))
        spp = ctx.enter_context(tc.tile_pool(name="spsum", bufs=sp_n,
                                             space="PSUM"))
        cpp = ctx.enter_context(tc.tile_pool(name="cpsum", bufs=cp_n,
                                             space="PSUM"))
        if up_n:
            upp = ctx.enter_context(tc.tile_pool(name="upsum", bufs=up_n,
                                                 space="PSUM"))

        # ================= v2 schedule =================
        def emit_v2():
            mm = nc.tensor.matmul

            # ---- persistent tiles ----
            xt = [sb.tile([P, S], dt.bfloat16, name=f"xt{ec}", tag=f"xt{ec}")
                  for ec in range(EC)]
            bqt = sb.tile([P, HH], dt.float32, name="bqt", tag="bqt")
            bo_rep = sb.tile([P, E], dt.float32, name="bo_rep", tag="bo_rep")
            v_sb = [sb.tile([P, EH], dt.bfloat16, name=f"v{sc}", tag=f"v{sc}")
                    for sc in range(KC)]
            ctxT = [sb.tile([P, S], dt.bfloat16, name=f"ctxT{h}",
                            tag=f"ctxT{h}") for h in range(HH)]

            def w_tiles(dram, col0, label, nec=EC):
                tiles = []
                for ec in range(nec):
                    t = wp.tile([P, 512], dt.bfloat16,
                                name=f"{label}{ec}", tag=f"w{ec}")
                    nc.sync.dma_start(
                        t[:], dram[ec * P:(ec + 1) * P, col0:col0 + 512])
                    tiles.append(t)
                return tiles

            # ---- DMA issue: wk0 and xt interleaved first (startup), then
            # ---- the rest; weight streaming is gated by wp tag rotation.
            wk0 = []
            for ec in range(EC):
                t = wp.tile([P, 512], dt.bfloat16, name=f"wk0_{ec}",
                            tag=f"w{ec}")
                nc.sync.dma_start(t[:], wkT_d[ec * P:(ec + 1) * P, 0:512])
                nc.sync.dma_start(xt[ec][:], xT_d[ec * P:(ec + 1) * P, :])
                wk0.append(t)
            nc.sync.dma_start(bqt[:], bqT_d[:])
            nc.sync.dma_start(bo_rep[:], bo_d.to_broadcast((P, E)))
            wq0 = w_tiles(wqT_d, 0, "wq0")
            wv0 = w_tiles(wvT_d, 0, "wv0")
            wk1 = w_tiles(wkT_d, 512, "wk1")
            wq1 = w_tiles(wqT_d, 512, "wq1")
            wv1 = w_tiles(wvT_d, 512, "wv1")

            kt = {}
            qt = {}

            def kq_chain(w, grp, hh4, pr, dst):
                """One 512-col block of K^T or Q^T for head grp*4+hh4."""
                hsl = slice(hh4 * P, (hh4 + 1) * P)
                c0 = pr * 512
                ps = ppp.tile([P, 512], dt.float32, name="kqps", tag="pp")
                for ec in range(EC):
                    mm(ps[:, 0:NB], w[ec][:, hsl], xt[ec][:, c0:c0 + NB],
                       start=(ec == 0), stop=False)
                    mm(ps[:, NB:2 * NB], w[ec][:, hsl],
                       xt[ec][:, c0 + NB:c0 + 512],
                       start=False, stop=(ec == EC - 1))
                h = grp * 4 + hh4
                if dst is kt:
                    nc.vector.tensor_copy(kt[h][:, c0:c0 + 512], ps[:])
                else:
                    nc.vector.tensor_scalar_add(
                        qt[h][:, c0:c0 + 512], ps[:], bqt[:, h:h + 1])

            def v_chain(wv, fw, sc):
                ps = ppp.tile([P, 512], dt.float32, name="vps", tag="pp")
                ssl = slice(sc * P, (sc + 1) * P)
                for ec in range(EC):
                    mm(ps[:, 0:NB], xt[ec][:, ssl], wv[ec][:, 0:NB],
                       start=(ec == 0), stop=False)
                    mm(ps[:, NB:2 * NB], xt[ec][:, ssl], wv[ec][:, NB:2 * NB],
                       start=False, stop=(ec == EC - 1))
                nc.vector.tensor_copy(
                    v_sb[sc][:, fw * 512:(fw + 1) * 512], ps[:])

            def op_chain(wo, fw, qc):
                qsl = slice(qc * P, (qc + 1) * P)
                ps = ppp.tile([P, 512], dt.float32, name="ops", tag="pp")
                for h in range(HH):
                    mm(ps[:, 0:NB], ctxT[h][:, qsl], wo[h][:, 0:NB],
                       start=(h == 0), stop=False)
                    mm(ps[:, NB:2 * NB], ctxT[h][:, qsl], wo[h][:, NB:2 * NB],
                       start=False, stop=(h == HH - 1))
                o0 = outp.tile([P, 512], out_dt, name="o0", tag="ot")
                f0 = fw * 512
                nc.vector.tensor_add(o0[:], ps[:], bo_rep[:, f0:f0 + 512])
                nc.sync.dma_start(out_d[qsl, f0:f0 + 512], o0[:])

            # allocate kt/qt tiles in head order (tag rotation = 8 live)
            for h in range(HH):
                kt[h] = ktp.tile([P, S], dt.bfloat16, name=f"kt{h}", tag="kt")
                qt[h] = qtp.tile([P, S], dt.bfloat16, name=f"qt{h}", tag="qt")

            # ---- filler queue ----
            filler = []

            def take(n):
                for _ in range(n):
                    if filler:
                        filler.pop(0)()

            def attn_block(h, qb, n_fill):
                hsl = slice(h * P, (h + 1) * P)
                qA = slice(qb * 512, qb * 512 + NB)
                qB = slice(qb * 512 + NB, (qb + 1) * 512)
                nf1 = n_fill // 3
                nf2 = n_fill // 3
                nf3 = n_fill - nf1 - nf2
                cpt = cpp.tile([P, 512], dt.float32, name=f"c{h}_{qb}",
                               tag="cp")
                pts = []
                acc = None
                for kc in range(KC):
                    ksl = slice(kc * P, (kc + 1) * P)
                    sp = spp.tile([P, 512], dt.float32, name="sp", tag="sp")
                    mm(sp[:, 0:NB], kt[h][:, ksl], qt[h][:, qA])
                    mm(sp[:, NB:2 * NB], kt[h][:, ksl], qt[h][:, qB])
                    pt = ptp.tile([P, 512], dt.bfloat16, name="pt", tag="pt")
                    nc.scalar.activation(pt[:], sp[:], AFT.Exp)
                    pts.append(pt)
                    if kc == 1:
                        acc = tp.tile([P, 512], dt.float32, name="acc",
                                      tag="acc")
                        nc.vector.tensor_add(acc[:], pts[0][:], pts[1][:])
                    elif kc > 1:
                        nc.vector.tensor_add(acc[:], acc[:], pt[:])
                    if kc == 2:
                        take(nf1)
                    elif kc == 5:
                        take(nf2)
                take(nf3)
                for kc in range(KC):
                    mm(cpt[:, 0:NB], v_sb[kc][:, hsl], pts[kc][:, 0:NB],
                       start=(kc == 0), stop=False)
                    mm(cpt[:, NB:2 * NB], v_sb[kc][:, hsl],
                       pts[kc][:, NB:2 * NB],
                       start=False, stop=(kc == KC - 1))
                rep = rowp.tile([P, 512], dt.float32, name="rep", tag="rep")
                nc.gpsimd.partition_all_reduce(
                    rep[:], acc[:], channels=P,
                    reduce_op=bass_isa.ReduceOp.add)
                rrec = rowp.tile([P, 512], dt.float32, name="rrec",
                                 tag="rrec")
                nc.vector.reciprocal(rrec[:], rep[:])
                nc.vector.tensor_mul(
                    ctxT[h][:, qb * 512:(qb + 1) * 512], cpt[:], rrec[:])

            # ---- P0: grp0 projections ----
            # first two K head chains ec-outer, 4 accumulators (2 proj banks
            # + 2 borrowed score banks): paced by the interleaved xt+wk DMA
            # stream, so the startup stall shrinks to the DMA/PE rate gap.
            accs = [ppp.tile([P, 512], dt.float32, name="k00", tag="pp"),
                    ppp.tile([P, 512], dt.float32, name="k01", tag="pp"),
                    spp.tile([P, 512], dt.float32, name="k10", tag="sp"),
                    spp.tile([P, 512], dt.float32, name="k11", tag="sp")]
            for ec in range(EC):
                for i, ps in enumerate(accs):
                    hh4, pr = divmod(i, 2)
                    hsl = slice(hh4 * P, (hh4 + 1) * P)
                    c0 = pr * 512
                    mm(ps[:, 0:NB], wk0[ec][:, hsl], xt[ec][:, c0:c0 + NB],
                       start=(ec == 0), stop=False)
                    mm(ps[:, NB:2 * NB], wk0[ec][:, hsl],
                       xt[ec][:, c0 + NB:c0 + 512],
                       start=False, stop=(ec == EC - 1))
            for i, ps in enumerate(accs):
                hh4, pr = divmod(i, 2)
                nc.vector.tensor_copy(kt[hh4][:, pr * 512:(pr + 1) * 512],
                                      ps[:])
            for hh4 in range(2, 4):
                for pr in range(2):
                    kq_chain(wk0, 0, hh4, pr, kt)
            for hh4 in range(4):
                for pr in range(2):
                    kq_chain(wq0, 0, hh4, pr, qt)
            for sc in range(KC):
                v_chain(wv0, 0, sc)

            if phase == "proj":
                for hh4 in range(4):
                    for pr in range(2):
                        kq_chain(wk1, 1, hh4, pr, kt)
                        kq_chain(wq1, 1, hh4, pr, qt)
                for sc in range(KC):
                    v_chain(wv1, 1, sc)
                return

            # ---- filler: grp1 projections (K/Q per head, then V) ----
            for hh4 in range(4):
                for pr in range(2):
                    filler.append(
                        lambda hh4=hh4, pr=pr: kq_chain(wk1, 1, hh4, pr, kt))
                for pr in range(2):
                    filler.append(
                        lambda hh4=hh4, pr=pr: kq_chain(wq1, 1, hh4, pr, qt))
            for sc in range(KC):
                filler.append(lambda sc=sc: v_chain(wv1, 1, sc))
            n_proj_fill = len(filler)  # 24

            # ---- attention qb0, grp0 heads: consume all proj filler ----
            for h in range(4):
                attn_block(h, 0, 6)
            take(len(filler))  # safety drain (no-op if consumed)
            assert n_proj_fill == 24

            # ---- attention qb0, grp1 heads: no filler available ----
            for h in range(4, 8):
                attn_block(h, 0, 0)

            if phase != "full":
                for h in range(HH):
                    attn_block(h, 1, 0)
                return

            # ---- out-proj filler for qb0 q-rows (fw-major; wo tiles are
            # ---- re-loaded per qb so the wp tag rotation stays deadlock
            # ---- free) ----
            wo_q0 = [w_tiles(woT_d, fw * 512, f"woA{fw}", nec=OC)
                     for fw in range(4)]
            for fw in range(4):
                for qc in range(4):
                    filler.append(
                        lambda fw=fw, qc=qc: op_chain(wo_q0[fw], fw, qc))
            # qb1's wo tiles: DMA emitted now (tag rotation gates the actual
            # loads on the qb0 out-proj chains) so the tail doesn't stall
            wo_q1 = [w_tiles(woT_d, fw * 512, f"woB{fw}", nec=OC)
                     for fw in range(4)]

            # ---- attention qb1 ----
            for h in range(4):
                attn_block(h, 1, 3)
            for h in range(4, 8):
                attn_block(h, 1, 1)
            take(len(filler))

            # ---- tail: out-proj for qb1 q-rows ----
            for fw in range(4):
                for qc in range(4, 8):
                    op_chain(wo_q1[fw], fw, qc)

        # ================= v1 schedule (previous baseline) =================
        def emit_v1():
            # ---- persistent loads ----
            xt = []
            for ec in range(EC):
                t = sb.tile([P, S], dt.bfloat16, name=f"xt{ec}", tag=f"xt{ec}")
                nc.sync.dma_start(t[:], xT_d[ec * P:(ec + 1) * P, :])
                xt.append(t)
            bqt = sb.tile([P, HH], dt.float32, name="bqt", tag="bqt")
            nc.sync.dma_start(bqt[:], bqT_d[:])
            bo_rep = sb.tile([P, E], dt.float32, name="bo_rep", tag="bo_rep")
            nc.sync.dma_start(bo_rep[:], bo_d.to_broadcast((P, E)))
            ones_col = sb.tile([P, 1], dt.bfloat16, name="ones_col",
                               tag="ones_col")
            nc.vector.memset(ones_col[:], 1.0)

            v_sb = []
            for sc in range(S // P):
                t = sb.tile([P, EH], dt.bfloat16, name=f"v{sc}", tag=f"v{sc}")
                v_sb.append(t)
            ctxT = []
            for h in range(HH):
                t = sb.tile([P, S], dt.bfloat16, name=f"ctxT{h}",
                            tag=f"ctxT{h}")
                ctxT.append(t)

            def load_w_tiles(dram, col0, label, nec=EC):
                tiles = []
                for ec in range(nec):
                    t = wp.tile([P, 512], dt.bfloat16,
                                name=f"{label}{ec}", tag=f"w{ec}")
                    nc.sync.dma_start(
                        t[:], dram[ec * P:(ec + 1) * P, col0:col0 + 512])
                    tiles.append(t)
                return tiles

            kt = {}
            qt = {}
            for grp in range(2):
                wk = load_w_tiles(wkT_d, grp * 512, f"wk{grp}")
                for hh4 in range(4):
                    h = grp * 4 + hh4
                    kth = ktp.tile([P, S], dt.bfloat16, name=f"kt{h}",
                                   tag="kt")
                    hsl = slice(hh4 * P, (hh4 + 1) * P)
                    for pr in range(2):
                        ps = ppp.tile([P, 512], dt.float32, name="kps",
                                      tag="pp")
                        c0 = pr * 512
                        for ec in range(EC):
                            nc.tensor.matmul(
                                ps[:, 0:NB], wk[ec][:, hsl],
                                xt[ec][:, c0:c0 + NB],
                                start=(ec == 0), stop=False)
                            nc.tensor.matmul(
                                ps[:, NB:2 * NB], wk[ec][:, hsl],
                                xt[ec][:, c0 + NB:c0 + 2 * NB],
                                start=False, stop=(ec == EC - 1))
                        nc.vector.tensor_copy(kth[:, c0:c0 + 512], ps[:])
                    kt[h] = kth

                wq = load_w_tiles(wqT_d, grp * 512, f"wq{grp}")
                for hh4 in range(4):
                    h = grp * 4 + hh4
                    qth = qtp.tile([P, S], dt.bfloat16, name=f"qt{h}",
                                   tag="qt")
                    hsl = slice(hh4 * P, (hh4 + 1) * P)
                    for pr in range(2):
                        ps = ppp.tile([P, 512], dt.float32, name="qps",
                                      tag="pp")
                        c0 = pr * 512
                        for ec in range(EC):
                            nc.tensor.matmul(
                                ps[:, 0:NB], wq[ec][:, hsl],
                                xt[ec][:, c0:c0 + NB],
                                start=(ec == 0), stop=False)
                            nc.tensor.matmul(
                                ps[:, NB:2 * NB], wq[ec][:, hsl],
                                xt[ec][:, c0 + NB:c0 + 2 * NB],
                                start=False, stop=(ec == EC - 1))
                        nc.vector.tensor_scalar_add(
                            qth[:, c0:c0 + 512], ps[:], bqt[:, h:h + 1])
                    qt[h] = qth

                fw = grp
                wv = load_w_tiles(wvT_d, fw * 512, f"wv{grp}")
                for sc in range(S // P):
                    ps = ppp.tile([P, 512], dt.float32, name="vps", tag="pp")
                    ssl = slice(sc * P, (sc + 1) * P)
                    for ec in range(EC):
                        nc.tensor.matmul(
                            ps[:, 0:NB], xt[ec][:, ssl], wv[ec][:, 0:NB],
                            start=(ec == 0), stop=False)
                        nc.tensor.matmul(
                            ps[:, NB:2 * NB], xt[ec][:, ssl],
                            wv[ec][:, NB:2 * NB],
                            start=False, stop=(ec == EC - 1))
                    nc.vector.tensor_copy(
                        v_sb[sc][:, fw * 512:(fw + 1) * 512], ps[:])

                for hh4 in range(4 if phase != "proj" else 0):
                    h = grp * 4 + hh4
                    hsl = slice(h * P, (h + 1) * P)
                    for qb in range(S // 512):
                        qA = slice(qb * 512, qb * 512 + NB)
                        qB = slice(qb * 512 + NB, qb * 512 + 2 * NB)
                        cpt = cpp.tile([P, 512], dt.float32,
                                       name=f"ctx{h}_{qb}", tag="cp")
                        ctxA = cpt[:, 0:NB]
                        ctxB = cpt[:, NB:2 * NB]
                        lag = opts.get("pipe_lag", 2)
                        pts = {}

                        def consume(kc):
                            pt = pts[kc]
                            nc.tensor.matmul(
                                ctxA, v_sb[kc][:, hsl], pt[:, 0:NB],
                                start=(kc == 0), stop=False)
                            nc.tensor.matmul(
                                ctxB, v_sb[kc][:, hsl], pt[:, NB:2 * NB],
                                start=False, stop=(kc == KC - 1))

                        for kc in range(KC):
                            ksl = slice(kc * P, (kc + 1) * P)
                            sp = spp.tile([P, 512], dt.float32, name="sp",
                                          tag="sp")
                            nc.tensor.matmul(sp[:, 0:NB], kt[h][:, ksl],
                                             qt[h][:, qA])
                            nc.tensor.matmul(sp[:, NB:2 * NB], kt[h][:, ksl],
                                             qt[h][:, qB])
                            pt = ptp.tile([P, 512], dt.bfloat16, name="pt",
                                          tag="pt")
                            nc.scalar.activation(pt[:], sp[:], AFT.Exp)
                            pts[kc] = pt
                            if kc >= lag:
                                consume(kc - lag)
                        for kc in range(KC - lag, KC):
                            consume(kc)

                        su = upp.tile([1, 512], dt.float32,
                                      name=f"su{h}_{qb}", tag="su")
                        if sums_on == "pe":
                            for kc in range(KC):
                                pt = pts[kc]
                                nc.tensor.matmul(su[:, 0:NB], ones_col[:],
                                                 pt[:, 0:NB],
                                                 start=(kc == 0),
                                                 stop=False)
                                nc.tensor.matmul(su[:, NB:2 * NB],
                                                 ones_col[:],
                                                 pt[:, NB:2 * NB],
                                                 start=False,
                                                 stop=(kc == KC - 1))
                        else:
                            tacc = tp.tile([P, 512], dt.float32, name="tacc",
                                           tag="tacc")
                            nc.vector.tensor_add(tacc[:], pts[0][:],
                                                 pts[1][:])
                            for kc in range(2, KC):
                                nc.vector.tensor_add(tacc[:], tacc[:],
                                                     pts[kc][:])
                            t16 = tp.tile([P, 512], dt.bfloat16, name="t16",
                                          tag="t16")
                            nc.vector.tensor_copy(t16[:], tacc[:])
                            nc.tensor.matmul(su[:, 0:NB], ones_col[:],
                                             t16[:, 0:NB])
                            nc.tensor.matmul(su[:, NB:2 * NB], ones_col[:],
                                             t16[:, NB:2 * NB])
                        r_row = rowp.tile([1, 512], dt.float32, name="r_row",
                                          tag="rrow")
                        nc.vector.reciprocal(r_row[:], su[:])
                        rep = rowp.tile([P, 512], dt.float32, name="rep",
                                        tag="rep")
                        nc.gpsimd.partition_broadcast(rep[:], r_row[:])
                        nc.vector.tensor_mul(
                            ctxT[h][:, qb * 512:(qb + 1) * 512], cpt[:],
                            rep[:])

            for fw in range(4 if phase == "full" else 0):
                wo = load_w_tiles(woT_d, fw * 512, f"wo{fw}", nec=OC)
                for qc in range(S // P):
                    qsl = slice(qc * P, (qc + 1) * P)
                    ps = ppp.tile([P, 512], dt.float32, name="ops", tag="pp")
                    for h in range(HH):
                        nc.tensor.matmul(
                            ps[:, 0:NB], ctxT[h][:, qsl], wo[h][:, 0:NB],
                            start=(h == 0), stop=False)
                        nc.tensor.matmul(
                            ps[:, NB:2 * NB], ctxT[h][:, qsl],
                            wo[h][:, NB:2 * NB],
                            start=False, stop=(h == HH - 1))
                    o0 = outp.tile([P, 512], out_dt, name="o0",
                                   tag="ot")
                    f0 = fw * 512
                    nc.vector.tensor_add(o0[:], ps[:],
                                         bo_rep[:, f0:f0 + 512])
                    nc.sync.dma_start(out_d[qsl, f0:f0 + 512], o0[:])

        emit = emit_v2 if sched == "v2" else emit_v1
        if bench_iters is None:
            emit()
        else:
            with tc.For_i(0, bench_iters, 1):
                emit()

    nc.compile()
    return nc


def _get_program():
    global _PROGRAM
    if _PROGRAM is None:
        _PROGRAM = _build_program()
    return _PROGRAM


def make_in_maps(query, in_proj_weight, in_proj_bias, out_proj_weight,
                 out_proj_bias):
    """Host-side sharding: slice/transpose/cast per core. Pure layout prep."""
    x = np.asarray(query, dtype=np.float32)
    W = np.asarray(in_proj_weight, dtype=np.float32)
    b = np.asarray(in_proj_bias, dtype=np.float32)
    Wo = np.asarray(out_proj_weight, dtype=np.float32)
    bo = np.asarray(out_proj_bias, dtype=np.float32)

    sc = np.float32(1.0 / np.sqrt(D))
    wqT = np.ascontiguousarray((W[:E] * sc).T).astype(BF16)       # [E, E]
    wkT = np.ascontiguousarray(W[E:2 * E].T).astype(BF16)
    wvT = np.ascontiguousarray(W[2 * E:].T).astype(BF16)
    woT = np.ascontiguousarray(Wo.T).astype(BF16)                 # [E, E]
    bq_s = (b[:E] * sc).reshape(H, P)
    bv = b[2 * E:]                                                # [E]

    in_maps = []
    for c in range(NCORES):
        bi, hh = c // 2, c % 2
        esl = slice(hh * EH, (hh + 1) * EH)
        xT = np.ascontiguousarray(x[bi].T).astype(BF16)
        # fold this half's share of Wo@bv into the output bias; add bo
        # itself only on the hh==0 core (partials are summed)
        bo_half = Wo[:, esl] @ bv[esl]
        if hh == 0:
            bo_half = bo_half + bo
        in_maps.append({
            "xT": xT,
            "wqT": np.ascontiguousarray(wqT[:, esl]),
            "wkT": np.ascontiguousarray(wkT[:, esl]),
            "wvT": np.ascontiguousarray(wvT[:, esl]),
            "woT": np.ascontiguousarray(woT[esl, :]),
            "bqT": np.ascontiguousarray(bq_s[hh * HH:(hh + 1) * HH].T),
            "bo": np.ascontiguousarray(bo_half.reshape(1, E)
                                       .astype(np.float32)),
        })
    return in_maps


def assemble_out(results):
    """Gather: sum each batch's two tensor-parallel partial outputs."""
    out = np.empty((B, S, E), dtype=np.float32)
    for bi in range(B):
        out[bi] = (results[2 * bi]["out"].astype(np.float32)
                   + results[2 * bi + 1]["out"].astype(np.float32))
    return out


def kernel(query, in_proj_weight, in_proj_bias, out_proj_weight,
           out_proj_bias):
    from concourse import bass_utils
    nc = _get_program()
    in_maps = make_in_maps(query, in_proj_weight, in_proj_bias,
                           out_proj_weight, out_proj_bias)
    res = bass_utils.run_bass_kernel_spmd(nc, in_maps,
                                          core_ids=list(range(NCORES)))
    return assemble_out(res.results)


# revision 13
# speedup vs baseline: 1.1560x; 1.0358x over previous
"""Trainium2 Bass kernel for fused multi-head attention (CompositeMHA).

Reference computation (B=4, S=1024, E=2048, H=16, D=128), fp32:
    proj = x @ in_proj_weight.T + in_proj_bias        # [B,S,3E]
    q,k,v = split(proj); heads of D=128
    ctx = softmax(q k^T / sqrt(D)) v                   # per (b, head)
    out = ctx @ out_proj_weight.T + out_proj_bias      # [B,S,E]

Sharding (8 cores, no on-device collectives), per the tensor-parallel hint:
data-parallel over the 4 batches x tensor-parallel over head halves.
Core c handles batch c//2 and heads [hh*8, hh*8+8) where hh = c%2 —
sharding the corresponding 3E rows of in_proj_weight and columns of
out_proj_weight.  Each core emits a partial output [S, E]; the gather
step sums each batch's two partials (the TP reduction).

Exact algebraic simplifications (no accuracy cost):
  - K bias dropped: softmax over j of (q+bq)·(k_j+bk) is invariant to
    the j-constant term (q+bq)·bk, so k_j needs no bias.
  - V bias folded into the output bias: sum_j p_j = 1, so
    ctx = ctxU + bv and out = ctxU@Wo^T + (bo + Wo@bv).

Tiling: ALL matmuls use 256-wide moving tensors (~107 ns measured at
full rate; 512-wide costs ~272 ns, so 2x256 wins).  Matmuls are emitted
as pairs sharing each loaded lhsT.

Schedule v2 (default): the PE queue is kept saturated by interleaving
independent projection / out-projection chains ("filler") into the
attention stream, so cross-engine waits (PE->ACT exp->PE) hide behind
queued matmuls instead of stalling the PE head-of-line:
    P0: K0/Q0/V0 projections (first two K head chains ec-outer, paced
        by the interleaved xT+Wk DMA stream to shrink the startup stall)
    blocks qb0 h0..h3   + filler K1/Q1/V1 chains (V1 spills into q0h4)
    blocks qb0 h4..h7   (little filler left -- ACT-paced stretch)
    blocks qb1 h0..h7   + filler out-proj chains for q-rows of qb0
    tail: out-proj chains for qb1 (pure PE)
Softmax sums+broadcast are FUSED into one PE accumulation per block:
rep[128,512] = ones[128x128] @ P^T summed over k-chunks replicates the
per-q denominators on every partition; only reciprocal + multiply run
on DVE.  (GPSIMD partition_all_reduce measured ~7 us/op on real HW --
111 us slower per kernel than this, despite a 0.5 us cost-model price;
keep ucode ops off the critical path.)  PSUM: 2 proj + 3 score + 2 ctx
+ 1 sums banks.

On-core dataflow (bf16 matmuls into fp32 PSUM):
    xT   = X_b^T                      [E, S]
    K^T[h] = Wk^T-chunk^T @ xT        [D, S]   per head (no bias)
    Q^T[h] likewise + bq (DVE)        [D, S]   (1/sqrt(D) folded in)
    V      = xT-chunk^T @ Wv^T        [S, E/2] natural layout (no bias)
    scoresT[k,q] = K^T-chunk^T @ Q^T  -> exp on ACT -> P^T (bf16)
    sums[q]: DVE add-tree over P^T chunks -> GPSIMD all-reduce -> recip
    ctx^T[h] = V-chunk^T @ P^T accumulated; * recip -> bf16
    out_partial = ctx^T-chunk^T @ Wout^T (+ bo') -> fp32
"""

import numpy as np
import ml_dtypes

B, S, E, H = 4, 1024, 2048, 16
D = 128          # head dim == partition size
P = 128
HH = 8           # heads per core (head half)
EH = HH * D      # 1024: e-columns of this half
EC = E // P      # 16 e-chunks (contraction for in-proj)
OC = EH // P     # 8 e-chunks (contraction for out-proj)
KC = S // P      # 8 key chunks
NCORES = 8
NB = 256         # moving-tensor width for all matmuls
BF16 = ml_dtypes.bfloat16

_PROGRAM = None


def _build_program(bench_iters=None, phase="full", opts=None):
    opts = opts or {}
    import concourse.bass as bass  # noqa: F401
    import concourse.tile as tile
    from concourse import bacc, bass_isa, mybir
    from contextlib import ExitStack

    dt = mybir.dt
    AFT = mybir.ActivationFunctionType
    sched = opts.get("sched", "v2")

    nc = bacc.Bacc("TRN2", target_bir_lowering=False, debug=False,
                   num_devices=NCORES)
    out_dt = dt.bfloat16 if opts.get("out_bf16", True) else dt.float32

    xT_d = nc.dram_tensor("xT", [E, S], dt.bfloat16, kind="ExternalInput").ap()
    wqT_d = nc.dram_tensor("wqT", [E, EH], dt.bfloat16, kind="ExternalInput").ap()
    wkT_d = nc.dram_tensor("wkT", [E, EH], dt.bfloat16, kind="ExternalInput").ap()
    wvT_d = nc.dram_tensor("wvT", [E, EH], dt.bfloat16, kind="ExternalInput").ap()
    woT_d = nc.dram_tensor("woT", [EH, E], dt.bfloat16, kind="ExternalInput").ap()
    bqT_d = nc.dram_tensor("bqT", [P, HH], dt.float32, kind="ExternalInput").ap()
    bo_d = nc.dram_tensor("bo", [1, E], dt.float32, kind="ExternalInput").ap()
    out_d = nc.dram_tensor("out", [S, E], out_dt,
                           kind="ExternalOutput").ap()

    sums_on = opts.get("sums_on", "pe")  # v1: "pe" | "dve"

    with tile.TileContext(nc) as tc, ExitStack() as ctx:
        sb = ctx.enter_context(tc.tile_pool(name="persist", bufs=1))
        wp = ctx.enter_context(tc.tile_pool(name="wstream",
                                            bufs=opts.get("wp_bufs", 2)))
        ktp = ctx.enter_context(tc.tile_pool(name="ktp", bufs=8))
        qtp = ctx.enter_context(tc.tile_pool(name="qtp", bufs=8))
        ptp = ctx.enter_context(tc.tile_pool(
            name="ptp", bufs=opts.get("ptp_bufs", 24 if sched == "v1" else 16)))
        outp = ctx.enter_context(tc.tile_pool(
            name="outp", bufs=6 if sched == "v1" else 16))
        rowp = ctx.enter_context(tc.tile_pool(
            name="rowp", bufs=4 if sched == "v1" else 2))
        tp = ctx.enter_context(tc.tile_pool(
            name="tsum", bufs=4 if sched == "v1" else 2))
        if sched == "v1":
            pp_n, sp_n, cp_n, up_n = (opts.get("pp_bufs", 4),
                                      opts.get("sp_bufs", 2),
                                      opts.get("cp_bufs", 1), 1)
        else:
            pp_n, sp_n, cp_n, up_n = (opts.get("pp_bufs", 2),
                                      opts.get("sp_bufs", 3),
                                      opts.get("cp_bufs", 2), 1)
        ppp = ctx.enter_context(tc.tile_pool(name="ppsum", bufs=pp_n,
                                             space="PSUM"))
        spp = ctx.enter_context(tc.tile_pool(name="spsum", bufs=sp_n,
                                             space="PSUM"))
        cpp = ctx.enter_context(tc.tile_pool(name="cpsum", bufs=cp_n,
                                             space="PSUM"))
        if up_n:
            upp = ctx.enter_context(tc.tile_pool(name="upsum", bufs=up_n,
                                                 space="PSUM"))

        # ================= v2 schedule =================
        def emit_v2():
            mm = nc.tensor.matmul

            # ---- persistent tiles ----
            xt = [sb.tile([P, S], dt.bfloat16, name=f"xt{ec}", tag=f"xt{ec}")
                  for ec in range(EC)]
            bqt = sb.tile([P, HH], dt.float32, name="bqt", tag="bqt")
            bo_rep = sb.tile([P, E], dt.float32, name="bo_rep", tag="bo_rep")
            ones128 = sb.tile([P, P], dt.bfloat16, name="ones128",
                              tag="ones128")
            nc.vector.memset(ones128[:], 1.0)
            v_sb = [sb.tile([P, EH], dt.bfloat16, name=f"v{sc}", tag=f"v{sc}")
                    for sc in range(KC)]
            ctxT = [sb.tile([P, S], dt.bfloat16, name=f"ctxT{h}",
                            tag=f"ctxT{h}") for h in range(HH)]

            def w_tiles(dram, col0, label, nec=EC):
                tiles = []
                for ec in range(nec):
                    t = wp.tile([P, 512], dt.bfloat16,
                                name=f"{label}{ec}", tag=f"w{ec}")
                    nc.sync.dma_start(
                        t[:], dram[ec * P:(ec + 1) * P, col0:col0 + 512])
                    tiles.append(t)
                return tiles

            # ---- DMA issue: wk0 and xt interleaved first (startup), then
            # ---- the rest; weight streaming is gated by wp tag rotation.
            wk0 = []
            for ec in range(EC):
                t = wp.tile([P, 512], dt.bfloat16, name=f"wk0_{ec}",
                            tag=f"w{ec}")
                nc.sync.dma_start(t[:], wkT_d[ec * P:(ec + 1) * P, 0:512])
                nc.sync.dma_start(xt[ec][:], xT_d[ec * P:(ec + 1) * P, :])
                wk0.append(t)
            nc.sync.dma_start(bqt[:], bqT_d[:])
            nc.sync.dma_start(bo_rep[:], bo_d.to_broadcast((P, E)))
            wq0 = w_tiles(wqT_d, 0, "wq0")
            wv0 = w_tiles(wvT_d, 0, "wv0")
            wk1 = w_tiles(wkT_d, 512, "wk1")
            wq1 = w_tiles(wqT_d, 512, "wq1")
            wv1 = w_tiles(wvT_d, 512, "wv1")

            kt = {}
            qt = {}

            def kq_chain(w, grp, hh4, pr, dst):
                """One 512-col block of K^T or Q^T for head grp*4+hh4."""
                hsl = slice(hh4 * P, (hh4 + 1) * P)
                c0 = pr * 512
                ps = ppp.tile([P, 512], dt.float32, name="kqps", tag="pp")
                for ec in range(EC):
                    mm(ps[:, 0:NB], w[ec][:, hsl], xt[ec][:, c0:c0 + NB],
                       start=(ec == 0), stop=False)
                    mm(ps[:, NB:2 * NB], w[ec][:, hsl],
                       xt[ec][:, c0 + NB:c0 + 512],
                       start=False, stop=(ec == EC - 1))
                h = grp * 4 + hh4
                if dst is kt:
                    nc.vector.tensor_copy(kt[h][:, c0:c0 + 512], ps[:])
                else:
                    nc.vector.tensor_scalar_add(
                        qt[h][:, c0:c0 + 512], ps[:], bqt[:, h:h + 1])

            def v_chain(wv, fw, sc):
                ps = ppp.tile([P, 512], dt.float32, name="vps", tag="pp")
                ssl = slice(sc * P, (sc + 1) * P)
                for ec in range(EC):
                    mm(ps[:, 0:NB], xt[ec][:, ssl], wv[ec][:, 0:NB],
                       start=(ec == 0), stop=False)
                    mm(ps[:, NB:2 * NB], xt[ec][:, ssl], wv[ec][:, NB:2 * NB],
                       start=False, stop=(ec == EC - 1))
                nc.vector.tensor_copy(
                    v_sb[sc][:, fw * 512:(fw + 1) * 512], ps[:])

            def op_chain(wo, fw, qc):
                qsl = slice(qc * P, (qc + 1) * P)
                ps = ppp.tile([P, 512], dt.float32, name="ops", tag="pp")
                for h in range(HH):
                    mm(ps[:, 0:NB], ctxT[h][:, qsl], wo[h][:, 0:NB],
                       start=(h == 0), stop=False)
                    mm(ps[:, NB:2 * NB], ctxT[h][:, qsl], wo[h][:, NB:2 * NB],
                       start=False, stop=(h == HH - 1))
                o0 = outp.tile([P, 512], out_dt, name="o0", tag="ot")
                f0 = fw * 512
                nc.vector.tensor_add(o0[:], ps[:], bo_rep[:, f0:f0 + 512])
                nc.sync.dma_start(out_d[qsl, f0:f0 + 512], o0[:])

            # allocate kt/qt tiles in head order (tag rotation = 8 live)
            for h in range(HH):
                kt[h] = ktp.tile([P, S], dt.bfloat16, name=f"kt{h}", tag="kt")
                qt[h] = qtp.tile([P, S], dt.bfloat16, name=f"qt{h}", tag="qt")

            # ---- filler queue ----
            filler = []

            def take(n):
                for _ in range(n):
                    if filler:
                        filler.pop(0)()

            probe = opts.get("probe")  # None|"const_p"|"no_tail"

            def attn_block(h, qb, n_fill):
                hsl = slice(h * P, (h + 1) * P)
                qA = slice(qb * 512, qb * 512 + NB)
                qB = slice(qb * 512 + NB, (qb + 1) * 512)
                nf1 = n_fill // 3
                nf2 = n_fill // 3
                nf3 = n_fill - nf1 - nf2
                cpt = cpp.tile([P, 512], dt.float32, name=f"c{h}_{qb}",
                               tag="cp")
                pts = []
                for kc in range(KC):
                    ksl = slice(kc * P, (kc + 1) * P)
                    sp = spp.tile([P, 512], dt.float32, name="sp", tag="sp")
                    mm(sp[:, 0:NB], kt[h][:, ksl], qt[h][:, qA])
                    mm(sp[:, NB:2 * NB], kt[h][:, ksl], qt[h][:, qB])
                    if probe != "const_p":
                        pt = ptp.tile([P, 512], dt.bfloat16, name="pt",
                                      tag="pt")
                        nc.scalar.activation(pt[:], sp[:], AFT.Exp)
                        pts.append(pt)
                    if kc == 2:
                        take(nf1)
                    elif kc == 5:
                        take(nf2)
                take(nf3)
                for kc in range(KC):
                    rhs = (v_sb[kc][:, 0:512] if probe == "const_p"
                           else pts[kc][:])
                    mm(cpt[:, 0:NB], v_sb[kc][:, hsl], rhs[:, 0:NB],
                       start=(kc == 0), stop=False)
                    mm(cpt[:, NB:2 * NB], v_sb[kc][:, hsl],
                       rhs[:, NB:2 * NB],
                       start=False, stop=(kc == KC - 1))
                if probe in ("const_p", "no_tail"):
                    nc.vector.tensor_copy(
                        ctxT[h][:, qb * 512:(qb + 1) * 512], cpt[:])
                    return
                # fused sums+broadcast on PE: rep[p, q] = sum_k P^T[k, q]
                # (ones lhsT replicates the column sums on every partition)
                rep = upp.tile([P, 512], dt.float32, name=f"rep{h}_{qb}",
                               tag="rep")
                for kc in range(KC):
                    mm(rep[:, 0:NB], ones128[:], pts[kc][:, 0:NB],
                       start=(kc == 0), stop=False)
                    mm(rep[:, NB:2 * NB], ones128[:], pts[kc][:, NB:2 * NB],
                       start=False, stop=(kc == KC - 1))
                rrec = rowp.tile([P, 512], dt.float32, name="rrec",
                                 tag="rrec")
                nc.vector.reciprocal(rrec[:], rep[:])
                nc.vector.tensor_mul(
                    ctxT[h][:, qb * 512:(qb + 1) * 512], cpt[:], rrec[:])

            # ---- P0: grp0 projections ----
            # first two K head chains ec-outer, 4 accumulators (2 proj banks
            # + 2 borrowed score banks): paced by the interleaved xt+wk DMA
            # stream, so the startup stall shrinks to the DMA/PE rate gap.
            accs = [ppp.tile([P, 512], dt.float32, name="k00", tag="pp"),
                    ppp.tile([P, 512], dt.float32, name="k01", tag="pp"),
                    spp.tile([P, 512], dt.float32, name="k10", tag="sp"),
                    spp.tile([P, 512], dt.float32, name="k11", tag="sp")]
            for ec in range(EC):
                for i, ps in enumerate(accs):
                    hh4, pr = divmod(i, 2)
                    hsl = slice(hh4 * P, (hh4 + 1) * P)
                    c0 = pr * 512
                    mm(ps[:, 0:NB], wk0[ec][:, hsl], xt[ec][:, c0:c0 + NB],
                       start=(ec == 0), stop=False)
                    mm(ps[:, NB:2 * NB], wk0[ec][:, hsl],
                       xt[ec][:, c0 + NB:c0 + 512],
                       start=False, stop=(ec == EC - 1))
            for i, ps in enumerate(accs):
                hh4, pr = divmod(i, 2)
                nc.vector.tensor_copy(kt[hh4][:, pr * 512:(pr + 1) * 512],
                                      ps[:])
            for hh4 in range(2, 4):
                for pr in range(2):
                    kq_chain(wk0, 0, hh4, pr, kt)
            for hh4 in range(4):
                for pr in range(2):
                    kq_chain(wq0, 0, hh4, pr, qt)
            for sc in range(KC):
                v_chain(wv0, 0, sc)

            if phase == "proj":
                for hh4 in range(4):
                    for pr in range(2):
                        kq_chain(wk1, 1, hh4, pr, kt)
                        kq_chain(wq1, 1, hh4, pr, qt)
                for sc in range(KC):
                    v_chain(wv1, 1, sc)
                return

            # ---- filler: grp1 projections (K/Q per head, then V) ----
            for hh4 in range(4):
                for pr in range(2):
                    filler.append(
                        lambda hh4=hh4, pr=pr: kq_chain(wk1, 1, hh4, pr, kt))
                for pr in range(2):
                    filler.append(
                        lambda hh4=hh4, pr=pr: kq_chain(wq1, 1, hh4, pr, qt))
            for sc in range(KC):
                filler.append(lambda sc=sc: v_chain(wv1, 1, sc))
            n_proj_fill = len(filler)  # 24

            # ---- attention qb0, grp0 heads: consume the proj filler; the
            # ---- V1 leftovers may spill into q0h4's scores section (they
            # ---- still precede its ctx consume in the PE queue) ----
            fa = opts.get("fill_a", 5)
            fb = opts.get("fill_b", 4)
            assert 4 * fa + fb >= n_proj_fill == 24
            for h in range(4):
                attn_block(h, 0, fa)
            attn_block(4, 0, fb)
            assert not filler, "proj filler must drain before q0h4 ctx"

            # ---- attention qb0, rest of grp1: no filler available ----
            for h in range(5, 8):
                attn_block(h, 0, 0)

            if phase != "full":
                for h in range(HH):
                    attn_block(h, 1, 0)
                return

            # ---- out-proj filler for qb0 q-rows (fw-major; wo tiles are
            # ---- re-loaded per qb so the wp tag rotation stays deadlock
            # ---- free) ----
            wo_q0 = [w_tiles(woT_d, fw * 512, f"woA{fw}", nec=OC)
                     for fw in range(4)]
            for fw in range(4):
                for qc in range(4):
                    filler.append(
                        lambda fw=fw, qc=qc: op_chain(wo_q0[fw], fw, qc))
            # qb1's wo tiles: DMA emitted now (tag rotation gates the actual
            # loads on the qb0 out-proj chains) so the tail doesn't stall
            wo_q1 = [w_tiles(woT_d, fw * 512, f"woB{fw}", nec=OC)
                     for fw in range(4)]

            # ---- attention qb1 ----
            for h in range(4):
                attn_block(h, 1, 3)
            for h in range(4, 8):
                attn_block(h, 1, 1)
            take(len(filler))

            # ---- tail: out-proj for qb1 q-rows ----
            for fw in range(4):
                for qc in range(4, 8):
                    op_chain(wo_q1[fw], fw, qc)

        # ================= v1 schedule (previous baseline) =================
        def emit_v1():
            # ---- persistent loads ----
            xt = []
            for ec in range(EC):
                t = sb.tile([P, S], dt.bfloat16, name=f"xt{ec}", tag=f"xt{ec}")
                nc.sync.dma_start(t[:], xT_d[ec * P:(ec + 1) * P, :])
                xt.append(t)
            bqt = sb.tile([P, HH], dt.float32, name="bqt", tag="bqt")
            nc.sync.dma_start(bqt[:], bqT_d[:])
            bo_rep = sb.tile([P, E], dt.float32, name="bo_rep", tag="bo_rep")
            nc.sync.dma_start(bo_rep[:], bo_d.to_broadcast((P, E)))
            ones_col = sb.tile([P, 1], dt.bfloat16, name="ones_col",
                               tag="ones_col")
            nc.vector.memset(ones_col[:], 1.0)

            v_sb = []
            for sc in range(S // P):
                t = sb.tile([P, EH], dt.bfloat16, name=f"v{sc}", tag=f"v{sc}")
                v_sb.append(t)
            ctxT = []
            for h in range(HH):
                t = sb.tile([P, S], dt.bfloat16, name=f"ctxT{h}",
                            tag=f"ctxT{h}")
                ctxT.append(t)

            def load_w_tiles(dram, col0, label, nec=EC):
                tiles = []
                for ec in range(nec):
                    t = wp.tile([P, 512], dt.bfloat16,
                                name=f"{label}{ec}", tag=f"w{ec}")
                    nc.sync.dma_start(
                        t[:], dram[ec * P:(ec + 1) * P, col0:col0 + 512])
                    tiles.append(t)
                return tiles

            kt = {}
            qt = {}
            for grp in range(2):
                wk = load_w_tiles(wkT_d, grp * 512, f"wk{grp}")
                for hh4 in range(4):
                    h = grp * 4 + hh4
                    kth = ktp.tile([P, S], dt.bfloat16, name=f"kt{h}",
                                   tag="kt")
                    hsl = slice(hh4 * P, (hh4 + 1) * P)
                    for pr in range(2):
                        ps = ppp.tile([P, 512], dt.float32, name="kps",
                                      tag="pp")
                        c0 = pr * 512
                        for ec in range(EC):
                            nc.tensor.matmul(
                                ps[:, 0:NB], wk[ec][:, hsl],
                                xt[ec][:, c0:c0 + NB],
                                start=(ec == 0), stop=False)
                            nc.tensor.matmul(
                                ps[:, NB:2 * NB], wk[ec][:, hsl],
                                xt[ec][:, c0 + NB:c0 + 2 * NB],
                                start=False, stop=(ec == EC - 1))
                        nc.vector.tensor_copy(kth[:, c0:c0 + 512], ps[:])
                    kt[h] = kth

                wq = load_w_tiles(wqT_d, grp * 512, f"wq{grp}")
                for hh4 in range(4):
                    h = grp * 4 + hh4
                    qth = qtp.tile([P, S], dt.bfloat16, name=f"qt{h}",
                                   tag="qt")
                    hsl = slice(hh4 * P, (hh4 + 1) * P)
                    for pr in range(2):
                        ps = ppp.tile([P, 512], dt.float32, name="qps",
                                      tag="pp")
                        c0 = pr * 512
                        for ec in range(EC):
                            nc.tensor.matmul(
                                ps[:, 0:NB], wq[ec][:, hsl],
                                xt[ec][:, c0:c0 + NB],
                                start=(ec == 0), stop=False)
                            nc.tensor.matmul(
                                ps[:, NB:2 * NB], wq[ec][:, hsl],
                                xt[ec][:, c0 + NB:c0 + 2 * NB],
                                start=False, stop=(ec == EC - 1))
                        nc.vector.tensor_scalar_add(
                            qth[:, c0:c0 + 512], ps[:], bqt[:, h:h + 1])
                    qt[h] = qth

                fw = grp
                wv = load_w_tiles(wvT_d, fw * 512, f"wv{grp}")
                for sc in range(S // P):
                    ps = ppp.tile([P, 512], dt.float32, name="vps", tag="pp")
                    ssl = slice(sc * P, (sc + 1) * P)
                    for ec in range(EC):
                        nc.tensor.matmul(
                            ps[:, 0:NB], xt[ec][:, ssl], wv[ec][:, 0:NB],
                            start=(ec == 0), stop=False)
                        nc.tensor.matmul(
                            ps[:, NB:2 * NB], xt[ec][:, ssl],
                            wv[ec][:, NB:2 * NB],
                            start=False, stop=(ec == EC - 1))
                    nc.vector.tensor_copy(
                        v_sb[sc][:, fw * 512:(fw + 1) * 512], ps[:])

                for hh4 in range(4 if phase != "proj" else 0):
                    h = grp * 4 + hh4
                    hsl = slice(h * P, (h + 1) * P)
                    for qb in range(S // 512):
                        qA = slice(qb * 512, qb * 512 + NB)
                        qB = slice(qb * 512 + NB, qb * 512 + 2 * NB)
                        cpt = cpp.tile([P, 512], dt.float32,
                                       name=f"ctx{h}_{qb}", tag="cp")
                        ctxA = cpt[:, 0:NB]
                        ctxB = cpt[:, NB:2 * NB]
                        lag = opts.get("pipe_lag", 2)
                        pts = {}

                        def consume(kc):
                            pt = pts[kc]
                            nc.tensor.matmul(
                                ctxA, v_sb[kc][:, hsl], pt[:, 0:NB],
                                start=(kc == 0), stop=False)
                            nc.tensor.matmul(
                                ctxB, v_sb[kc][:, hsl], pt[:, NB:2 * NB],
                                start=False, stop=(kc == KC - 1))

                        for kc in range(KC):
                            ksl = slice(kc * P, (kc + 1) * P)
                            sp = spp.tile([P, 512], dt.float32, name="sp",
                                          tag="sp")
                            nc.tensor.matmul(sp[:, 0:NB], kt[h][:, ksl],
                                             qt[h][:, qA])
                            nc.tensor.matmul(sp[:, NB:2 * NB], kt[h][:, ksl],
                                             qt[h][:, qB])
                            pt = ptp.tile([P, 512], dt.bfloat16, name="pt",
                                          tag="pt")
                            nc.scalar.activation(pt[:], sp[:], AFT.Exp)
                            pts[kc] = pt
                            if kc >= lag:
                                consume(kc - lag)
                        for kc in range(KC - lag, KC):
                            consume(kc)

                        su = upp.tile([1, 512], dt.float32,
                                      name=f"su{h}_{qb}", tag="su")
                        if sums_on == "pe":
                            for kc in range(KC):
                                pt = pts[kc]
                                nc.tensor.matmul(su[:, 0:NB], ones_col[:],
                                                 pt[:, 0:NB],
                                                 start=(kc == 0),
                                                 stop=False)
                                nc.tensor.matmul(su[:, NB:2 * NB],
                                                 ones_col[:],
                                                 pt[:, NB:2 * NB],
                                                 start=False,
                                                 stop=(kc == KC - 1))
                        else:
                            tacc = tp.tile([P, 512], dt.float32, name="tacc",
                                           tag="tacc")
                            nc.vector.tensor_add(tacc[:], pts[0][:],
                                                 pts[1][:])
                            for kc in range(2, KC):
                                nc.vector.tensor_add(tacc[:], tacc[:],
                                                     pts[kc][:])
                            t16 = tp.tile([P, 512], dt.bfloat16, name="t16",
                                          tag="t16")
                            nc.vector.tensor_copy(t16[:], tacc[:])
                            nc.tensor.matmul(su[:, 0:NB], ones_col[:],
                                             t16[:, 0:NB])
                            nc.tensor.matmul(su[:, NB:2 * NB], ones_col[:],
                                             t16[:, NB:2 * NB])
                        r_row = rowp.tile([1, 512], dt.float32, name="r_row",
                                          tag="rrow")
                        nc.vector.reciprocal(r_row[:], su[:])
                        rep = rowp.tile([P, 512], dt.float32, name="rep",
                                        tag="rep")
                        nc.gpsimd.partition_broadcast(rep[:], r_row[:])
                        nc.vector.tensor_mul(
                            ctxT[h][:, qb * 512:(qb + 1) * 512], cpt[:],
                            rep[:])

            for fw in range(4 if phase == "full" else 0):
                wo = load_w_tiles(woT_d, fw * 512, f"wo{fw}", nec=OC)
                for qc in range(S // P):
                    qsl = slice(qc * P, (qc + 1) * P)
                    ps = ppp.tile([P, 512], dt.float32, name="ops", tag="pp")
                    for h in range(HH):
                        nc.tensor.matmul(
                            ps[:, 0:NB], ctxT[h][:, qsl], wo[h][:, 0:NB],
                            start=(h == 0), stop=False)
                        nc.tensor.matmul(
                            ps[:, NB:2 * NB], ctxT[h][:, qsl],
                            wo[h][:, NB:2 * NB],
                            start=False, stop=(h == HH - 1))
                    o0 = outp.tile([P, 512], out_dt, name="o0",
                                   tag="ot")
                    f0 = fw * 512
                    nc.vector.tensor_add(o0[:], ps[:],
                                         bo_rep[:, f0:f0 + 512])
                    nc.sync.dma_start(out_d[qsl, f0:f0 + 512], o0[:])

        emit = emit_v2 if sched == "v2" else emit_v1
        if bench_iters is None:
            emit()
        else:
            with tc.For_i(0, bench_iters, 1):
                emit()

    nc.compile()
    return nc


def _get_program():
    global _PROGRAM
    if _PROGRAM is None:
        _PROGRAM = _build_program()
    return _PROGRAM


def make_in_maps(query, in_proj_weight, in_proj_bias, out_proj_weight,
                 out_proj_bias):
    """Host-side sharding: slice/transpose/cast per core. Pure layout prep."""
    x = np.asarray(query, dtype=np.float32)
    W = np.asarray(in_proj_weight, dtype=np.float32)
    b = np.asarray(in_proj_bias, dtype=np.float32)
    Wo = np.asarray(out_proj_weight, dtype=np.float32)
    bo = np.asarray(out_proj_bias, dtype=np.float32)

    sc = np.float32(1.0 / np.sqrt(D))
    wqT = np.ascontiguousarray((W[:E] * sc).T).astype(BF16)       # [E, E]
    wkT = np.ascontiguousarray(W[E:2 * E].T).astype(BF16)
    wvT = np.ascontiguousarray(W[2 * E:].T).astype(BF16)
    woT = np.ascontiguousarray(Wo.T).astype(BF16)                 # [E, E]
    bq_s = (b[:E] * sc).reshape(H, P)
    bv = b[2 * E:]                                                # [E]

    in_maps = []
    for c in range(NCORES):
        bi, hh = c // 2, c % 2
        esl = slice(hh * EH, (hh + 1) * EH)
        xT = np.ascontiguousarray(x[bi].T).astype(BF16)
        # fold this half's share of Wo@bv into the output bias; add bo
        # itself only on the hh==0 core (partials are summed)
        bo_half = Wo[:, esl] @ bv[esl]
        if hh == 0:
            bo_half = bo_half + bo
        in_maps.append({
            "xT": xT,
            "wqT": np.ascontiguousarray(wqT[:, esl]),
            "wkT": np.ascontiguousarray(wkT[:, esl]),
            "wvT": np.ascontiguousarray(wvT[:, esl]),
            "woT": np.ascontiguousarray(woT[esl, :]),
            "bqT": np.ascontiguousarray(bq_s[hh * HH:(hh + 1) * HH].T),
            "bo": np.ascontiguousarray(bo_half.reshape(1, E)
                                       .astype(np.float32)),
        })
    return in_maps


def assemble_out(results):
    """Gather: sum each batch's two tensor-parallel partial outputs."""
    out = np.empty((B, S, E), dtype=np.float32)
    for bi in range(B):
        out[bi] = (results[2 * bi]["out"].astype(np.float32)
                   + results[2 * bi + 1]["out"].astype(np.float32))
    return out


def kernel(query, in_proj_weight, in_proj_bias, out_proj_weight,
           out_proj_bias):
    from concourse import bass_utils
    nc = _get_program()
    in_maps = make_in_maps(query, in_proj_weight, in_proj_bias,
                           out_proj_weight, out_proj_bias)
    res = bass_utils.run_bass_kernel_spmd(nc, in_maps,
                                          core_ids=list(range(NCORES)))
    return assemble_out(res.results)


# revision 19
# speedup vs baseline: 1.1611x; 1.0044x over previous
"""Trainium2 Bass kernel for fused multi-head attention (CompositeMHA).

Reference computation (B=4, S=1024, E=2048, H=16, D=128), fp32:
    proj = x @ in_proj_weight.T + in_proj_bias        # [B,S,3E]
    q,k,v = split(proj); heads of D=128
    ctx = softmax(q k^T / sqrt(D)) v                   # per (b, head)
    out = ctx @ out_proj_weight.T + out_proj_bias      # [B,S,E]

Sharding (8 cores, no on-device collectives), per the tensor-parallel hint:
data-parallel over the 4 batches x tensor-parallel over head halves.
Core c handles batch c//2 and heads [hh*8, hh*8+8) where hh = c%2 —
sharding the corresponding 3E rows of in_proj_weight and columns of
out_proj_weight.  Each core emits a partial output [S, E]; the gather
step sums each batch's two partials (the TP reduction).

Exact algebraic simplifications (no accuracy cost):
  - K bias dropped: softmax over j of (q+bq)·(k_j+bk) is invariant to
    the j-constant term (q+bq)·bk, so k_j needs no bias.
  - V bias folded into the output bias: sum_j p_j = 1, so
    ctx = ctxU + bv and out = ctxU@Wo^T + (bo + Wo@bv).

Tiling: ALL matmuls use 256-wide moving tensors (~107 ns measured at
full rate; 512-wide costs ~272 ns, so 2x256 wins).  Matmuls are emitted
as pairs sharing each loaded lhsT.

Schedule v2 (default): the PE queue is kept saturated by interleaving
independent projection / out-projection chains ("filler") into the
attention stream, so cross-engine waits (PE->ACT exp->PE) hide behind
queued matmuls instead of stalling the PE head-of-line:
    P0: K0/Q0/V0 projections (first two K head chains ec-outer, paced
        by the interleaved xT+Wk DMA stream to shrink the startup stall)
    blocks qb0 h0..h3   + filler K1/Q1/V1 chains (V1 spills into q0h4)
    blocks qb0 h4..h7   (little filler left -- ACT-paced stretch)
    blocks qb1 h0..h7   + filler out-proj chains for q-rows of qb0
    tail: out-proj chains for qb1 (pure PE)
Softmax sums+broadcast are FUSED into one PE accumulation per block:
rep[128,512] = ones[128x128] @ P^T summed over k-chunks replicates the
per-q denominators on every partition; only reciprocal + multiply run
on DVE.  (GPSIMD partition_all_reduce measured ~7 us/op on real HW --
111 us slower per kernel than this, despite a 0.5 us cost-model price;
keep ucode ops off the critical path.)  PSUM: 2 proj + 3 score + 2 ctx
+ 1 sums banks.

On-core dataflow (bf16 matmuls into fp32 PSUM):
    xT   = X_b^T                      [E, S]
    K^T[h] = Wk^T-chunk^T @ xT        [D, S]   per head (no bias)
    Q^T[h] likewise + bq (DVE)        [D, S]   (1/sqrt(D) folded in)
    V      = xT-chunk^T @ Wv^T        [S, E/2] natural layout (no bias)
    scoresT[k,q] = K^T-chunk^T @ Q^T  -> exp on ACT -> P^T (bf16)
    sums[q]: DVE add-tree over P^T chunks -> GPSIMD all-reduce -> recip
    ctx^T[h] = V-chunk^T @ P^T accumulated; * recip -> bf16
    out_partial = ctx^T-chunk^T @ Wout^T (+ bo') -> fp32
"""

import numpy as np
import ml_dtypes

B, S, E, H = 4, 1024, 2048, 16
D = 128          # head dim == partition size
P = 128
HH = 8           # heads per core (head half)
EH = HH * D      # 1024: e-columns of this half
EC = E // P      # 16 e-chunks (contraction for in-proj)
OC = EH // P     # 8 e-chunks (contraction for out-proj)
KC = S // P      # 8 key chunks
NCORES = 8
NB = 256         # moving-tensor width for all matmuls
BF16 = ml_dtypes.bfloat16

_PROGRAM = None


def _build_program(bench_iters=None, phase="full", opts=None):
    opts = opts or {}
    import concourse.bass as bass  # noqa: F401
    import concourse.tile as tile
    from concourse import bacc, bass_isa, mybir
    from contextlib import ExitStack

    dt = mybir.dt
    AFT = mybir.ActivationFunctionType
    sched = opts.get("sched", "v2")

    nc = bacc.Bacc("TRN2", target_bir_lowering=False, debug=False,
                   num_devices=NCORES)
    out_dt = dt.bfloat16 if opts.get("out_bf16", True) else dt.float32

    xT_d = nc.dram_tensor("xT", [E, S], dt.bfloat16, kind="ExternalInput").ap()
    wqT_d = nc.dram_tensor("wqT", [E, EH], dt.bfloat16, kind="ExternalInput").ap()
    wkT_d = nc.dram_tensor("wkT", [E, EH], dt.bfloat16, kind="ExternalInput").ap()
    wvT_d = nc.dram_tensor("wvT", [E, EH], dt.bfloat16, kind="ExternalInput").ap()
    woT_d = nc.dram_tensor("woT", [EH, E], dt.bfloat16, kind="ExternalInput").ap()
    bqT_d = nc.dram_tensor("bqT", [P, HH], dt.float32, kind="ExternalInput").ap()
    bo_d = nc.dram_tensor("bo", [1, E], dt.float32, kind="ExternalInput").ap()
    out_d = nc.dram_tensor("out", [S, E], out_dt,
                           kind="ExternalOutput").ap()

    sums_on = opts.get("sums_on", "pe")  # v1: "pe" | "dve"

    with tile.TileContext(nc) as tc, ExitStack() as ctx:
        sb = ctx.enter_context(tc.tile_pool(name="persist", bufs=1))
        wp = ctx.enter_context(tc.tile_pool(name="wstream",
                                            bufs=opts.get("wp_bufs", 2)))
        ktp = ctx.enter_context(tc.tile_pool(name="ktp", bufs=8))
        qtp = ctx.enter_context(tc.tile_pool(name="qtp", bufs=8))
        ptp = ctx.enter_context(tc.tile_pool(
            name="ptp", bufs=opts.get("ptp_bufs", 24 if sched == "v1" else 16)))
        outp = ctx.enter_context(tc.tile_pool(
            name="outp", bufs=6 if sched == "v1" else 16))
        rowp = ctx.enter_context(tc.tile_pool(
            name="rowp", bufs=4 if sched == "v1" else 2))
        tp = ctx.enter_context(tc.tile_pool(
            name="tsum", bufs=4 if sched == "v1" else 2))
        if sched == "v1":
            pp_n, sp_n, cp_n, up_n = (opts.get("pp_bufs", 4),
                                      opts.get("sp_bufs", 2),
                                      opts.get("cp_bufs", 1), 1)
        else:
            pp_n, sp_n, cp_n, up_n = (opts.get("pp_bufs", 2),
                                      opts.get("sp_bufs", 3),
                                      opts.get("cp_bufs", 2), 1)
        ppp = ctx.enter_context(tc.tile_pool(name="ppsum", bufs=pp_n,
                                             space="PSUM"))
        spp = ctx.enter_context(tc.tile_pool(name="spsum", bufs=sp_n,
                                             space="PSUM"))
        cpp = ctx.enter_context(tc.tile_pool(name="cpsum", bufs=cp_n,
                                             space="PSUM"))
        if up_n:
            upp = ctx.enter_context(tc.tile_pool(name="upsum", bufs=up_n,
                                                 space="PSUM"))

        # ================= v2 schedule =================
        def emit_v2():
            mm = nc.tensor.matmul

            # ---- persistent tiles ----
            xt = [sb.tile([P, S], dt.bfloat16, name=f"xt{ec}", tag=f"xt{ec}")
                  for ec in range(EC)]
            bqt = sb.tile([P, HH], dt.float32, name="bqt", tag="bqt")
            bo_rep = sb.tile([P, E], dt.float32, name="bo_rep", tag="bo_rep")
            ones128 = sb.tile([P, P], dt.bfloat16, name="ones128",
                              tag="ones128")
            nc.vector.memset(ones128[:], 1.0)
            v_sb = [sb.tile([P, EH], dt.bfloat16, name=f"v{sc}", tag=f"v{sc}")
                    for sc in range(KC)]
            ctxT = [sb.tile([P, S], dt.bfloat16, name=f"ctxT{h}",
                            tag=f"ctxT{h}") for h in range(HH)]

            def w_tiles(dram, col0, label, nec=EC):
                tiles = []
                for ec in range(nec):
                    t = wp.tile([P, 512], dt.bfloat16,
                                name=f"{label}{ec}", tag=f"w{ec}")
                    nc.sync.dma_start(
                        t[:], dram[ec * P:(ec + 1) * P, col0:col0 + 512])
                    tiles.append(t)
                return tiles

            # ---- DMA issue: wk0 and xt interleaved first (startup), then
            # ---- the rest; weight streaming is gated by wp tag rotation.
            wk0 = []
            for ec in range(EC):
                t = wp.tile([P, 512], dt.bfloat16, name=f"wk0_{ec}",
                            tag=f"w{ec}")
                nc.sync.dma_start(t[:], wkT_d[ec * P:(ec + 1) * P, 0:512])
                nc.sync.dma_start(xt[ec][:], xT_d[ec * P:(ec + 1) * P, :])
                wk0.append(t)
            nc.sync.dma_start(bqt[:], bqT_d[:])
            nc.sync.dma_start(bo_rep[:], bo_d.to_broadcast((P, E)))
            wq0 = w_tiles(wqT_d, 0, "wq0")
            wv0 = w_tiles(wvT_d, 0, "wv0")
            wk1 = w_tiles(wkT_d, 512, "wk1")
            wq1 = w_tiles(wqT_d, 512, "wq1")
            wv1 = w_tiles(wvT_d, 512, "wv1")

            kt = {}
            qt = {}

            # During P0 (and the proj-only phase) the score/ctx/sums banks
            # are idle: rotate projection chains across ALL psum pools for
            # 8 banks of copy-drain runway instead of ppp's 2.
            p0_pools = [(ppp, "pp"), (spp, "sp"), (cpp, "cp"), (upp, "rep")]
            p0_idx = [0]

            def chain_psum(pool):
                if pool is None or not opts.get("p0mp", True):
                    return ppp.tile([P, 512], dt.float32, name="chps",
                                    tag="pp")
                pl, tag = p0_pools[p0_idx[0] % len(p0_pools)]
                p0_idx[0] += 1
                return pl.tile([P, 512], dt.float32, name="chps", tag=tag)

            def kq_chain(w, grp, hh4, pr, dst, pool=None):
                """One 512-col block of K^T or Q^T for head grp*4+hh4."""
                hsl = slice(hh4 * P, (hh4 + 1) * P)
                c0 = pr * 512
                ps = chain_psum(pool)
                for ec in range(EC):
                    mm(ps[:, 0:NB], w[ec][:, hsl], xt[ec][:, c0:c0 + NB],
                       start=(ec == 0), stop=False)
                    mm(ps[:, NB:2 * NB], w[ec][:, hsl],
                       xt[ec][:, c0 + NB:c0 + 512],
                       start=False, stop=(ec == EC - 1))
                h = grp * 4 + hh4
                if dst is kt:
                    nc.vector.tensor_copy(kt[h][:, c0:c0 + 512], ps[:])
                else:
                    nc.vector.tensor_scalar_add(
                        qt[h][:, c0:c0 + 512], ps[:], bqt[:, h:h + 1])

            def v_chain(wv, fw, sc, pool=None):
                ps = chain_psum(pool)
                ssl = slice(sc * P, (sc + 1) * P)
                for ec in range(EC):
                    mm(ps[:, 0:NB], xt[ec][:, ssl], wv[ec][:, 0:NB],
                       start=(ec == 0), stop=False)
                    mm(ps[:, NB:2 * NB], xt[ec][:, ssl], wv[ec][:, NB:2 * NB],
                       start=False, stop=(ec == EC - 1))
                nc.vector.tensor_copy(
                    v_sb[sc][:, fw * 512:(fw + 1) * 512], ps[:])

            def op_chain(wo, fw, qc):
                qsl = slice(qc * P, (qc + 1) * P)
                ps = ppp.tile([P, 512], dt.float32, name="ops", tag="pp")
                for h in range(HH):
                    mm(ps[:, 0:NB], ctxT[h][:, qsl], wo[h][:, 0:NB],
                       start=(h == 0), stop=False)
                    mm(ps[:, NB:2 * NB], ctxT[h][:, qsl], wo[h][:, NB:2 * NB],
                       start=False, stop=(h == HH - 1))
                o0 = outp.tile([P, 512], out_dt, name="o0", tag="ot")
                f0 = fw * 512
                nc.vector.tensor_add(o0[:], ps[:], bo_rep[:, f0:f0 + 512])
                nc.sync.dma_start(out_d[qsl, f0:f0 + 512], o0[:])

            # allocate kt/qt tiles in head order (tag rotation = 8 live)
            for h in range(HH):
                kt[h] = ktp.tile([P, S], dt.bfloat16, name=f"kt{h}", tag="kt")
                qt[h] = qtp.tile([P, S], dt.bfloat16, name=f"qt{h}", tag="qt")

            # ---- filler queue ----
            filler = []

            def take(n):
                for _ in range(n):
                    if filler:
                        filler.pop(0)()

            probe = opts.get("probe")  # None|"const_p"|"no_tail"

            def attn_block(h, qb, n_fill):
                hsl = slice(h * P, (h + 1) * P)
                qA = slice(qb * 512, qb * 512 + NB)
                qB = slice(qb * 512 + NB, (qb + 1) * 512)
                nf1 = n_fill // 3
                nf2 = n_fill // 3
                nf3 = n_fill - nf1 - nf2
                cpt = cpp.tile([P, 512], dt.float32, name=f"c{h}_{qb}",
                               tag="cp")
                pts = []
                for kc in range(KC):
                    ksl = slice(kc * P, (kc + 1) * P)
                    sp = spp.tile([P, 512], dt.float32, name="sp", tag="sp")
                    mm(sp[:, 0:NB], kt[h][:, ksl], qt[h][:, qA])
                    mm(sp[:, NB:2 * NB], kt[h][:, ksl], qt[h][:, qB])
                    if probe != "const_p":
                        pt = ptp.tile([P, 512], dt.bfloat16, name="pt",
                                      tag="pt")
                        if opts.get("exp_split"):
                            nc.scalar.activation(pt[:, 0:NB], sp[:, 0:NB],
                                                 AFT.Exp)
                            nc.scalar.activation(pt[:, NB:2 * NB],
                                                 sp[:, NB:2 * NB], AFT.Exp)
                        else:
                            nc.scalar.activation(pt[:], sp[:], AFT.Exp)
                        pts.append(pt)
                    if kc == 2:
                        take(nf1)
                    elif kc == 5:
                        take(nf2)
                take(nf3)
                for kc in range(KC):
                    rhs = (v_sb[kc][:, 0:512] if probe == "const_p"
                           else pts[kc][:])
                    mm(cpt[:, 0:NB], v_sb[kc][:, hsl], rhs[:, 0:NB],
                       start=(kc == 0), stop=False)
                    mm(cpt[:, NB:2 * NB], v_sb[kc][:, hsl],
                       rhs[:, NB:2 * NB],
                       start=False, stop=(kc == KC - 1))
                if probe in ("const_p", "no_tail"):
                    nc.vector.tensor_copy(
                        ctxT[h][:, qb * 512:(qb + 1) * 512], cpt[:])
                    return
                # fused sums+broadcast on PE: rep[p, q] = sum_k P^T[k, q]
                # (ones lhsT replicates the column sums on every partition)
                rep = upp.tile([P, 512], dt.float32, name=f"rep{h}_{qb}",
                               tag="rep")
                for kc in range(KC):
                    mm(rep[:, 0:NB], ones128[:], pts[kc][:, 0:NB],
                       start=(kc == 0), stop=False)
                    mm(rep[:, NB:2 * NB], ones128[:], pts[kc][:, NB:2 * NB],
                       start=False, stop=(kc == KC - 1))
                rrec = rowp.tile([P, 512], dt.float32, name="rrec",
                                 tag="rrec")
                nc.vector.reciprocal(rrec[:], rep[:])
                nc.vector.tensor_mul(
                    ctxT[h][:, qb * 512:(qb + 1) * 512], cpt[:], rrec[:])

            # ---- P0: grp0 projections ----
            # first two K head chains ec-outer, 4 accumulators (2 proj banks
            # + 2 borrowed score banks): paced by the interleaved xt+wk DMA
            # stream, so the startup stall shrinks to the DMA/PE rate gap.
            accs = [ppp.tile([P, 512], dt.float32, name="k00", tag="pp"),
                    ppp.tile([P, 512], dt.float32, name="k01", tag="pp"),
                    spp.tile([P, 512], dt.float32, name="k10", tag="sp"),
                    spp.tile([P, 512], dt.float32, name="k11", tag="sp")]
            for ec in range(EC):
                for i, ps in enumerate(accs):
                    hh4, pr = divmod(i, 2)
                    hsl = slice(hh4 * P, (hh4 + 1) * P)
                    c0 = pr * 512
                    mm(ps[:, 0:NB], wk0[ec][:, hsl], xt[ec][:, c0:c0 + NB],
                       start=(ec == 0), stop=False)
                    mm(ps[:, NB:2 * NB], wk0[ec][:, hsl],
                       xt[ec][:, c0 + NB:c0 + 512],
                       start=False, stop=(ec == EC - 1))
            for i, ps in enumerate(accs):
                hh4, pr = divmod(i, 2)
                nc.vector.tensor_copy(kt[hh4][:, pr * 512:(pr + 1) * 512],
                                      ps[:])
            for hh4 in range(2, 4):
                for pr in range(2):
                    kq_chain(wk0, 0, hh4, pr, kt, pool="p0")
            for hh4 in range(4):
                for pr in range(2):
                    kq_chain(wq0, 0, hh4, pr, qt, pool="p0")
            for sc in range(KC):
                v_chain(wv0, 0, sc, pool="p0")

            if phase == "proj":
                for hh4 in range(4):
                    for pr in range(2):
                        kq_chain(wk1, 1, hh4, pr, kt, pool="p0")
                        kq_chain(wq1, 1, hh4, pr, qt, pool="p0")
                for sc in range(KC):
                    v_chain(wv1, 1, sc, pool="p0")
                return

            # ---- filler: grp1 projections (K/Q per head, then V) ----
            for hh4 in range(4):
                for pr in range(2):
                    filler.append(
                        lambda hh4=hh4, pr=pr: kq_chain(wk1, 1, hh4, pr, kt))
                for pr in range(2):
                    filler.append(
                        lambda hh4=hh4, pr=pr: kq_chain(wq1, 1, hh4, pr, qt))
            for sc in range(KC):
                filler.append(lambda sc=sc: v_chain(wv1, 1, sc))
            n_proj_fill = len(filler)  # 24

            # ---- attention qb0, grp0 heads: consume the proj filler; the
            # ---- V1 leftovers may spill into q0h4's scores section (they
            # ---- still precede its ctx consume in the PE queue) ----
            fa = opts.get("fill_a", 5)
            fb = opts.get("fill_b", 4)
            assert 4 * fa + fb >= n_proj_fill == 24
            for h in range(4):
                attn_block(h, 0, fa)
            attn_block(4, 0, fb)
            assert not filler, "proj filler must drain before q0h4 ctx"

            # ---- attention qb0, rest of grp1: no filler available ----
            for h in range(5, 8):
                attn_block(h, 0, 0)

            if phase != "full":
                for h in range(HH):
                    attn_block(h, 1, 0)
                return

            # ---- out-proj filler for qb0 q-rows (fw-major; wo tiles are
            # ---- re-loaded per qb so the wp tag rotation stays deadlock
            # ---- free) ----
            wo_q0 = [w_tiles(woT_d, fw * 512, f"woA{fw}", nec=OC)
                     for fw in range(4)]
            for fw in range(4):
                for qc in range(4):
                    filler.append(
                        lambda fw=fw, qc=qc: op_chain(wo_q0[fw], fw, qc))
            # qb1's wo tiles: DMA emitted now (tag rotation gates the actual
            # loads on the qb0 out-proj chains) so the tail doesn't stall
            wo_q1 = [w_tiles(woT_d, fw * 512, f"woB{fw}", nec=OC)
                     for fw in range(4)]

            # ---- attention qb1 ----
            fc = opts.get("fill_c", 2)
            for h in range(4):
                attn_block(h, 1, fc)
            for h in range(4, 8):
                attn_block(h, 1, (16 - 4 * fc) // 4)
            take(len(filler))

            # ---- tail: out-proj for qb1 q-rows ----
            for fw in range(4):
                for qc in range(4, 8):
                    op_chain(wo_q1[fw], fw, qc)

        # ================= v1 schedule (previous baseline) =================
        def emit_v1():
            # ---- persistent loads ----
            xt = []
            for ec in range(EC):
                t = sb.tile([P, S], dt.bfloat16, name=f"xt{ec}", tag=f"xt{ec}")
                nc.sync.dma_start(t[:], xT_d[ec * P:(ec + 1) * P, :])
                xt.append(t)
            bqt = sb.tile([P, HH], dt.float32, name="bqt", tag="bqt")
            nc.sync.dma_start(bqt[:], bqT_d[:])
            bo_rep = sb.tile([P, E], dt.float32, name="bo_rep", tag="bo_rep")
            nc.sync.dma_start(bo_rep[:], bo_d.to_broadcast((P, E)))
            ones_col = sb.tile([P, 1], dt.bfloat16, name="ones_col",
                               tag="ones_col")
            nc.vector.memset(ones_col[:], 1.0)

            v_sb = []
            for sc in range(S // P):
                t = sb.tile([P, EH], dt.bfloat16, name=f"v{sc}", tag=f"v{sc}")
                v_sb.append(t)
            ctxT = []
            for h in range(HH):
                t = sb.tile([P, S], dt.bfloat16, name=f"ctxT{h}",
                            tag=f"ctxT{h}")
                ctxT.append(t)

            def load_w_tiles(dram, col0, label, nec=EC):
                tiles = []
                for ec in range(nec):
                    t = wp.tile([P, 512], dt.bfloat16,
                                name=f"{label}{ec}", tag=f"w{ec}")
                    nc.sync.dma_start(
                        t[:], dram[ec * P:(ec + 1) * P, col0:col0 + 512])
                    tiles.append(t)
                return tiles

            kt = {}
            qt = {}
            for grp in range(2):
                wk = load_w_tiles(wkT_d, grp * 512, f"wk{grp}")
                for hh4 in range(4):
                    h = grp * 4 + hh4
                    kth = ktp.tile([P, S], dt.bfloat16, name=f"kt{h}",
                                   tag="kt")
                    hsl = slice(hh4 * P, (hh4 + 1) * P)
                    for pr in range(2):
                        ps = ppp.tile([P, 512], dt.float32, name="kps",
                                      tag="pp")
                        c0 = pr * 512
                        for ec in range(EC):
                            nc.tensor.matmul(
                                ps[:, 0:NB], wk[ec][:, hsl],
                                xt[ec][:, c0:c0 + NB],
                                start=(ec == 0), stop=False)
                            nc.tensor.matmul(
                                ps[:, NB:2 * NB], wk[ec][:, hsl],
                                xt[ec][:, c0 + NB:c0 + 2 * NB],
                                start=False, stop=(ec == EC - 1))
                        nc.vector.tensor_copy(kth[:, c0:c0 + 512], ps[:])
                    kt[h] = kth

                wq = load_w_tiles(wqT_d, grp * 512, f"wq{grp}")
                for hh4 in range(4):
                    h = grp * 4 + hh4
                    qth = qtp.tile([P, S], dt.bfloat16, name=f"qt{h}",
                                   tag="qt")
                    hsl = slice(hh4 * P, (hh4 + 1) * P)
                    for pr in range(2):
                        ps = ppp.tile([P, 512], dt.float32, name="qps",
                                      tag="pp")
                        c0 = pr * 512
                        for ec in range(EC):
                            nc.tensor.matmul(
                                ps[:, 0:NB], wq[ec][:, hsl],
                                xt[ec][:, c0:c0 + NB],
                                start=(ec == 0), stop=False)
                            nc.tensor.matmul(
                                ps[:, NB:2 * NB], wq[ec][:, hsl],
                                xt[ec][:, c0 + NB:c0 + 2 * NB],
                                start=False, stop=(ec == EC - 1))
                        nc.vector.tensor_scalar_add(
                            qth[:, c0:c0 + 512], ps[:], bqt[:, h:h + 1])
                    qt[h] = qth

                fw = grp
                wv = load_w_tiles(wvT_d, fw * 512, f"wv{grp}")
                for sc in range(S // P):
                    ps = ppp.tile([P, 512], dt.float32, name="vps", tag="pp")
                    ssl = slice(sc * P, (sc + 1) * P)
                    for ec in range(EC):
                        nc.tensor.matmul(
                            ps[:, 0:NB], xt[ec][:, ssl], wv[ec][:, 0:NB],
                            start=(ec == 0), stop=False)
                        nc.tensor.matmul(
                            ps[:, NB:2 * NB], xt[ec][:, ssl],
                            wv[ec][:, NB:2 * NB],
                            start=False, stop=(ec == EC - 1))
                    nc.vector.tensor_copy(
                        v_sb[sc][:, fw * 512:(fw + 1) * 512], ps[:])

                for hh4 in range(4 if phase != "proj" else 0):
                    h = grp * 4 + hh4
                    hsl = slice(h * P, (h + 1) * P)
                    for qb in range(S // 512):
                        qA = slice(qb * 512, qb * 512 + NB)
                        qB = slice(qb * 512 + NB, qb * 512 + 2 * NB)
                        cpt = cpp.tile([P, 512], dt.float32,
                                       name=f"ctx{h}_{qb}", tag="cp")
                        ctxA = cpt[:, 0:NB]
                        ctxB = cpt[:, NB:2 * NB]
                        lag = opts.get("pipe_lag", 2)
                        pts = {}

                        def consume(kc):
                            pt = pts[kc]
                            nc.tensor.matmul(
                                ctxA, v_sb[kc][:, hsl], pt[:, 0:NB],
                                start=(kc == 0), stop=False)
                            nc.tensor.matmul(
                                ctxB, v_sb[kc][:, hsl], pt[:, NB:2 * NB],
                                start=False, stop=(kc == KC - 1))

                        for kc in range(KC):
                            ksl = slice(kc * P, (kc + 1) * P)
                            sp = spp.tile([P, 512], dt.float32, name="sp",
                                          tag="sp")
                            nc.tensor.matmul(sp[:, 0:NB], kt[h][:, ksl],
                                             qt[h][:, qA])
                            nc.tensor.matmul(sp[:, NB:2 * NB], kt[h][:, ksl],
                                             qt[h][:, qB])
                            pt = ptp.tile([P, 512], dt.bfloat16, name="pt",
                                          tag="pt")
                            nc.scalar.activation(pt[:], sp[:], AFT.Exp)
                            pts[kc] = pt
                            if kc >= lag:
                                consume(kc - lag)
                        for kc in range(KC - lag, KC):
                            consume(kc)

                        su = upp.tile([1, 512], dt.float32,
                                      name=f"su{h}_{qb}", tag="su")
                        if sums_on == "pe":
                            for kc in range(KC):
                                pt = pts[kc]
                                nc.tensor.matmul(su[:, 0:NB], ones_col[:],
                                                 pt[:, 0:NB],
                                                 start=(kc == 0),
                                                 stop=False)
                                nc.tensor.matmul(su[:, NB:2 * NB],
                                                 ones_col[:],
                                                 pt[:, NB:2 * NB],
                                                 start=False,
                                                 stop=(kc == KC - 1))
                        else:
                            tacc = tp.tile([P, 512], dt.float32, name="tacc",
                                           tag="tacc")
                            nc.vector.tensor_add(tacc[:], pts[0][:],
                                                 pts[1][:])
                            for kc in range(2, KC):
                                nc.vector.tensor_add(tacc[:], tacc[:],
                                                     pts[kc][:])
                            t16 = tp.tile([P, 512], dt.bfloat16, name="t16",
                                          tag="t16")
                            nc.vector.tensor_copy(t16[:], tacc[:])
                            nc.tensor.matmul(su[:, 0:NB], ones_col[:],
                                             t16[:, 0:NB])
                            nc.tensor.matmul(su[:, NB:2 * NB], ones_col[:],
                                             t16[:, NB:2 * NB])
                        r_row = rowp.tile([1, 512], dt.float32, name="r_row",
                                          tag="rrow")
                        nc.vector.reciprocal(r_row[:], su[:])
                        rep = rowp.tile([P, 512], dt.float32, name="rep",
                                        tag="rep")
                        nc.gpsimd.partition_broadcast(rep[:], r_row[:])
                        nc.vector.tensor_mul(
                            ctxT[h][:, qb * 512:(qb + 1) * 512], cpt[:],
                            rep[:])

            for fw in range(4 if phase == "full" else 0):
                wo = load_w_tiles(woT_d, fw * 512, f"wo{fw}", nec=OC)
                for qc in range(S // P):
                    qsl = slice(qc * P, (qc + 1) * P)
                    ps = ppp.tile([P, 512], dt.float32, name="ops", tag="pp")
                    for h in range(HH):
                        nc.tensor.matmul(
                            ps[:, 0:NB], ctxT[h][:, qsl], wo[h][:, 0:NB],
                            start=(h == 0), stop=False)
                        nc.tensor.matmul(
                            ps[:, NB:2 * NB], ctxT[h][:, qsl],
                            wo[h][:, NB:2 * NB],
                            start=False, stop=(h == HH - 1))
                    o0 = outp.tile([P, 512], out_dt, name="o0",
                                   tag="ot")
                    f0 = fw * 512
                    nc.vector.tensor_add(o0[:], ps[:],
                                         bo_rep[:, f0:f0 + 512])
                    nc.sync.dma_start(out_d[qsl, f0:f0 + 512], o0[:])

        emit = emit_v2 if sched == "v2" else emit_v1
        if bench_iters is None:
            emit()
        else:
            with tc.For_i(0, bench_iters, 1):
                emit()

    nc.compile()
    return nc


def _get_program():
    global _PROGRAM
    if _PROGRAM is None:
        _PROGRAM = _build_program()
    return _PROGRAM


def make_in_maps(query, in_proj_weight, in_proj_bias, out_proj_weight,
                 out_proj_bias):
    """Host-side sharding: slice/transpose/cast per core. Pure layout prep."""
    x = np.asarray(query, dtype=np.float32)
    W = np.asarray(in_proj_weight, dtype=np.float32)
    b = np.asarray(in_proj_bias, dtype=np.float32)
    Wo = np.asarray(out_proj_weight, dtype=np.float32)
    bo = np.asarray(out_proj_bias, dtype=np.float32)

    sc = np.float32(1.0 / np.sqrt(D))
    wqT = np.ascontiguousarray((W[:E] * sc).T).astype(BF16)       # [E, E]
    wkT = np.ascontiguousarray(W[E:2 * E].T).astype(BF16)
    wvT = np.ascontiguousarray(W[2 * E:].T).astype(BF16)
    woT = np.ascontiguousarray(Wo.T).astype(BF16)                 # [E, E]
    bq_s = (b[:E] * sc).reshape(H, P)
    bv = b[2 * E:]                                                # [E]

    in_maps = []
    for c in range(NCORES):
        bi, hh = c // 2, c % 2
        esl = slice(hh * EH, (hh + 1) * EH)
        xT = np.ascontiguousarray(x[bi].T).astype(BF16)
        # fold this half's share of Wo@bv into the output bias; add bo
        # itself only on the hh==0 core (partials are summed)
        bo_half = Wo[:, esl] @ bv[esl]
        if hh == 0:
            bo_half = bo_half + bo
        in_maps.append({
            "xT": xT,
            "wqT": np.ascontiguousarray(wqT[:, esl]),
            "wkT": np.ascontiguousarray(wkT[:, esl]),
            "wvT": np.ascontiguousarray(wvT[:, esl]),
            "woT": np.ascontiguousarray(woT[esl, :]),
            "bqT": np.ascontiguousarray(bq_s[hh * HH:(hh + 1) * HH].T),
            "bo": np.ascontiguousarray(bo_half.reshape(1, E)
                                       .astype(np.float32)),
        })
    return in_maps


def assemble_out(results):
    """Gather: sum each batch's two tensor-parallel partial outputs."""
    out = np.empty((B, S, E), dtype=np.float32)
    for bi in range(B):
        out[bi] = (results[2 * bi]["out"].astype(np.float32)
                   + results[2 * bi + 1]["out"].astype(np.float32))
    return out


def kernel(query, in_proj_weight, in_proj_bias, out_proj_weight,
           out_proj_bias):
    from concourse import bass_utils
    nc = _get_program()
    in_maps = make_in_maps(query, in_proj_weight, in_proj_bias,
                           out_proj_weight, out_proj_bias)
    res = bass_utils.run_bass_kernel_spmd(nc, in_maps,
                                          core_ids=list(range(NCORES)))
    return assemble_out(res.results)


# revision 21
# speedup vs baseline: 1.1921x; 1.0266x over previous
"""Trainium2 Bass kernel for fused multi-head attention (CompositeMHA).

Reference computation (B=4, S=1024, E=2048, H=16, D=128), fp32:
    proj = x @ in_proj_weight.T + in_proj_bias        # [B,S,3E]
    q,k,v = split(proj); heads of D=128
    ctx = softmax(q k^T / sqrt(D)) v                   # per (b, head)
    out = ctx @ out_proj_weight.T + out_proj_bias      # [B,S,E]

Sharding (8 cores, no on-device collectives), per the tensor-parallel hint:
data-parallel over the 4 batches x tensor-parallel over head halves.
Core c handles batch c//2 and heads [hh*8, hh*8+8) where hh = c%2 —
sharding the corresponding 3E rows of in_proj_weight and columns of
out_proj_weight.  Each core emits a partial output [S, E]; the gather
step sums each batch's two partials (the TP reduction).

Exact algebraic simplifications (no accuracy cost):
  - K bias dropped: softmax over j of (q+bq)·(k_j+bk) is invariant to
    the j-constant term (q+bq)·bk, so k_j needs no bias.
  - V bias folded into the output bias: sum_j p_j = 1, so
    ctx = ctxU + bv and out = ctxU@Wo^T + (bo + Wo@bv).

Tiling: ALL matmuls use 256-wide moving tensors (~107 ns measured at
full rate; 512-wide costs ~272 ns, so 2x256 wins).  Matmuls are emitted
as pairs sharing each loaded lhsT.

Schedule v2 (default): the PE queue is kept saturated by interleaving
independent projection / out-projection chains ("filler") into the
attention stream, so cross-engine waits (PE->ACT exp->PE) hide behind
queued matmuls instead of stalling the PE head-of-line:
    P0: K0/Q0/V0 projections (first two K head chains ec-outer, paced
        by the interleaved xT+Wk DMA stream to shrink the startup stall)
    blocks qb0 h0..h3   + filler K1/Q1/V1 chains (V1 spills into q0h4)
    blocks qb0 h4..h7   (little filler left -- ACT-paced stretch)
    blocks qb1 h0..h7   + filler out-proj chains for q-rows of qb0
    tail: out-proj chains for qb1 (pure PE)
Softmax sums+broadcast are FUSED into one PE accumulation per block:
rep[128,512] = ones[128x128] @ P^T summed over k-chunks replicates the
per-q denominators on every partition; only reciprocal + multiply run
on DVE.  (GPSIMD partition_all_reduce measured ~7 us/op on real HW --
111 us slower per kernel than this, despite a 0.5 us cost-model price;
keep ucode ops off the critical path.)  PSUM: 2 proj + 3 score + 2 ctx
+ 1 sums banks.

On-core dataflow (bf16 matmuls into fp32 PSUM):
    xT   = X_b^T                      [E, S]
    K^T[h] = Wk^T-chunk^T @ xT        [D, S]   per head (no bias)
    Q^T[h] likewise + bq (DVE)        [D, S]   (1/sqrt(D) folded in)
    V      = xT-chunk^T @ Wv^T        [S, E/2] natural layout (no bias)
    scoresT[k,q] = K^T-chunk^T @ Q^T  -> exp on ACT -> P^T (bf16)
    sums[q]: DVE add-tree over P^T chunks -> GPSIMD all-reduce -> recip
    ctx^T[h] = V-chunk^T @ P^T accumulated; * recip -> bf16
    out_partial = ctx^T-chunk^T @ Wout^T (+ bo') -> fp32
"""

import numpy as np
import ml_dtypes

B, S, E, H = 4, 1024, 2048, 16
D = 128          # head dim == partition size
P = 128
HH = 8           # heads per core (head half)
EH = HH * D      # 1024: e-columns of this half
EC = E // P      # 16 e-chunks (contraction for in-proj)
OC = EH // P     # 8 e-chunks (contraction for out-proj)
KC = S // P      # 8 key chunks
NCORES = 8
NB = 256         # moving-tensor width for all matmuls
BF16 = ml_dtypes.bfloat16

_PROGRAM = None


def _build_program(bench_iters=None, phase="full", opts=None):
    opts = opts or {}
    import concourse.bass as bass  # noqa: F401
    import concourse.tile as tile
    from concourse import bacc, bass_isa, mybir
    from contextlib import ExitStack

    dt = mybir.dt
    AFT = mybir.ActivationFunctionType
    sched = opts.get("sched", "v2")

    nc = bacc.Bacc("TRN2", target_bir_lowering=False, debug=False,
                   num_devices=NCORES)
    out_dt = dt.bfloat16 if opts.get("out_bf16", True) else dt.float32

    xT_d = nc.dram_tensor("xT", [E, S], dt.bfloat16, kind="ExternalInput").ap()
    wqT_d = nc.dram_tensor("wqT", [E, EH], dt.bfloat16, kind="ExternalInput").ap()
    wkT_d = nc.dram_tensor("wkT", [E, EH], dt.bfloat16, kind="ExternalInput").ap()
    wvT_d = nc.dram_tensor("wvT", [E, EH], dt.bfloat16, kind="ExternalInput").ap()
    woT_d = nc.dram_tensor("woT", [EH, E], dt.bfloat16, kind="ExternalInput").ap()
    bqT_d = nc.dram_tensor("bqT", [P, HH], dt.float32, kind="ExternalInput").ap()
    bo_d = nc.dram_tensor("bo", [1, E], dt.float32, kind="ExternalInput").ap()
    out_d = nc.dram_tensor("out", [S, E], out_dt,
                           kind="ExternalOutput").ap()

    sums_on = opts.get("sums_on", "pe")  # v1: "pe" | "dve"

    with tile.TileContext(nc) as tc, ExitStack() as ctx:
        sb = ctx.enter_context(tc.tile_pool(name="persist", bufs=1))
        wp = ctx.enter_context(tc.tile_pool(name="wstream",
                                            bufs=opts.get("wp_bufs", 2)))
        ktp = ctx.enter_context(tc.tile_pool(name="ktp", bufs=8))
        qtp = ctx.enter_context(tc.tile_pool(name="qtp", bufs=8))
        ptp = ctx.enter_context(tc.tile_pool(
            name="ptp", bufs=opts.get("ptp_bufs", 24 if sched == "v1" else 16)))
        outp = ctx.enter_context(tc.tile_pool(
            name="outp", bufs=6 if sched == "v1" else 16))
        rowp = ctx.enter_context(tc.tile_pool(
            name="rowp", bufs=4 if sched == "v1" else 2))
        tp = ctx.enter_context(tc.tile_pool(
            name="tsum", bufs=4 if sched == "v1" else 2))
        if sched == "v1":
            pp_n, sp_n, cp_n, up_n = (opts.get("pp_bufs", 4),
                                      opts.get("sp_bufs", 2),
                                      opts.get("cp_bufs", 1), 1)
        else:
            pp_n, sp_n, cp_n, up_n = (opts.get("pp_bufs", 2),
                                      opts.get("sp_bufs", 3),
                                      opts.get("cp_bufs", 2), 1)
        ppp = ctx.enter_context(tc.tile_pool(name="ppsum", bufs=pp_n,
                                             space="PSUM"))
        spp = ctx.enter_context(tc.tile_pool(name="spsum", bufs=sp_n,
                                             space="PSUM"))
        cpp = ctx.enter_context(tc.tile_pool(name="cpsum", bufs=cp_n,
                                             space="PSUM"))
        if up_n:
            upp = ctx.enter_context(tc.tile_pool(name="upsum", bufs=up_n,
                                                 space="PSUM"))

        # ================= v2 schedule =================
        def emit_v2():
            mm = nc.tensor.matmul

            # ---- persistent tiles ----
            xt = [sb.tile([P, S], dt.bfloat16, name=f"xt{ec}", tag=f"xt{ec}")
                  for ec in range(EC)]
            bqt = sb.tile([P, HH], dt.float32, name="bqt", tag="bqt")
            bo_rep = sb.tile([P, E], dt.float32, name="bo_rep", tag="bo_rep")
            ones128 = sb.tile([P, P], dt.bfloat16, name="ones128",
                              tag="ones128")
            nc.vector.memset(ones128[:], 1.0)
            v_sb = [sb.tile([P, EH], dt.bfloat16, name=f"v{sc}", tag=f"v{sc}")
                    for sc in range(KC)]
            ctxT = [sb.tile([P, S], dt.bfloat16, name=f"ctxT{h}",
                            tag=f"ctxT{h}") for h in range(HH)]

            def w_tiles(dram, col0, label, nec=EC):
                tiles = []
                for ec in range(nec):
                    t = wp.tile([P, 512], dt.bfloat16,
                                name=f"{label}{ec}", tag=f"w{ec}")
                    nc.sync.dma_start(
                        t[:], dram[ec * P:(ec + 1) * P, col0:col0 + 512])
                    tiles.append(t)
                return tiles

            # ---- DMA issue: wk0 and xt interleaved first (startup), then
            # ---- the rest; weight streaming is gated by wp tag rotation.
            wk0 = []
            for ec in range(EC):
                t = wp.tile([P, 512], dt.bfloat16, name=f"wk0_{ec}",
                            tag=f"w{ec}")
                nc.sync.dma_start(t[:], wkT_d[ec * P:(ec + 1) * P, 0:512])
                nc.sync.dma_start(xt[ec][:], xT_d[ec * P:(ec + 1) * P, :])
                wk0.append(t)
            nc.sync.dma_start(bqt[:], bqT_d[:])
            nc.sync.dma_start(bo_rep[:], bo_d.to_broadcast((P, E)))
            wq0 = w_tiles(wqT_d, 0, "wq0")
            wv0 = w_tiles(wvT_d, 0, "wv0")
            wk1 = w_tiles(wkT_d, 512, "wk1")
            wq1 = w_tiles(wqT_d, 512, "wq1")
            wv1 = w_tiles(wvT_d, 512, "wv1")

            kt = {}
            qt = {}

            # During P0 (and the proj-only phase) the score/ctx/sums banks
            # are idle: rotate projection chains across ALL psum pools for
            # 8 banks of copy-drain runway instead of ppp's 2.
            p0_pools = [(ppp, "pp"), (spp, "sp"), (cpp, "cp"), (upp, "rep")]
            p0_idx = [0]

            def chain_psum(pool):
                if pool is None or not opts.get("p0mp", True):
                    return ppp.tile([P, 512], dt.float32, name="chps",
                                    tag="pp")
                pl, tag = p0_pools[p0_idx[0] % len(p0_pools)]
                p0_idx[0] += 1
                return pl.tile([P, 512], dt.float32, name="chps", tag=tag)

            def kq_chain(w, grp, hh4, pr, dst, pool=None):
                """One 512-col block of K^T or Q^T for head grp*4+hh4."""
                hsl = slice(hh4 * P, (hh4 + 1) * P)
                c0 = pr * 512
                ps = chain_psum(pool)
                for ec in range(EC):
                    mm(ps[:, 0:NB], w[ec][:, hsl], xt[ec][:, c0:c0 + NB],
                       start=(ec == 0), stop=False)
                    mm(ps[:, NB:2 * NB], w[ec][:, hsl],
                       xt[ec][:, c0 + NB:c0 + 512],
                       start=False, stop=(ec == EC - 1))
                h = grp * 4 + hh4
                if dst is kt:
                    nc.vector.tensor_copy(kt[h][:, c0:c0 + 512], ps[:])
                else:
                    nc.vector.tensor_scalar_add(
                        qt[h][:, c0:c0 + 512], ps[:], bqt[:, h:h + 1])

            def v_chain(wv, fw, sc, pool=None):
                ps = chain_psum(pool)
                ssl = slice(sc * P, (sc + 1) * P)
                for ec in range(EC):
                    mm(ps[:, 0:NB], xt[ec][:, ssl], wv[ec][:, 0:NB],
                       start=(ec == 0), stop=False)
                    mm(ps[:, NB:2 * NB], xt[ec][:, ssl], wv[ec][:, NB:2 * NB],
                       start=False, stop=(ec == EC - 1))
                nc.vector.tensor_copy(
                    v_sb[sc][:, fw * 512:(fw + 1) * 512], ps[:])

            def op_chain(wo, fw, qc):
                qsl = slice(qc * P, (qc + 1) * P)
                ps = ppp.tile([P, 512], dt.float32, name="ops", tag="pp")
                for h in range(HH):
                    mm(ps[:, 0:NB], ctxT[h][:, qsl], wo[h][:, 0:NB],
                       start=(h == 0), stop=False)
                    mm(ps[:, NB:2 * NB], ctxT[h][:, qsl], wo[h][:, NB:2 * NB],
                       start=False, stop=(h == HH - 1))
                o0 = outp.tile([P, 512], out_dt, name="o0", tag="ot")
                f0 = fw * 512
                nc.vector.tensor_add(o0[:], ps[:], bo_rep[:, f0:f0 + 512])
                nc.sync.dma_start(out_d[qsl, f0:f0 + 512], o0[:])

            # allocate kt/qt tiles in head order (tag rotation = 8 live)
            for h in range(HH):
                kt[h] = ktp.tile([P, S], dt.bfloat16, name=f"kt{h}", tag="kt")
                qt[h] = qtp.tile([P, S], dt.bfloat16, name=f"qt{h}", tag="qt")

            # ---- filler queue ----
            filler = []

            def take(n):
                for _ in range(n):
                    if filler:
                        filler.pop(0)()

            probe = opts.get("probe")  # None|"const_p"|"no_tail"

            def attn_block(h, qb, n_fill):
                hsl = slice(h * P, (h + 1) * P)
                qA = slice(qb * 512, qb * 512 + NB)
                qB = slice(qb * 512 + NB, (qb + 1) * 512)
                nf1 = n_fill // 3
                nf2 = n_fill // 3
                nf3 = n_fill - nf1 - nf2
                # DVE-tree sums only in filler-filled blocks: there DVE has
                # slack and this trims 14 PE sum-matmuls to 2; in starved
                # blocks DVE (10 ops ~6us) would out-pace PE, so keep PE sums
                dsum_mode = opts.get("dsum", "fill")
                dsum = (dsum_mode == "all"
                        or (dsum_mode == "fill" and n_fill > 0))
                cpt = cpp.tile([P, 512], dt.float32, name=f"c{h}_{qb}",
                               tag="cp")
                pts = []
                acc = None
                for kc in range(KC):
                    ksl = slice(kc * P, (kc + 1) * P)
                    sp = spp.tile([P, 512], dt.float32, name="sp", tag="sp")
                    mm(sp[:, 0:NB], kt[h][:, ksl], qt[h][:, qA])
                    mm(sp[:, NB:2 * NB], kt[h][:, ksl], qt[h][:, qB])
                    if probe != "const_p":
                        pt = ptp.tile([P, 512], dt.bfloat16, name="pt",
                                      tag="pt")
                        if opts.get("exp_split"):
                            nc.scalar.activation(pt[:, 0:NB], sp[:, 0:NB],
                                                 AFT.Exp)
                            nc.scalar.activation(pt[:, NB:2 * NB],
                                                 sp[:, NB:2 * NB], AFT.Exp)
                        else:
                            nc.scalar.activation(pt[:], sp[:], AFT.Exp)
                        pts.append(pt)
                        if dsum and probe is None:
                            if kc == 1:
                                acc = tp.tile([P, 512], dt.float32,
                                              name="acc", tag="acc")
                                nc.vector.tensor_add(acc[:], pts[0][:],
                                                     pts[1][:])
                            elif kc > 1:
                                nc.vector.tensor_add(acc[:], acc[:], pt[:])
                    if kc == 2:
                        take(nf1)
                    elif kc == 5:
                        take(nf2)
                take(nf3)
                for kc in range(KC):
                    rhs = (v_sb[kc][:, 0:512] if probe == "const_p"
                           else pts[kc][:])
                    mm(cpt[:, 0:NB], v_sb[kc][:, hsl], rhs[:, 0:NB],
                       start=(kc == 0), stop=False)
                    mm(cpt[:, NB:2 * NB], v_sb[kc][:, hsl],
                       rhs[:, NB:2 * NB],
                       start=False, stop=(kc == KC - 1))
                if probe in ("const_p", "no_tail"):
                    nc.vector.tensor_copy(
                        ctxT[h][:, qb * 512:(qb + 1) * 512], cpt[:])
                    return
                # fused sums+broadcast on PE: rep[p, q] = sum_k P^T[k, q]
                # (ones lhsT replicates the column sums on every partition)
                rep = upp.tile([P, 512], dt.float32, name=f"rep{h}_{qb}",
                               tag="rep")
                if dsum:
                    t16 = tp.tile([P, 512], dt.bfloat16, name="t16",
                                  tag="t16")
                    nc.vector.tensor_copy(t16[:], acc[:])
                    mm(rep[:, 0:NB], ones128[:], t16[:, 0:NB])
                    mm(rep[:, NB:2 * NB], ones128[:], t16[:, NB:2 * NB])
                else:
                    for kc in range(KC):
                        mm(rep[:, 0:NB], ones128[:], pts[kc][:, 0:NB],
                           start=(kc == 0), stop=False)
                        mm(rep[:, NB:2 * NB], ones128[:],
                           pts[kc][:, NB:2 * NB],
                           start=False, stop=(kc == KC - 1))
                rrec = rowp.tile([P, 512], dt.float32, name="rrec",
                                 tag="rrec")
                nc.vector.reciprocal(rrec[:], rep[:])
                nc.vector.tensor_mul(
                    ctxT[h][:, qb * 512:(qb + 1) * 512], cpt[:], rrec[:])

            # ---- P0: grp0 projections ----
            # first two K head chains ec-outer, 4 accumulators (2 proj banks
            # + 2 borrowed score banks): paced by the interleaved xt+wk DMA
            # stream, so the startup stall shrinks to the DMA/PE rate gap.
            accs = [ppp.tile([P, 512], dt.float32, name="k00", tag="pp"),
                    ppp.tile([P, 512], dt.float32, name="k01", tag="pp"),
                    spp.tile([P, 512], dt.float32, name="k10", tag="sp"),
                    spp.tile([P, 512], dt.float32, name="k11", tag="sp")]
            for ec in range(EC):
                for i, ps in enumerate(accs):
                    hh4, pr = divmod(i, 2)
                    hsl = slice(hh4 * P, (hh4 + 1) * P)
                    c0 = pr * 512
                    mm(ps[:, 0:NB], wk0[ec][:, hsl], xt[ec][:, c0:c0 + NB],
                       start=(ec == 0), stop=False)
                    mm(ps[:, NB:2 * NB], wk0[ec][:, hsl],
                       xt[ec][:, c0 + NB:c0 + 512],
                       start=False, stop=(ec == EC - 1))
            for i, ps in enumerate(accs):
                hh4, pr = divmod(i, 2)
                nc.vector.tensor_copy(kt[hh4][:, pr * 512:(pr + 1) * 512],
                                      ps[:])
            for hh4 in range(2, 4):
                for pr in range(2):
                    kq_chain(wk0, 0, hh4, pr, kt, pool="p0")
            for hh4 in range(4):
                for pr in range(2):
                    kq_chain(wq0, 0, hh4, pr, qt, pool="p0")
            for sc in range(KC):
                v_chain(wv0, 0, sc, pool="p0")

            if phase == "proj":
                for hh4 in range(4):
                    for pr in range(2):
                        kq_chain(wk1, 1, hh4, pr, kt, pool="p0")
                        kq_chain(wq1, 1, hh4, pr, qt, pool="p0")
                for sc in range(KC):
                    v_chain(wv1, 1, sc, pool="p0")
                return

            # ---- filler: grp1 projections (K/Q per head, then V) ----
            for hh4 in range(4):
                for pr in range(2):
                    filler.append(
                        lambda hh4=hh4, pr=pr: kq_chain(wk1, 1, hh4, pr, kt))
                for pr in range(2):
                    filler.append(
                        lambda hh4=hh4, pr=pr: kq_chain(wq1, 1, hh4, pr, qt))
            for sc in range(KC):
                filler.append(lambda sc=sc: v_chain(wv1, 1, sc))
            n_proj_fill = len(filler)  # 24

            # ---- attention qb0, grp0 heads: consume the proj filler; the
            # ---- V1 leftovers may spill into q0h4's scores section (they
            # ---- still precede its ctx consume in the PE queue) ----
            fa = opts.get("fill_a", 5)
            fb = opts.get("fill_b", 4)
            assert 4 * fa + fb >= n_proj_fill == 24
            for h in range(4):
                attn_block(h, 0, fa)
            attn_block(4, 0, fb)
            assert not filler, "proj filler must drain before q0h4 ctx"

            # ---- attention qb0, rest of grp1: no filler available ----
            for h in range(5, 8):
                attn_block(h, 0, 0)

            if phase != "full":
                for h in range(HH):
                    attn_block(h, 1, 0)
                return

            # ---- out-proj filler for qb0 q-rows (fw-major; wo tiles are
            # ---- re-loaded per qb so the wp tag rotation stays deadlock
            # ---- free) ----
            wo_q0 = [w_tiles(woT_d, fw * 512, f"woA{fw}", nec=OC)
                     for fw in range(4)]
            for fw in range(4):
                for qc in range(4):
                    filler.append(
                        lambda fw=fw, qc=qc: op_chain(wo_q0[fw], fw, qc))
            # qb1's wo tiles: DMA emitted now (tag rotation gates the actual
            # loads on the qb0 out-proj chains) so the tail doesn't stall
            wo_q1 = [w_tiles(woT_d, fw * 512, f"woB{fw}", nec=OC)
                     for fw in range(4)]

            # ---- attention qb1 ----
            fc = opts.get("fill_c", 2)
            for h in range(4):
                attn_block(h, 1, fc)
            for h in range(4, 8):
                attn_block(h, 1, (16 - 4 * fc) // 4)
            take(len(filler))

            # ---- tail: out-proj for qb1 q-rows ----
            for fw in range(4):
                for qc in range(4, 8):
                    op_chain(wo_q1[fw], fw, qc)

        # ================= v1 schedule (previous baseline) =================
        def emit_v1():
            # ---- persistent loads ----
            xt = []
            for ec in range(EC):
                t = sb.tile([P, S], dt.bfloat16, name=f"xt{ec}", tag=f"xt{ec}")
                nc.sync.dma_start(t[:], xT_d[ec * P:(ec + 1) * P, :])
                xt.append(t)
            bqt = sb.tile([P, HH], dt.float32, name="bqt", tag="bqt")
            nc.sync.dma_start(bqt[:], bqT_d[:])
            bo_rep = sb.tile([P, E], dt.float32, name="bo_rep", tag="bo_rep")
            nc.sync.dma_start(bo_rep[:], bo_d.to_broadcast((P, E)))
            ones_col = sb.tile([P, 1], dt.bfloat16, name="ones_col",
                               tag="ones_col")
            nc.vector.memset(ones_col[:], 1.0)

            v_sb = []
            for sc in range(S // P):
                t = sb.tile([P, EH], dt.bfloat16, name=f"v{sc}", tag=f"v{sc}")
                v_sb.append(t)
            ctxT = []
            for h in range(HH):
                t = sb.tile([P, S], dt.bfloat16, name=f"ctxT{h}",
                            tag=f"ctxT{h}")
                ctxT.append(t)

            def load_w_tiles(dram, col0, label, nec=EC):
                tiles = []
                for ec in range(nec):
                    t = wp.tile([P, 512], dt.bfloat16,
                                name=f"{label}{ec}", tag=f"w{ec}")
                    nc.sync.dma_start(
                        t[:], dram[ec * P:(ec + 1) * P, col0:col0 + 512])
                    tiles.append(t)
                return tiles

            kt = {}
            qt = {}
            for grp in range(2):
                wk = load_w_tiles(wkT_d, grp * 512, f"wk{grp}")
                for hh4 in range(4):
                    h = grp * 4 + hh4
                    kth = ktp.tile([P, S], dt.bfloat16, name=f"kt{h}",
                                   tag="kt")
                    hsl = slice(hh4 * P, (hh4 + 1) * P)
                    for pr in range(2):
                        ps = ppp.tile([P, 512], dt.float32, name="kps",
                                      tag="pp")
                        c0 = pr * 512
                        for ec in range(EC):
                            nc.tensor.matmul(
                                ps[:, 0:NB], wk[ec][:, hsl],
                                xt[ec][:, c0:c0 + NB],
                                start=(ec == 0), stop=False)
                            nc.tensor.matmul(
                                ps[:, NB:2 * NB], wk[ec][:, hsl],
                                xt[ec][:, c0 + NB:c0 + 2 * NB],
                                start=False, stop=(ec == EC - 1))
                        nc.vector.tensor_copy(kth[:, c0:c0 + 512], ps[:])
                    kt[h] = kth

                wq = load_w_tiles(wqT_d, grp * 512, f"wq{grp}")
                for hh4 in range(4):
                    h = grp * 4 + hh4
                    qth = qtp.tile([P, S], dt.bfloat16, name=f"qt{h}",
                                   tag="qt")
                    hsl = slice(hh4 * P, (hh4 + 1) * P)
                    for pr in range(2):
                        ps = ppp.tile([P, 512], dt.float32, name="qps",
                                      tag="pp")
                        c0 = pr * 512
                        for ec in range(EC):
                            nc.tensor.matmul(
                                ps[:, 0:NB], wq[ec][:, hsl],
                                xt[ec][:, c0:c0 + NB],
                                start=(ec == 0), stop=False)
                            nc.tensor.matmul(
                                ps[:, NB:2 * NB], wq[ec][:, hsl],
                                xt[ec][:, c0 + NB:c0 + 2 * NB],
                                start=False, stop=(ec == EC - 1))
                        nc.vector.tensor_scalar_add(
                            qth[:, c0:c0 + 512], ps[:], bqt[:, h:h + 1])
                    qt[h] = qth

                fw = grp
                wv = load_w_tiles(wvT_d, fw * 512, f"wv{grp}")
                for sc in range(S // P):
                    ps = ppp.tile([P, 512], dt.float32, name="vps", tag="pp")
                    ssl = slice(sc * P, (sc + 1) * P)
                    for ec in range(EC):
                        nc.tensor.matmul(
                            ps[:, 0:NB], xt[ec][:, ssl], wv[ec][:, 0:NB],
                            start=(ec == 0), stop=False)
                        nc.tensor.matmul(
                            ps[:, NB:2 * NB], xt[ec][:, ssl],
                            wv[ec][:, NB:2 * NB],
                            start=False, stop=(ec == EC - 1))
                    nc.vector.tensor_copy(
                        v_sb[sc][:, fw * 512:(fw + 1) * 512], ps[:])

                for hh4 in range(4 if phase != "proj" else 0):
                    h = grp * 4 + hh4
                    hsl = slice(h * P, (h + 1) * P)
                    for qb in range(S // 512):
                        qA = slice(qb * 512, qb * 512 + NB)
                        qB = slice(qb * 512 + NB, qb * 512 + 2 * NB)
                        cpt = cpp.tile([P, 512], dt.float32,
                                       name=f"ctx{h}_{qb}", tag="cp")
                        ctxA = cpt[:, 0:NB]
                        ctxB = cpt[:, NB:2 * NB]
                        lag = opts.get("pipe_lag", 2)
                        pts = {}

                        def consume(kc):
                            pt = pts[kc]
                            nc.tensor.matmul(
                                ctxA, v_sb[kc][:, hsl], pt[:, 0:NB],
                                start=(kc == 0), stop=False)
                            nc.tensor.matmul(
                                ctxB, v_sb[kc][:, hsl], pt[:, NB:2 * NB],
                                start=False, stop=(kc == KC - 1))

                        for kc in range(KC):
                            ksl = slice(kc * P, (kc + 1) * P)
                            sp = spp.tile([P, 512], dt.float32, name="sp",
                                          tag="sp")
                            nc.tensor.matmul(sp[:, 0:NB], kt[h][:, ksl],
                                             qt[h][:, qA])
                            nc.tensor.matmul(sp[:, NB:2 * NB], kt[h][:, ksl],
                                             qt[h][:, qB])
                            pt = ptp.tile([P, 512], dt.bfloat16, name="pt",
                                          tag="pt")
                            nc.scalar.activation(pt[:], sp[:], AFT.Exp)
                            pts[kc] = pt
                            if kc >= lag:
                                consume(kc - lag)
                        for kc in range(KC - lag, KC):
                            consume(kc)

                        su = upp.tile([1, 512], dt.float32,
                                      name=f"su{h}_{qb}", tag="su")
                        if sums_on == "pe":
                            for kc in range(KC):
                                pt = pts[kc]
                                nc.tensor.matmul(su[:, 0:NB], ones_col[:],
                                                 pt[:, 0:NB],
                                                 start=(kc == 0),
                                                 stop=False)
                                nc.tensor.matmul(su[:, NB:2 * NB],
                                                 ones_col[:],
                                                 pt[:, NB:2 * NB],
                                                 start=False,
                                                 stop=(kc == KC - 1))
                        else:
                            tacc = tp.tile([P, 512], dt.float32, name="tacc",
                                           tag="tacc")
                            nc.vector.tensor_add(tacc[:], pts[0][:],
                                                 pts[1][:])
                            for kc in range(2, KC):
                                nc.vector.tensor_add(tacc[:], tacc[:],
                                                     pts[kc][:])
                            t16 = tp.tile([P, 512], dt.bfloat16, name="t16",
                                          tag="t16")
                            nc.vector.tensor_copy(t16[:], tacc[:])
                            nc.tensor.matmul(su[:, 0:NB], ones_col[:],
                                             t16[:, 0:NB])
                            nc.tensor.matmul(su[:, NB:2 * NB], ones_col[:],
                                             t16[:, NB:2 * NB])
                        r_row = rowp.tile([1, 512], dt.float32, name="r_row",
                                          tag="rrow")
                        nc.vector.reciprocal(r_row[:], su[:])
                        rep = rowp.tile([P, 512], dt.float32, name="rep",
                                        tag="rep")
                        nc.gpsimd.partition_broadcast(rep[:], r_row[:])
                        nc.vector.tensor_mul(
                            ctxT[h][:, qb * 512:(qb + 1) * 512], cpt[:],
                            rep[:])

            for fw in range(4 if phase == "full" else 0):
                wo = load_w_tiles(woT_d, fw * 512, f"wo{fw}", nec=OC)
                for qc in range(S // P):
                    qsl = slice(qc * P, (qc + 1) * P)
                    ps = ppp.tile([P, 512], dt.float32, name="ops", tag="pp")
                    for h in range(HH):
                        nc.tensor.matmul(
                            ps[:, 0:NB], ctxT[h][:, qsl], wo[h][:, 0:NB],
                            start=(h == 0), stop=False)
                        nc.tensor.matmul(
                            ps[:, NB:2 * NB], ctxT[h][:, qsl],
                            wo[h][:, NB:2 * NB],
                            start=False, stop=(h == HH - 1))
                    o0 = outp.tile([P, 512], out_dt, name="o0",
                                   tag="ot")
                    f0 = fw * 512
                    nc.vector.tensor_add(o0[:], ps[:],
                                         bo_rep[:, f0:f0 + 512])
                    nc.sync.dma_start(out_d[qsl, f0:f0 + 512], o0[:])

        emit = emit_v2 if sched == "v2" else emit_v1
        if bench_iters is None:
            emit()
        else:
            with tc.For_i(0, bench_iters, 1):
                emit()

    nc.compile()
    return nc


def _get_program():
    global _PROGRAM
    if _PROGRAM is None:
        _PROGRAM = _build_program()
    return _PROGRAM


def make_in_maps(query, in_proj_weight, in_proj_bias, out_proj_weight,
                 out_proj_bias):
    """Host-side sharding: slice/transpose/cast per core. Pure layout prep."""
    x = np.asarray(query, dtype=np.float32)
    W = np.asarray(in_proj_weight, dtype=np.float32)
    b = np.asarray(in_proj_bias, dtype=np.float32)
    Wo = np.asarray(out_proj_weight, dtype=np.float32)
    bo = np.asarray(out_proj_bias, dtype=np.float32)

    sc = np.float32(1.0 / np.sqrt(D))
    wqT = np.ascontiguousarray((W[:E] * sc).T).astype(BF16)       # [E, E]
    wkT = np.ascontiguousarray(W[E:2 * E].T).astype(BF16)
    wvT = np.ascontiguousarray(W[2 * E:].T).astype(BF16)
    woT = np.ascontiguousarray(Wo.T).astype(BF16)                 # [E, E]
    bq_s = (b[:E] * sc).reshape(H, P)
    bv = b[2 * E:]                                                # [E]

    in_maps = []
    for c in range(NCORES):
        bi, hh = c // 2, c % 2
        esl = slice(hh * EH, (hh + 1) * EH)
        xT = np.ascontiguousarray(x[bi].T).astype(BF16)
        # fold this half's share of Wo@bv into the output bias; add bo
        # itself only on the hh==0 core (partials are summed)
        bo_half = Wo[:, esl] @ bv[esl]
        if hh == 0:
            bo_half = bo_half + bo
        in_maps.append({
            "xT": xT,
            "wqT": np.ascontiguousarray(wqT[:, esl]),
            "wkT": np.ascontiguousarray(wkT[:, esl]),
            "wvT": np.ascontiguousarray(wvT[:, esl]),
            "woT": np.ascontiguousarray(woT[esl, :]),
            "bqT": np.ascontiguousarray(bq_s[hh * HH:(hh + 1) * HH].T),
            "bo": np.ascontiguousarray(bo_half.reshape(1, E)
                                       .astype(np.float32)),
        })
    return in_maps


def assemble_out(results):
    """Gather: sum each batch's two tensor-parallel partial outputs."""
    out = np.empty((B, S, E), dtype=np.float32)
    for bi in range(B):
        out[bi] = (results[2 * bi]["out"].astype(np.float32)
                   + results[2 * bi + 1]["out"].astype(np.float32))
    return out


def kernel(query, in_proj_weight, in_proj_bias, out_proj_weight,
           out_proj_bias):
    from concourse import bass_utils
    nc = _get_program()
    in_maps = make_in_maps(query, in_proj_weight, in_proj_bias,
                           out_proj_weight, out_proj_bias)
    res = bass_utils.run_bass_kernel_spmd(nc, in_maps,
                                          core_ids=list(range(NCORES)))
    return assemble_out(res.results)


# revision 26
# speedup vs baseline: 1.1934x; 1.0011x over previous
"""Trainium2 Bass kernel for fused multi-head attention (CompositeMHA).

Reference computation (B=4, S=1024, E=2048, H=16, D=128), fp32:
    proj = x @ in_proj_weight.T + in_proj_bias        # [B,S,3E]
    q,k,v = split(proj); heads of D=128
    ctx = softmax(q k^T / sqrt(D)) v                   # per (b, head)
    out = ctx @ out_proj_weight.T + out_proj_bias      # [B,S,E]

Sharding (8 cores, no on-device collectives), per the tensor-parallel hint:
data-parallel over the 4 batches x tensor-parallel over head halves.
Core c handles batch c//2 and heads [hh*8, hh*8+8) where hh = c%2 —
sharding the corresponding 3E rows of in_proj_weight and columns of
out_proj_weight.  Each core emits a partial output [S, E]; the gather
step sums each batch's two partials (the TP reduction).

Exact algebraic simplifications (no accuracy cost):
  - K bias dropped: softmax over j of (q+bq)·(k_j+bk) is invariant to
    the j-constant term (q+bq)·bk, so k_j needs no bias.
  - V bias folded into the output bias: sum_j p_j = 1, so
    ctx = ctxU + bv and out = ctxU@Wo^T + (bo + Wo@bv).

Tiling: ALL matmuls use 256-wide moving tensors (~107 ns measured at
full rate; 512-wide costs ~272 ns, so 2x256 wins).  Matmuls are emitted
as pairs sharing each loaded lhsT.

Schedule v2 (default): the PE queue is kept saturated by interleaving
independent projection / out-projection chains ("filler") into the
attention stream, so cross-engine waits (PE->ACT exp->PE) hide behind
queued matmuls instead of stalling the PE head-of-line:
    P0: K0/Q0/V0 projections (first two K head chains ec-outer, paced
        by the interleaved xT+Wk DMA stream to shrink the startup stall)
    blocks qb0 h0..h3   + filler K1/Q1/V1 chains (V1 spills into q0h4)
    blocks qb0 h4..h7   (little filler left -- ACT-paced stretch)
    blocks qb1 h0..h7   + filler out-proj chains for q-rows of qb0
    tail: out-proj chains for qb1 (pure PE)
Softmax sums+broadcast are FUSED into PE accumulation: rep[128,512] =
ones[128x128] @ P^T replicates the per-q denominators on every
partition; only reciprocal + multiply run on DVE.  In filler-filled
blocks (where DVE has slack) the k-chunk accumulation runs as a DVE
add-tree first, cutting the PE part to one matmul pair; starved blocks
keep the full 8-pair PE accumulation since 10 DVE ops (~6 us) would
out-pace PE there.  (GPSIMD partition_all_reduce measured ~7 us/op on
real HW -- 111 us slower per kernel, despite a 0.5 us cost-model
price; keep ucode ops off the critical path.)  PSUM: 2 proj + 3 score
+ 2 ctx + 1 sums banks; projection chains in the opening stage rotate
across ALL four pools (8 banks of copy-drain runway) while the
score/ctx/sums banks are idle.

On-core dataflow (bf16 matmuls into fp32 PSUM):
    xT   = X_b^T                      [E, S]
    K^T[h] = Wk^T-chunk^T @ xT        [D, S]   per head (no bias)
    Q^T[h] likewise + bq (DVE)        [D, S]   (1/sqrt(D) folded in)
    V      = xT-chunk^T @ Wv^T        [S, E/2] natural layout (no bias)
    scoresT[k,q] = K^T-chunk^T @ Q^T  -> exp on ACT -> P^T (bf16)
    sums[q]: DVE add-tree over P^T chunks -> GPSIMD all-reduce -> recip
    ctx^T[h] = V-chunk^T @ P^T accumulated; * recip -> bf16
    out_partial = ctx^T-chunk^T @ Wout^T (+ bo') -> fp32
"""

import numpy as np
import ml_dtypes

B, S, E, H = 4, 1024, 2048, 16
D = 128          # head dim == partition size
P = 128
HH = 8           # heads per core (head half)
EH = HH * D      # 1024: e-columns of this half
EC = E // P      # 16 e-chunks (contraction for in-proj)
OC = EH // P     # 8 e-chunks (contraction for out-proj)
KC = S // P      # 8 key chunks
NCORES = 8
NB = 256         # moving-tensor width for all matmuls
BF16 = ml_dtypes.bfloat16

_PROGRAM = None


def _build_program(bench_iters=None, phase="full", opts=None):
    opts = opts or {}
    import concourse.bass as bass  # noqa: F401
    import concourse.tile as tile
    from concourse import bacc, bass_isa, mybir
    from contextlib import ExitStack

    dt = mybir.dt
    AFT = mybir.ActivationFunctionType
    sched = opts.get("sched", "v2")

    nc = bacc.Bacc("TRN2", target_bir_lowering=False, debug=False,
                   num_devices=NCORES)
    out_dt = dt.bfloat16 if opts.get("out_bf16", True) else dt.float32

    xT_d = nc.dram_tensor("xT", [E, S], dt.bfloat16, kind="ExternalInput").ap()
    wqT_d = nc.dram_tensor("wqT", [E, EH], dt.bfloat16, kind="ExternalInput").ap()
    wkT_d = nc.dram_tensor("wkT", [E, EH], dt.bfloat16, kind="ExternalInput").ap()
    wvT_d = nc.dram_tensor("wvT", [E, EH], dt.bfloat16, kind="ExternalInput").ap()
    woT_d = nc.dram_tensor("woT", [EH, E], dt.bfloat16, kind="ExternalInput").ap()
    bqT_d = nc.dram_tensor("bqT", [P, HH], dt.float32, kind="ExternalInput").ap()
    bo_d = nc.dram_tensor("bo", [1, E], dt.float32, kind="ExternalInput").ap()
    out_d = nc.dram_tensor("out", [S, E], out_dt,
                           kind="ExternalOutput").ap()

    sums_on = opts.get("sums_on", "pe")  # v1: "pe" | "dve"

    with tile.TileContext(nc) as tc, ExitStack() as ctx:
        sb = ctx.enter_context(tc.tile_pool(name="persist", bufs=1))
        wp = ctx.enter_context(tc.tile_pool(name="wstream",
                                            bufs=opts.get("wp_bufs", 2)))
        ktp = ctx.enter_context(tc.tile_pool(name="ktp", bufs=8))
        qtp = ctx.enter_context(tc.tile_pool(name="qtp", bufs=8))
        ptp = ctx.enter_context(tc.tile_pool(
            name="ptp", bufs=opts.get("ptp_bufs", 24 if sched == "v1" else 16)))
        outp = ctx.enter_context(tc.tile_pool(
            name="outp", bufs=6 if sched == "v1" else 8))
        wop = ctx.enter_context(tc.tile_pool(name="wop", bufs=1))
        rowp = ctx.enter_context(tc.tile_pool(
            name="rowp", bufs=4 if sched == "v1" else 2))
        tp = ctx.enter_context(tc.tile_pool(
            name="tsum", bufs=4 if sched == "v1" else 2))
        if sched == "v1":
            pp_n, sp_n, cp_n, up_n = (opts.get("pp_bufs", 4),
                                      opts.get("sp_bufs", 2),
                                      opts.get("cp_bufs", 1), 1)
        else:
            pp_n, sp_n, cp_n, up_n = (opts.get("pp_bufs", 2),
                                      opts.get("sp_bufs", 3),
                                      opts.get("cp_bufs", 2), 1)
        ppp = ctx.enter_context(tc.tile_pool(name="ppsum", bufs=pp_n,
                                             space="PSUM"))
        spp = ctx.enter_context(tc.tile_pool(name="spsum", bufs=sp_n,
                                             space="PSUM"))
        cpp = ctx.enter_context(tc.tile_pool(name="cpsum", bufs=cp_n,
                                             space="PSUM"))
        if up_n:
            upp = ctx.enter_context(tc.tile_pool(name="upsum", bufs=up_n,
                                                 space="PSUM"))

        # ================= v2 schedule =================
        def emit_v2():
            mm = nc.tensor.matmul

            # ---- persistent tiles ----
            xt = [sb.tile([P, S], dt.bfloat16, name=f"xt{ec}", tag=f"xt{ec}")
                  for ec in range(EC)]
            bqt = sb.tile([P, HH], dt.float32, name="bqt", tag="bqt")
            bo_rep = sb.tile([P, E], dt.float32, name="bo_rep", tag="bo_rep")
            ones128 = sb.tile([P, P], dt.bfloat16, name="ones128",
                              tag="ones128")
            nc.vector.memset(ones128[:], 1.0)
            v_sb = [sb.tile([P, EH], dt.bfloat16, name=f"v{sc}", tag=f"v{sc}")
                    for sc in range(KC)]
            ctxT = [sb.tile([P, S], dt.bfloat16, name=f"ctxT{h}",
                            tag=f"ctxT{h}") for h in range(HH)]

            def w_tiles(dram, col0, label, nec=EC):
                tiles = []
                for ec in range(nec):
                    t = wp.tile([P, 512], dt.bfloat16,
                                name=f"{label}{ec}", tag=f"w{ec}")
                    nc.sync.dma_start(
                        t[:], dram[ec * P:(ec + 1) * P, col0:col0 + 512])
                    tiles.append(t)
                return tiles

            # ---- DMA issue: wk0 and xt interleaved first (startup), then
            # ---- the rest; weight streaming is gated by wp tag rotation.
            wk0 = []
            for ec in range(EC):
                t = wp.tile([P, 512], dt.bfloat16, name=f"wk0_{ec}",
                            tag=f"w{ec}")
                nc.sync.dma_start(t[:], wkT_d[ec * P:(ec + 1) * P, 0:512])
                nc.sync.dma_start(xt[ec][:], xT_d[ec * P:(ec + 1) * P, :])
                wk0.append(t)
            nc.sync.dma_start(bqt[:], bqT_d[:])
            nc.sync.dma_start(bo_rep[:], bo_d.to_broadcast((P, E)))
            wq0 = w_tiles(wqT_d, 0, "wq0")
            wv0 = w_tiles(wvT_d, 0, "wv0")
            wk1 = w_tiles(wkT_d, 512, "wk1")
            wq1 = w_tiles(wqT_d, 512, "wq1")
            wv1 = w_tiles(wvT_d, 512, "wv1")
            # out-proj weights: persistent (own tags, loaded once) — saves
            # a 4MB re-load and its tag-rotation gating; DMA is a shared
            # cross-core resource (8-core run measured +14% over 1-core)
            wo_all = []
            for fw in range(4):
                tiles = []
                for ec in range(OC):
                    t = wop.tile([P, 512], dt.bfloat16, name=f"wo{fw}_{ec}",
                                 tag=f"wo{fw}_{ec}")
                    nc.sync.dma_start(
                        t[:], woT_d[ec * P:(ec + 1) * P,
                                    fw * 512:(fw + 1) * 512])
                    tiles.append(t)
                wo_all.append(tiles)

            kt = {}
            qt = {}

            # During P0 (and the proj-only phase) the score/ctx/sums banks
            # are idle: rotate projection chains across ALL psum pools for
            # 8 banks of copy-drain runway instead of ppp's 2.
            p0_pools = [(ppp, "pp"), (spp, "sp"), (cpp, "cp"), (upp, "rep")]
            p0_idx = [0]

            def chain_psum(pool):
                if pool is None or not opts.get("p0mp", True):
                    return ppp.tile([P, 512], dt.float32, name="chps",
                                    tag="pp")
                pl, tag = p0_pools[p0_idx[0] % len(p0_pools)]
                p0_idx[0] += 1
                return pl.tile([P, 512], dt.float32, name="chps", tag=tag)

            def kq_chain(w, grp, hh4, pr, dst, pool=None):
                """One 512-col block of K^T or Q^T for head grp*4+hh4."""
                hsl = slice(hh4 * P, (hh4 + 1) * P)
                c0 = pr * 512
                ps = chain_psum(pool)
                for ec in range(EC):
                    mm(ps[:, 0:NB], w[ec][:, hsl], xt[ec][:, c0:c0 + NB],
                       start=(ec == 0), stop=False)
                    mm(ps[:, NB:2 * NB], w[ec][:, hsl],
                       xt[ec][:, c0 + NB:c0 + 512],
                       start=False, stop=(ec == EC - 1))
                h = grp * 4 + hh4
                if dst is kt:
                    nc.vector.tensor_copy(kt[h][:, c0:c0 + 512], ps[:])
                else:
                    nc.vector.tensor_scalar_add(
                        qt[h][:, c0:c0 + 512], ps[:], bqt[:, h:h + 1])

            def v_chain(wv, fw, sc, pool=None):
                ps = chain_psum(pool)
                ssl = slice(sc * P, (sc + 1) * P)
                for ec in range(EC):
                    mm(ps[:, 0:NB], xt[ec][:, ssl], wv[ec][:, 0:NB],
                       start=(ec == 0), stop=False)
                    mm(ps[:, NB:2 * NB], xt[ec][:, ssl], wv[ec][:, NB:2 * NB],
                       start=False, stop=(ec == EC - 1))
                nc.vector.tensor_copy(
                    v_sb[sc][:, fw * 512:(fw + 1) * 512], ps[:])

            def op_chain(wo, fw, qc):
                qsl = slice(qc * P, (qc + 1) * P)
                ps = ppp.tile([P, 512], dt.float32, name="ops", tag="pp")
                for h in range(HH):
                    mm(ps[:, 0:NB], ctxT[h][:, qsl], wo[h][:, 0:NB],
                       start=(h == 0), stop=False)
                    mm(ps[:, NB:2 * NB], ctxT[h][:, qsl], wo[h][:, NB:2 * NB],
                       start=False, stop=(h == HH - 1))
                o0 = outp.tile([P, 512], out_dt, name="o0", tag="ot")
                f0 = fw * 512
                nc.vector.tensor_add(o0[:], ps[:], bo_rep[:, f0:f0 + 512])
                nc.sync.dma_start(out_d[qsl, f0:f0 + 512], o0[:])

            # allocate kt/qt tiles in head order (tag rotation = 8 live)
            for h in range(HH):
                kt[h] = ktp.tile([P, S], dt.bfloat16, name=f"kt{h}", tag="kt")
                qt[h] = qtp.tile([P, S], dt.bfloat16, name=f"qt{h}", tag="qt")

            # ---- filler queue ----
            filler = []

            def take(n):
                for _ in range(n):
                    if filler:
                        filler.pop(0)()

            probe = opts.get("probe")  # None|"const_p"|"no_tail"

            def attn_block(h, qb, n_fill):
                hsl = slice(h * P, (h + 1) * P)
                qA = slice(qb * 512, qb * 512 + NB)
                qB = slice(qb * 512 + NB, (qb + 1) * 512)
                nf1 = n_fill // 3
                nf2 = n_fill // 3
                nf3 = n_fill - nf1 - nf2
                # DVE-tree sums only in filler-filled blocks: there DVE has
                # slack and this trims 14 PE sum-matmuls to 2; in starved
                # blocks DVE (10 ops ~6us) would out-pace PE, so keep PE sums
                dsum_mode = opts.get("dsum", "fill")
                dsum = (dsum_mode == "all"
                        or (dsum_mode == "fill" and n_fill > 0))
                cpt = cpp.tile([P, 512], dt.float32, name=f"c{h}_{qb}",
                               tag="cp")
                pts = []
                acc = None
                for kc in range(KC):
                    ksl = slice(kc * P, (kc + 1) * P)
                    sp = spp.tile([P, 512], dt.float32, name="sp", tag="sp")
                    mm(sp[:, 0:NB], kt[h][:, ksl], qt[h][:, qA])
                    mm(sp[:, NB:2 * NB], kt[h][:, ksl], qt[h][:, qB])
                    if probe != "const_p":
                        pt = ptp.tile([P, 512], dt.bfloat16, name="pt",
                                      tag="pt")
                        if opts.get("exp_split"):
                            nc.scalar.activation(pt[:, 0:NB], sp[:, 0:NB],
                                                 AFT.Exp)
                            nc.scalar.activation(pt[:, NB:2 * NB],
                                                 sp[:, NB:2 * NB], AFT.Exp)
                        else:
                            nc.scalar.activation(pt[:], sp[:], AFT.Exp)
                        pts.append(pt)
                        if dsum and probe is None:
                            if kc == 1:
                                acc = tp.tile([P, 512], dt.float32,
                                              name="acc", tag="acc")
                                nc.vector.tensor_add(acc[:], pts[0][:],
                                                     pts[1][:])
                            elif kc > 1:
                                nc.vector.tensor_add(acc[:], acc[:], pt[:])
                    if kc == 2:
                        take(nf1)
                    elif kc == 5:
                        take(nf2)
                take(nf3)
                for kc in range(KC):
                    rhs = (v_sb[kc][:, 0:512] if probe == "const_p"
                           else pts[kc][:])
                    mm(cpt[:, 0:NB], v_sb[kc][:, hsl], rhs[:, 0:NB],
                       start=(kc == 0), stop=False)
                    mm(cpt[:, NB:2 * NB], v_sb[kc][:, hsl],
                       rhs[:, NB:2 * NB],
                       start=False, stop=(kc == KC - 1))
                if probe in ("const_p", "no_tail"):
                    nc.vector.tensor_copy(
                        ctxT[h][:, qb * 512:(qb + 1) * 512], cpt[:])
                    return
                # fused sums+broadcast on PE: rep[p, q] = sum_k P^T[k, q]
                # (ones lhsT replicates the column sums on every partition)
                rep = upp.tile([P, 512], dt.float32, name=f"rep{h}_{qb}",
                               tag="rep")
                if dsum:
                    t16 = tp.tile([P, 512], dt.bfloat16, name="t16",
                                  tag="t16")
                    nc.vector.tensor_copy(t16[:], acc[:])
                    mm(rep[:, 0:NB], ones128[:], t16[:, 0:NB])
                    mm(rep[:, NB:2 * NB], ones128[:], t16[:, NB:2 * NB])
                else:
                    for kc in range(KC):
                        mm(rep[:, 0:NB], ones128[:], pts[kc][:, 0:NB],
                           start=(kc == 0), stop=False)
                        mm(rep[:, NB:2 * NB], ones128[:],
                           pts[kc][:, NB:2 * NB],
                           start=False, stop=(kc == KC - 1))
                rrec = rowp.tile([P, 512], dt.float32, name="rrec",
                                 tag="rrec")
                nc.vector.reciprocal(rrec[:], rep[:])
                nc.vector.tensor_mul(
                    ctxT[h][:, qb * 512:(qb + 1) * 512], cpt[:], rrec[:])

            # ---- P0: grp0 projections ----
            # first two K head chains ec-outer, 4 accumulators (2 proj banks
            # + 2 borrowed score banks): paced by the interleaved xt+wk DMA
            # stream, so the startup stall shrinks to the DMA/PE rate gap.
            accs = [ppp.tile([P, 512], dt.float32, name="k00", tag="pp"),
                    ppp.tile([P, 512], dt.float32, name="k01", tag="pp"),
                    spp.tile([P, 512], dt.float32, name="k10", tag="sp"),
                    spp.tile([P, 512], dt.float32, name="k11", tag="sp")]
            for ec in range(EC):
                for i, ps in enumerate(accs):
                    hh4, pr = divmod(i, 2)
                    hsl = slice(hh4 * P, (hh4 + 1) * P)
                    c0 = pr * 512
                    mm(ps[:, 0:NB], wk0[ec][:, hsl], xt[ec][:, c0:c0 + NB],
                       start=(ec == 0), stop=False)
                    mm(ps[:, NB:2 * NB], wk0[ec][:, hsl],
                       xt[ec][:, c0 + NB:c0 + 512],
                       start=False, stop=(ec == EC - 1))
            for i, ps in enumerate(accs):
                hh4, pr = divmod(i, 2)
                nc.vector.tensor_copy(kt[hh4][:, pr * 512:(pr + 1) * 512],
                                      ps[:])
            for hh4 in range(2, 4):
                for pr in range(2):
                    kq_chain(wk0, 0, hh4, pr, kt, pool="p0")
            for hh4 in range(4):
                for pr in range(2):
                    kq_chain(wq0, 0, hh4, pr, qt, pool="p0")
            for sc in range(KC):
                v_chain(wv0, 0, sc, pool="p0")

            if phase == "proj":
                for hh4 in range(4):
                    for pr in range(2):
                        kq_chain(wk1, 1, hh4, pr, kt, pool="p0")
                        kq_chain(wq1, 1, hh4, pr, qt, pool="p0")
                for sc in range(KC):
                    v_chain(wv1, 1, sc, pool="p0")
                return

            # ---- filler: grp1 projections (K/Q per head, then V) ----
            for hh4 in range(4):
                for pr in range(2):
                    filler.append(
                        lambda hh4=hh4, pr=pr: kq_chain(wk1, 1, hh4, pr, kt))
                for pr in range(2):
                    filler.append(
                        lambda hh4=hh4, pr=pr: kq_chain(wq1, 1, hh4, pr, qt))
            for sc in range(KC):
                filler.append(lambda sc=sc: v_chain(wv1, 1, sc))
            n_proj_fill = len(filler)  # 24

            # ---- attention qb0, grp0 heads: consume the proj filler; the
            # ---- V1 leftovers may spill into q0h4's scores section (they
            # ---- still precede its ctx consume in the PE queue) ----
            fa = opts.get("fill_a", 5)
            fb = opts.get("fill_b", 4)
            assert 4 * fa + fb >= n_proj_fill == 24
            for h in range(4):
                attn_block(h, 0, fa)
            attn_block(4, 0, fb)
            assert not filler, "proj filler must drain before q0h4 ctx"

            # ---- attention qb0, rest of grp1: no filler available ----
            for h in range(5, 8):
                attn_block(h, 0, 0)

            if phase != "full":
                for h in range(HH):
                    attn_block(h, 1, 0)
                return

            # ---- out-proj filler for qb0 q-rows (fw-major; wo tiles are
            # ---- re-loaded per qb so the wp tag rotation stays deadlock
            # ---- free) ----
            for fw in range(4):
                for qc in range(4):
                    filler.append(
                        lambda fw=fw, qc=qc: op_chain(wo_all[fw], fw, qc))

            # ---- attention qb1 ----
            fc = opts.get("fill_c", 2)
            for h in range(4):
                attn_block(h, 1, fc)
            for h in range(4, 8):
                attn_block(h, 1, (16 - 4 * fc) // 4)
            take(len(filler))

            # ---- tail: out-proj for qb1 q-rows ----
            for fw in range(4):
                for qc in range(4, 8):
                    op_chain(wo_all[fw], fw, qc)

        # ================= v1 schedule (previous baseline) =================
        def emit_v1():
            # ---- persistent loads ----
            xt = []
            for ec in range(EC):
                t = sb.tile([P, S], dt.bfloat16, name=f"xt{ec}", tag=f"xt{ec}")
                nc.sync.dma_start(t[:], xT_d[ec * P:(ec + 1) * P, :])
                xt.append(t)
            bqt = sb.tile([P, HH], dt.float32, name="bqt", tag="bqt")
            nc.sync.dma_start(bqt[:], bqT_d[:])
            bo_rep = sb.tile([P, E], dt.float32, name="bo_rep", tag="bo_rep")
            nc.sync.dma_start(bo_rep[:], bo_d.to_broadcast((P, E)))
            ones_col = sb.tile([P, 1], dt.bfloat16, name="ones_col",
                               tag="ones_col")
            nc.vector.memset(ones_col[:], 1.0)

            v_sb = []
            for sc in range(S // P):
                t = sb.tile([P, EH], dt.bfloat16, name=f"v{sc}", tag=f"v{sc}")
                v_sb.append(t)
            ctxT = []
            for h in range(HH):
                t = sb.tile([P, S], dt.bfloat16, name=f"ctxT{h}",
                            tag=f"ctxT{h}")
                ctxT.append(t)

            def load_w_tiles(dram, col0, label, nec=EC):
                tiles = []
                for ec in range(nec):
                    t = wp.tile([P, 512], dt.bfloat16,
                                name=f"{label}{ec}", tag=f"w{ec}")
                    nc.sync.dma_start(
                        t[:], dram[ec * P:(ec + 1) * P, col0:col0 + 512])
                    tiles.append(t)
                return tiles

            kt = {}
            qt = {}
            for grp in range(2):
                wk = load_w_tiles(wkT_d, grp * 512, f"wk{grp}")
                for hh4 in range(4):
                    h = grp * 4 + hh4
                    kth = ktp.tile([P, S], dt.bfloat16, name=f"kt{h}",
                                   tag="kt")
                    hsl = slice(hh4 * P, (hh4 + 1) * P)
                    for pr in range(2):
                        ps = ppp.tile([P, 512], dt.float32, name="kps",
                                      tag="pp")
                        c0 = pr * 512
                        for ec in range(EC):
                            nc.tensor.matmul(
                                ps[:, 0:NB], wk[ec][:, hsl],
                                xt[ec][:, c0:c0 + NB],
                                start=(ec == 0), stop=False)
                            nc.tensor.matmul(
                                ps[:, NB:2 * NB], wk[ec][:, hsl],
                                xt[ec][:, c0 + NB:c0 + 2 * NB],
                                start=False, stop=(ec == EC - 1))
                        nc.vector.tensor_copy(kth[:, c0:c0 + 512], ps[:])
                    kt[h] = kth

                wq = load_w_tiles(wqT_d, grp * 512, f"wq{grp}")
                for hh4 in range(4):
                    h = grp * 4 + hh4
                    qth = qtp.tile([P, S], dt.bfloat16, name=f"qt{h}",
                                   tag="qt")
                    hsl = slice(hh4 * P, (hh4 + 1) * P)
                    for pr in range(2):
                        ps = ppp.tile([P, 512], dt.float32, name="qps",
                                      tag="pp")
                        c0 = pr * 512
                        for ec in range(EC):
                            nc.tensor.matmul(
                                ps[:, 0:NB], wq[ec][:, hsl],
                                xt[ec][:, c0:c0 + NB],
                                start=(ec == 0), stop=False)
                            nc.tensor.matmul(
                                ps[:, NB:2 * NB], wq[ec][:, hsl],
                                xt[ec][:, c0 + NB:c0 + 2 * NB],
                                start=False, stop=(ec == EC - 1))
                        nc.vector.tensor_scalar_add(
                            qth[:, c0:c0 + 512], ps[:], bqt[:, h:h + 1])
                    qt[h] = qth

                fw = grp
                wv = load_w_tiles(wvT_d, fw * 512, f"wv{grp}")
                for sc in range(S // P):
                    ps = ppp.tile([P, 512], dt.float32, name="vps", tag="pp")
                    ssl = slice(sc * P, (sc + 1) * P)
                    for ec in range(EC):
                        nc.tensor.matmul(
                            ps[:, 0:NB], xt[ec][:, ssl], wv[ec][:, 0:NB],
                            start=(ec == 0), stop=False)
                        nc.tensor.matmul(
                            ps[:, NB:2 * NB], xt[ec][:, ssl],
                            wv[ec][:, NB:2 * NB],
                            start=False, stop=(ec == EC - 1))
                    nc.vector.tensor_copy(
                        v_sb[sc][:, fw * 512:(fw + 1) * 512], ps[:])

                for hh4 in range(4 if phase != "proj" else 0):
                    h = grp * 4 + hh4
                    hsl = slice(h * P, (h + 1) * P)
                    for qb in range(S // 512):
                        qA = slice(qb * 512, qb * 512 + NB)
                        qB = slice(qb * 512 + NB, qb * 512 + 2 * NB)
                        cpt = cpp.tile([P, 512], dt.float32,
                                       name=f"ctx{h}_{qb}", tag="cp")
                        ctxA = cpt[:, 0:NB]
                        ctxB = cpt[:, NB:2 * NB]
                        lag = opts.get("pipe_lag", 2)
                        pts = {}

                        def consume(kc):
                            pt = pts[kc]
                            nc.tensor.matmul(
                                ctxA, v_sb[kc][:, hsl], pt[:, 0:NB],
                                start=(kc == 0), stop=False)
                            nc.tensor.matmul(
                                ctxB, v_sb[kc][:, hsl], pt[:, NB:2 * NB],
                                start=False, stop=(kc == KC - 1))

                        for kc in range(KC):
                            ksl = slice(kc * P, (kc + 1) * P)
                            sp = spp.tile([P, 512], dt.float32, name="sp",
                                          tag="sp")
                            nc.tensor.matmul(sp[:, 0:NB], kt[h][:, ksl],
                                             qt[h][:, qA])
                            nc.tensor.matmul(sp[:, NB:2 * NB], kt[h][:, ksl],
                                             qt[h][:, qB])
                            pt = ptp.tile([P, 512], dt.bfloat16, name="pt",
                                          tag="pt")
                            nc.scalar.activation(pt[:], sp[:], AFT.Exp)
                            pts[kc] = pt
                            if kc >= lag:
                                consume(kc - lag)
                        for kc in range(KC - lag, KC):
                            consume(kc)

                        su = upp.tile([1, 512], dt.float32,
                                      name=f"su{h}_{qb}", tag="su")
                        if sums_on == "pe":
                            for kc in range(KC):
                                pt = pts[kc]
                                nc.tensor.matmul(su[:, 0:NB], ones_col[:],
                                                 pt[:, 0:NB],
                                                 start=(kc == 0),
                                                 stop=False)
                                nc.tensor.matmul(su[:, NB:2 * NB],
                                                 ones_col[:],
                                                 pt[:, NB:2 * NB],
                                                 start=False,
                                                 stop=(kc == KC - 1))
                        else:
                            tacc = tp.tile([P, 512], dt.float32, name="tacc",
                                           tag="tacc")
                            nc.vector.tensor_add(tacc[:], pts[0][:],
                                                 pts[1][:])
                            for kc in range(2, KC):
                                nc.vector.tensor_add(tacc[:], tacc[:],
                                                     pts[kc][:])
                            t16 = tp.tile([P, 512], dt.bfloat16, name="t16",
                                          tag="t16")
                            nc.vector.tensor_copy(t16[:], tacc[:])
                            nc.tensor.matmul(su[:, 0:NB], ones_col[:],
                                             t16[:, 0:NB])
                            nc.tensor.matmul(su[:, NB:2 * NB], ones_col[:],
                                             t16[:, NB:2 * NB])
                        r_row = rowp.tile([1, 512], dt.float32, name="r_row",
                                          tag="rrow")
                        nc.vector.reciprocal(r_row[:], su[:])
                        rep = rowp.tile([P, 512], dt.float32, name="rep",
                                        tag="rep")
                        nc.gpsimd.partition_broadcast(rep[:], r_row[:])
                        nc.vector.tensor_mul(
                            ctxT[h][:, qb * 512:(qb + 1) * 512], cpt[:],
                            rep[:])

            for fw in range(4 if phase == "full" else 0):
                wo = load_w_tiles(woT_d, fw * 512, f"wo{fw}", nec=OC)
                for qc in range(S // P):
                    qsl = slice(qc * P, (qc + 1) * P)
                    ps = ppp.tile([P, 512], dt.float32, name="ops", tag="pp")
                    for h in range(HH):
                        nc.tensor.matmul(
                            ps[:, 0:NB], ctxT[h][:, qsl], wo[h][:, 0:NB],
                            start=(h == 0), stop=False)
                        nc.tensor.matmul(
                            ps[:, NB:2 * NB], ctxT[h][:, qsl],
                            wo[h][:, NB:2 * NB],
                            start=False, stop=(h == HH - 1))
                    o0 = outp.tile([P, 512], out_dt, name="o0",
                                   tag="ot")
                    f0 = fw * 512
                    nc.vector.tensor_add(o0[:], ps[:],
                                         bo_rep[:, f0:f0 + 512])
                    nc.sync.dma_start(out_d[qsl, f0:f0 + 512], o0[:])

        emit = emit_v2 if sched == "v2" else emit_v1
        if bench_iters is None:
            emit()
        else:
            with tc.For_i(0, bench_iters, 1):
                emit()

    nc.compile()
    return nc


def _get_program():
    global _PROGRAM
    if _PROGRAM is None:
        _PROGRAM = _build_program()
    return _PROGRAM


def make_in_maps(query, in_proj_weight, in_proj_bias, out_proj_weight,
                 out_proj_bias):
    """Host-side sharding: slice/transpose/cast per core. Pure layout prep."""
    x = np.asarray(query, dtype=np.float32)
    W = np.asarray(in_proj_weight, dtype=np.float32)
    b = np.asarray(in_proj_bias, dtype=np.float32)
    Wo = np.asarray(out_proj_weight, dtype=np.float32)
    bo = np.asarray(out_proj_bias, dtype=np.float32)

    sc = np.float32(1.0 / np.sqrt(D))
    wqT = np.ascontiguousarray((W[:E] * sc).T).astype(BF16)       # [E, E]
    wkT = np.ascontiguousarray(W[E:2 * E].T).astype(BF16)
    wvT = np.ascontiguousarray(W[2 * E:].T).astype(BF16)
    woT = np.ascontiguousarray(Wo.T).astype(BF16)                 # [E, E]
    bq_s = (b[:E] * sc).reshape(H, P)
    bv = b[2 * E:]                                                # [E]

    in_maps = []
    for c in range(NCORES):
        bi, hh = c // 2, c % 2
        esl = slice(hh * EH, (hh + 1) * EH)
        xT = np.ascontiguousarray(x[bi].T).astype(BF16)
        # fold this half's share of Wo@bv into the output bias; add bo
        # itself only on the hh==0 core (partials are summed)
        bo_half = Wo[:, esl] @ bv[esl]
        if hh == 0:
            bo_half = bo_half + bo
        in_maps.append({
            "xT": xT,
            "wqT": np.ascontiguousarray(wqT[:, esl]),
            "wkT": np.ascontiguousarray(wkT[:, esl]),
            "wvT": np.ascontiguousarray(wvT[:, esl]),
            "woT": np.ascontiguousarray(woT[esl, :]),
            "bqT": np.ascontiguousarray(bq_s[hh * HH:(hh + 1) * HH].T),
            "bo": np.ascontiguousarray(bo_half.reshape(1, E)
                                       .astype(np.float32)),
        })
    return in_maps


def assemble_out(results):
    """Gather: sum each batch's two tensor-parallel partial outputs."""
    out = np.empty((B, S, E), dtype=np.float32)
    for bi in range(B):
        out[bi] = (results[2 * bi]["out"].astype(np.float32)
                   + results[2 * bi + 1]["out"].astype(np.float32))
    return out


def kernel(query, in_proj_weight, in_proj_bias, out_proj_weight,
           out_proj_bias):
    from concourse import bass_utils
    nc = _get_program()
    in_maps = make_in_maps(query, in_proj_weight, in_proj_bias,
                           out_proj_weight, out_proj_bias)
    res = bass_utils.run_bass_kernel_spmd(nc, in_maps,
                                          core_ids=list(range(NCORES)))
    return assemble_out(res.results)


# revision 27
# speedup vs baseline: 1.2023x; 1.0075x over previous
"""Trainium2 Bass kernel for fused multi-head attention (CompositeMHA).

Reference computation (B=4, S=1024, E=2048, H=16, D=128), fp32:
    proj = x @ in_proj_weight.T + in_proj_bias        # [B,S,3E]
    q,k,v = split(proj); heads of D=128
    ctx = softmax(q k^T / sqrt(D)) v                   # per (b, head)
    out = ctx @ out_proj_weight.T + out_proj_bias      # [B,S,E]

Sharding (8 cores, no on-device collectives), per the tensor-parallel hint:
data-parallel over the 4 batches x tensor-parallel over head halves.
Core c handles batch c//2 and heads [hh*8, hh*8+8) where hh = c%2 —
sharding the corresponding 3E rows of in_proj_weight and columns of
out_proj_weight.  Each core emits a partial output [S, E]; the gather
step sums each batch's two partials (the TP reduction).

Exact algebraic simplifications (no accuracy cost):
  - K bias dropped: softmax over j of (q+bq)·(k_j+bk) is invariant to
    the j-constant term (q+bq)·bk, so k_j needs no bias.
  - V bias folded into the output bias: sum_j p_j = 1, so
    ctx = ctxU + bv and out = ctxU@Wo^T + (bo + Wo@bv).

Tiling: ALL matmuls use 256-wide moving tensors (~107 ns measured at
full rate; 512-wide costs ~272 ns, so 2x256 wins).  Matmuls are emitted
as pairs sharing each loaded lhsT.

Schedule v2 (default): the PE queue is kept saturated by interleaving
independent projection / out-projection chains ("filler") into the
attention stream, so cross-engine waits (PE->ACT exp->PE) hide behind
queued matmuls instead of stalling the PE head-of-line:
    P0: K0/Q0/V0 projections (first two K head chains ec-outer, paced
        by the interleaved xT+Wk DMA stream to shrink the startup stall)
    blocks qb0 h0..h3   + filler K1/Q1/V1 chains (V1 spills into q0h4)
    blocks qb0 h4..h7   (little filler left -- ACT-paced stretch)
    blocks qb1 h0..h7   + filler out-proj chains for q-rows of qb0
    tail: out-proj chains for qb1 (pure PE)
Softmax sums+broadcast are FUSED into PE accumulation: rep[128,512] =
ones[128x128] @ P^T replicates the per-q denominators on every
partition; only reciprocal + multiply run on DVE.  In filler-filled
blocks (where DVE has slack) the k-chunk accumulation runs as a DVE
add-tree first, cutting the PE part to one matmul pair; starved blocks
keep the full 8-pair PE accumulation since 10 DVE ops (~6 us) would
out-pace PE there.  (GPSIMD partition_all_reduce measured ~7 us/op on
real HW -- 111 us slower per kernel, despite a 0.5 us cost-model
price; keep ucode ops off the critical path.)  PSUM: 2 proj + 3 score
+ 2 ctx + 1 sums banks; projection chains in the opening stage rotate
across ALL four pools (8 banks of copy-drain runway) while the
score/ctx/sums banks are idle.

On-core dataflow (bf16 matmuls into fp32 PSUM):
    xT   = X_b^T                      [E, S]
    K^T[h] = Wk^T-chunk^T @ xT        [D, S]   per head (no bias)
    Q^T[h] likewise + bq (DVE)        [D, S]   (1/sqrt(D) folded in)
    V      = xT-chunk^T @ Wv^T        [S, E/2] natural layout (no bias)
    scoresT[k,q] = K^T-chunk^T @ Q^T  -> exp on ACT -> P^T (bf16)
    sums[q]: DVE add-tree over P^T chunks -> GPSIMD all-reduce -> recip
    ctx^T[h] = V-chunk^T @ P^T accumulated; * recip -> bf16
    out_partial = ctx^T-chunk^T @ Wout^T (+ bo') -> fp32
"""

import numpy as np
import ml_dtypes

B, S, E, H = 4, 1024, 2048, 16
D = 128          # head dim == partition size
P = 128
HH = 8           # heads per core (head half)
EH = HH * D      # 1024: e-columns of this half
EC = E // P      # 16 e-chunks (contraction for in-proj)
OC = EH // P     # 8 e-chunks (contraction for out-proj)
KC = S // P      # 8 key chunks
NCORES = 8
NB = 256         # moving-tensor width for all matmuls
BF16 = ml_dtypes.bfloat16

_PROGRAM = None


def _build_program(bench_iters=None, phase="full", opts=None):
    opts = opts or {}
    import concourse.bass as bass  # noqa: F401
    import concourse.tile as tile
    from concourse import bacc, bass_isa, mybir
    from contextlib import ExitStack

    dt = mybir.dt
    AFT = mybir.ActivationFunctionType
    sched = opts.get("sched", "v2")

    nc = bacc.Bacc("TRN2", target_bir_lowering=False, debug=False,
                   num_devices=NCORES)
    out_dt = dt.bfloat16 if opts.get("out_bf16", True) else dt.float32

    xT_d = nc.dram_tensor("xT", [E, S], dt.bfloat16, kind="ExternalInput").ap()
    wqT_d = nc.dram_tensor("wqT", [E, EH], dt.bfloat16, kind="ExternalInput").ap()
    wkT_d = nc.dram_tensor("wkT", [E, EH], dt.bfloat16, kind="ExternalInput").ap()
    wvT_d = nc.dram_tensor("wvT", [E, EH], dt.bfloat16, kind="ExternalInput").ap()
    woT_d = nc.dram_tensor("woT", [EH, E], dt.bfloat16, kind="ExternalInput").ap()
    bqT_d = nc.dram_tensor("bqT", [P, HH], dt.float32, kind="ExternalInput").ap()
    bo_d = nc.dram_tensor("bo", [1, E], dt.float32, kind="ExternalInput").ap()
    out_d = nc.dram_tensor("out", [S, E], out_dt,
                           kind="ExternalOutput").ap()

    sums_on = opts.get("sums_on", "pe")  # v1: "pe" | "dve"

    with tile.TileContext(nc) as tc, ExitStack() as ctx:
        sb = ctx.enter_context(tc.tile_pool(name="persist", bufs=1))
        wp = ctx.enter_context(tc.tile_pool(name="wstream",
                                            bufs=opts.get("wp_bufs", 2)))
        ktp = ctx.enter_context(tc.tile_pool(name="ktp", bufs=8))
        qtp = ctx.enter_context(tc.tile_pool(name="qtp", bufs=8))
        ptp = ctx.enter_context(tc.tile_pool(
            name="ptp", bufs=opts.get("ptp_bufs", 24 if sched == "v1" else 16)))
        outp = ctx.enter_context(tc.tile_pool(
            name="outp", bufs=6 if sched == "v1" else 8))
        wop = ctx.enter_context(tc.tile_pool(name="wop", bufs=1))
        rowp = ctx.enter_context(tc.tile_pool(
            name="rowp", bufs=4 if sched == "v1" else 2))
        tp = ctx.enter_context(tc.tile_pool(
            name="tsum", bufs=4 if sched == "v1" else 2))
        if sched == "v1":
            pp_n, sp_n, cp_n, up_n = (opts.get("pp_bufs", 4),
                                      opts.get("sp_bufs", 2),
                                      opts.get("cp_bufs", 1), 1)
        else:
            pp_n, sp_n, cp_n, up_n = (opts.get("pp_bufs", 2),
                                      opts.get("sp_bufs", 3),
                                      opts.get("cp_bufs", 2), 1)
        ppp = ctx.enter_context(tc.tile_pool(name="ppsum", bufs=pp_n,
                                             space="PSUM"))
        spp = ctx.enter_context(tc.tile_pool(name="spsum", bufs=sp_n,
                                             space="PSUM"))
        cpp = ctx.enter_context(tc.tile_pool(name="cpsum", bufs=cp_n,
                                             space="PSUM"))
        if up_n:
            upp = ctx.enter_context(tc.tile_pool(name="upsum", bufs=up_n,
                                                 space="PSUM"))

        # ================= v2 schedule =================
        def emit_v2():
            mm = nc.tensor.matmul

            # ---- persistent tiles ----
            xt = [sb.tile([P, S], dt.bfloat16, name=f"xt{ec}", tag=f"xt{ec}")
                  for ec in range(EC)]
            bqt = sb.tile([P, HH], dt.float32, name="bqt", tag="bqt")
            bo_rep = sb.tile([P, E], dt.float32, name="bo_rep", tag="bo_rep")
            ones128 = sb.tile([P, P], dt.bfloat16, name="ones128",
                              tag="ones128")
            nc.vector.memset(ones128[:], 1.0)
            v_sb = [sb.tile([P, EH], dt.bfloat16, name=f"v{sc}", tag=f"v{sc}")
                    for sc in range(KC)]
            ctxT = [sb.tile([P, S], dt.bfloat16, name=f"ctxT{h}",
                            tag=f"ctxT{h}") for h in range(HH)]

            def w_tiles(dram, col0, label, nec=EC):
                tiles = []
                for ec in range(nec):
                    t = wp.tile([P, 512], dt.bfloat16,
                                name=f"{label}{ec}", tag=f"w{ec}")
                    nc.sync.dma_start(
                        t[:], dram[ec * P:(ec + 1) * P, col0:col0 + 512])
                    tiles.append(t)
                return tiles

            # ---- DMA issue: wk0 and xt interleaved first (startup), then
            # ---- the rest; weight streaming is gated by wp tag rotation.
            wk0 = []
            for ec in range(EC):
                t = wp.tile([P, 512], dt.bfloat16, name=f"wk0_{ec}",
                            tag=f"w{ec}")
                nc.sync.dma_start(t[:], wkT_d[ec * P:(ec + 1) * P, 0:512])
                nc.sync.dma_start(xt[ec][:], xT_d[ec * P:(ec + 1) * P, :])
                wk0.append(t)
            nc.sync.dma_start(bqt[:], bqT_d[:])
            nc.sync.dma_start(bo_rep[:], bo_d.to_broadcast((P, E)))
            wq0 = w_tiles(wqT_d, 0, "wq0")
            wv0 = w_tiles(wvT_d, 0, "wv0")
            wk1 = w_tiles(wkT_d, 512, "wk1")
            wq1 = w_tiles(wqT_d, 512, "wq1")
            wv1 = w_tiles(wvT_d, 512, "wv1")
            # out-proj weights: persistent (own tags, loaded once) — saves
            # a 4MB re-load and its tag-rotation gating; DMA is a shared
            # cross-core resource (8-core run measured +14% over 1-core)
            wo_all = []
            for fw in range(4):
                tiles = []
                for ec in range(OC):
                    t = wop.tile([P, 512], dt.bfloat16, name=f"wo{fw}_{ec}",
                                 tag=f"wo{fw}_{ec}")
                    nc.sync.dma_start(
                        t[:], woT_d[ec * P:(ec + 1) * P,
                                    fw * 512:(fw + 1) * 512])
                    tiles.append(t)
                wo_all.append(tiles)

            kt = {}
            qt = {}

            # During P0 (and the proj-only phase) the score/ctx/sums banks
            # are idle: rotate projection chains across ALL psum pools for
            # 8 banks of copy-drain runway instead of ppp's 2.
            p0_pools = [(ppp, "pp"), (spp, "sp"), (cpp, "cp"), (upp, "rep")]
            p0_idx = [0]

            def chain_psum(pool):
                if pool is None or not opts.get("p0mp", True):
                    return ppp.tile([P, 512], dt.float32, name="chps",
                                    tag="pp")
                pl, tag = p0_pools[p0_idx[0] % len(p0_pools)]
                p0_idx[0] += 1
                return pl.tile([P, 512], dt.float32, name="chps", tag=tag)

            def kq_chain(w, grp, hh4, pr, dst, pool=None):
                """One 512-col block of K^T or Q^T for head grp*4+hh4."""
                hsl = slice(hh4 * P, (hh4 + 1) * P)
                c0 = pr * 512
                ps = chain_psum(pool)
                for ec in range(EC):
                    mm(ps[:, 0:NB], w[ec][:, hsl], xt[ec][:, c0:c0 + NB],
                       start=(ec == 0), stop=False)
                    mm(ps[:, NB:2 * NB], w[ec][:, hsl],
                       xt[ec][:, c0 + NB:c0 + 512],
                       start=False, stop=(ec == EC - 1))
                h = grp * 4 + hh4
                if dst is kt:
                    nc.vector.tensor_copy(kt[h][:, c0:c0 + 512], ps[:])
                else:
                    nc.vector.tensor_scalar_add(
                        qt[h][:, c0:c0 + 512], ps[:], bqt[:, h:h + 1])

            def v_chain(wv, fw, sc, pool=None):
                ps = chain_psum(pool)
                ssl = slice(sc * P, (sc + 1) * P)
                for ec in range(EC):
                    mm(ps[:, 0:NB], xt[ec][:, ssl], wv[ec][:, 0:NB],
                       start=(ec == 0), stop=False)
                    mm(ps[:, NB:2 * NB], xt[ec][:, ssl], wv[ec][:, NB:2 * NB],
                       start=False, stop=(ec == EC - 1))
                nc.vector.tensor_copy(
                    v_sb[sc][:, fw * 512:(fw + 1) * 512], ps[:])

            def op_chain(wo, fw, qc):
                qsl = slice(qc * P, (qc + 1) * P)
                ps = ppp.tile([P, 512], dt.float32, name="ops", tag="pp")
                for h in range(HH):
                    mm(ps[:, 0:NB], ctxT[h][:, qsl], wo[h][:, 0:NB],
                       start=(h == 0), stop=False)
                    mm(ps[:, NB:2 * NB], ctxT[h][:, qsl], wo[h][:, NB:2 * NB],
                       start=False, stop=(h == HH - 1))
                o0 = outp.tile([P, 512], out_dt, name="o0", tag="ot")
                f0 = fw * 512
                nc.vector.tensor_add(o0[:], ps[:], bo_rep[:, f0:f0 + 512])
                nc.sync.dma_start(out_d[qsl, f0:f0 + 512], o0[:])

            # allocate kt/qt tiles in head order (tag rotation = 8 live)
            for h in range(HH):
                kt[h] = ktp.tile([P, S], dt.bfloat16, name=f"kt{h}", tag="kt")
                qt[h] = qtp.tile([P, S], dt.bfloat16, name=f"qt{h}", tag="qt")

            # ---- filler queue ----
            filler = []

            def take(n):
                for _ in range(n):
                    if filler:
                        filler.pop(0)()

            probe = opts.get("probe")  # None|"const_p"|"no_tail"

            def attn_block(h, qb, n_fill):
                hsl = slice(h * P, (h + 1) * P)
                qA = slice(qb * 512, qb * 512 + NB)
                qB = slice(qb * 512 + NB, (qb + 1) * 512)
                nf1 = n_fill // 3
                nf2 = n_fill // 3
                nf3 = n_fill - nf1 - nf2
                # DVE-tree sums only in filler-filled blocks: there DVE has
                # slack and this trims 14 PE sum-matmuls to 2; in starved
                # blocks DVE (10 ops ~6us) would out-pace PE, so keep PE sums
                dsum_mode = opts.get("dsum", "fill")
                dsum = (dsum_mode == "all"
                        or (dsum_mode == "fill" and n_fill > 0))
                cpt = cpp.tile([P, 512], dt.float32, name=f"c{h}_{qb}",
                               tag="cp")
                pts = []
                acc = None
                for kc in range(KC):
                    ksl = slice(kc * P, (kc + 1) * P)
                    sp = spp.tile([P, 512], dt.float32, name="sp", tag="sp")
                    mm(sp[:, 0:NB], kt[h][:, ksl], qt[h][:, qA])
                    mm(sp[:, NB:2 * NB], kt[h][:, ksl], qt[h][:, qB])
                    if probe != "const_p":
                        pt = ptp.tile([P, 512], dt.bfloat16, name="pt",
                                      tag="pt")
                        if opts.get("exp_split"):
                            nc.scalar.activation(pt[:, 0:NB], sp[:, 0:NB],
                                                 AFT.Exp)
                            nc.scalar.activation(pt[:, NB:2 * NB],
                                                 sp[:, NB:2 * NB], AFT.Exp)
                        else:
                            nc.scalar.activation(pt[:], sp[:], AFT.Exp)
                        pts.append(pt)
                        if dsum and probe is None:
                            if kc == 1:
                                acc = tp.tile([P, 512], dt.float32,
                                              name="acc", tag="acc")
                                nc.vector.tensor_add(acc[:], pts[0][:],
                                                     pts[1][:])
                            elif kc > 1:
                                nc.vector.tensor_add(acc[:], acc[:], pt[:])
                    if kc == 2:
                        take(nf1)
                    elif kc == 5:
                        take(nf2)
                take(nf3)
                for kc in range(KC):
                    rhs = (v_sb[kc][:, 0:512] if probe == "const_p"
                           else pts[kc][:])
                    mm(cpt[:, 0:NB], v_sb[kc][:, hsl], rhs[:, 0:NB],
                       start=(kc == 0), stop=False)
                    mm(cpt[:, NB:2 * NB], v_sb[kc][:, hsl],
                       rhs[:, NB:2 * NB],
                       start=False, stop=(kc == KC - 1))
                if probe in ("const_p", "no_tail"):
                    nc.vector.tensor_copy(
                        ctxT[h][:, qb * 512:(qb + 1) * 512], cpt[:])
                    return
                # fused sums+broadcast on PE: rep[p, q] = sum_k P^T[k, q]
                # (ones lhsT replicates the column sums on every partition)
                rep = upp.tile([P, 512], dt.float32, name=f"rep{h}_{qb}",
                               tag="rep")
                if dsum:
                    t16 = tp.tile([P, 512], dt.bfloat16, name="t16",
                                  tag="t16")
                    nc.vector.tensor_copy(t16[:], acc[:])
                    mm(rep[:, 0:NB], ones128[:], t16[:, 0:NB])
                    mm(rep[:, NB:2 * NB], ones128[:], t16[:, NB:2 * NB])
                else:
                    for kc in range(KC):
                        mm(rep[:, 0:NB], ones128[:], pts[kc][:, 0:NB],
                           start=(kc == 0), stop=False)
                        mm(rep[:, NB:2 * NB], ones128[:],
                           pts[kc][:, NB:2 * NB],
                           start=False, stop=(kc == KC - 1))
                rrec = rowp.tile([P, 512], dt.float32, name="rrec",
                                 tag="rrec")
                nc.vector.reciprocal(rrec[:], rep[:])
                nc.vector.tensor_mul(
                    ctxT[h][:, qb * 512:(qb + 1) * 512], cpt[:], rrec[:])

            # ---- P0: grp0 projections ----
            # first three K head chains ec-outer, 6 accumulators (2 proj +
            # 3 score + 1 ctx banks, all idle at startup): paced by the
            # interleaved xt+wk DMA stream, so the startup stall shrinks to
            # the DMA/PE rate gap (the DMA side is also slowed by all 8
            # cores pulling their xt bursts simultaneously).
            accs = [ppp.tile([P, 512], dt.float32, name="k00", tag="pp"),
                    ppp.tile([P, 512], dt.float32, name="k01", tag="pp"),
                    spp.tile([P, 512], dt.float32, name="k10", tag="sp"),
                    spp.tile([P, 512], dt.float32, name="k11", tag="sp"),
                    spp.tile([P, 512], dt.float32, name="k20", tag="sp"),
                    cpp.tile([P, 512], dt.float32, name="k21", tag="cp")]
            for ec in range(EC):
                for i, ps in enumerate(accs):
                    hh4, pr = divmod(i, 2)
                    hsl = slice(hh4 * P, (hh4 + 1) * P)
                    c0 = pr * 512
                    mm(ps[:, 0:NB], wk0[ec][:, hsl], xt[ec][:, c0:c0 + NB],
                       start=(ec == 0), stop=False)
                    mm(ps[:, NB:2 * NB], wk0[ec][:, hsl],
                       xt[ec][:, c0 + NB:c0 + 512],
                       start=False, stop=(ec == EC - 1))
            for i, ps in enumerate(accs):
                hh4, pr = divmod(i, 2)
                nc.vector.tensor_copy(kt[hh4][:, pr * 512:(pr + 1) * 512],
                                      ps[:])
            for hh4 in range(3, 4):
                for pr in range(2):
                    kq_chain(wk0, 0, hh4, pr, kt, pool="p0")
            for hh4 in range(4):
                for pr in range(2):
                    kq_chain(wq0, 0, hh4, pr, qt, pool="p0")
            for sc in range(KC):
                v_chain(wv0, 0, sc, pool="p0")

            if phase == "proj":
                for hh4 in range(4):
                    for pr in range(2):
                        kq_chain(wk1, 1, hh4, pr, kt, pool="p0")
                        kq_chain(wq1, 1, hh4, pr, qt, pool="p0")
                for sc in range(KC):
                    v_chain(wv1, 1, sc, pool="p0")
                return

            # ---- filler: grp1 projections (K/Q per head, then V) ----
            for hh4 in range(4):
                for pr in range(2):
                    filler.append(
                        lambda hh4=hh4, pr=pr: kq_chain(wk1, 1, hh4, pr, kt))
                for pr in range(2):
                    filler.append(
                        lambda hh4=hh4, pr=pr: kq_chain(wq1, 1, hh4, pr, qt))
            for sc in range(KC):
                filler.append(lambda sc=sc: v_chain(wv1, 1, sc))
            n_proj_fill = len(filler)  # 24

            # ---- attention qb0, grp0 heads: consume the proj filler; the
            # ---- V1 leftovers may spill into q0h4's scores section (they
            # ---- still precede its ctx consume in the PE queue) ----
            fa = opts.get("fill_a", 5)
            fb = opts.get("fill_b", 4)
            assert 4 * fa + fb >= n_proj_fill == 24
            for h in range(4):
                attn_block(h, 0, fa)
            attn_block(4, 0, fb)
            assert not filler, "proj filler must drain before q0h4 ctx"

            # ---- attention qb0, rest of grp1: no filler available ----
            for h in range(5, 8):
                attn_block(h, 0, 0)

            if phase != "full":
                for h in range(HH):
                    attn_block(h, 1, 0)
                return

            # ---- out-proj filler for qb0 q-rows (fw-major; wo tiles are
            # ---- re-loaded per qb so the wp tag rotation stays deadlock
            # ---- free) ----
            for fw in range(4):
                for qc in range(4):
                    filler.append(
                        lambda fw=fw, qc=qc: op_chain(wo_all[fw], fw, qc))

            # ---- attention qb1 ----
            fc = opts.get("fill_c", 2)
            for h in range(4):
                attn_block(h, 1, fc)
            for h in range(4, 8):
                attn_block(h, 1, (16 - 4 * fc) // 4)
            take(len(filler))

            # ---- tail: out-proj for qb1 q-rows ----
            for fw in range(4):
                for qc in range(4, 8):
                    op_chain(wo_all[fw], fw, qc)

        # ================= v1 schedule (previous baseline) =================
        def emit_v1():
            # ---- persistent loads ----
            xt = []
            for ec in range(EC):
                t = sb.tile([P, S], dt.bfloat16, name=f"xt{ec}", tag=f"xt{ec}")
                nc.sync.dma_start(t[:], xT_d[ec * P:(ec + 1) * P, :])
                xt.append(t)
            bqt = sb.tile([P, HH], dt.float32, name="bqt", tag="bqt")
            nc.sync.dma_start(bqt[:], bqT_d[:])
            bo_rep = sb.tile([P, E], dt.float32, name="bo_rep", tag="bo_rep")
            nc.sync.dma_start(bo_rep[:], bo_d.to_broadcast((P, E)))
            ones_col = sb.tile([P, 1], dt.bfloat16, name="ones_col",
                               tag="ones_col")
            nc.vector.memset(ones_col[:], 1.0)

            v_sb = []
            for sc in range(S // P):
                t = sb.tile([P, EH], dt.bfloat16, name=f"v{sc}", tag=f"v{sc}")
                v_sb.append(t)
            ctxT = []
            for h in range(HH):
                t = sb.tile([P, S], dt.bfloat16, name=f"ctxT{h}",
                            tag=f"ctxT{h}")
                ctxT.append(t)

            def load_w_tiles(dram, col0, label, nec=EC):
                tiles = []
                for ec in range(nec):
                    t = wp.tile([P, 512], dt.bfloat16,
                                name=f"{label}{ec}", tag=f"w{ec}")
                    nc.sync.dma_start(
                        t[:], dram[ec * P:(ec + 1) * P, col0:col0 + 512])
                    tiles.append(t)
                return tiles

            kt = {}
            qt = {}
            for grp in range(2):
                wk = load_w_tiles(wkT_d, grp * 512, f"wk{grp}")
                for hh4 in range(4):
                    h = grp * 4 + hh4
                    kth = ktp.tile([P, S], dt.bfloat16, name=f"kt{h}",
                                   tag="kt")
                    hsl = slice(hh4 * P, (hh4 + 1) * P)
                    for pr in range(2):
                        ps = ppp.tile([P, 512], dt.float32, name="kps",
                                      tag="pp")
                        c0 = pr * 512
                        for ec in range(EC):
                            nc.tensor.matmul(
                                ps[:, 0:NB], wk[ec][:, hsl],
                                xt[ec][:, c0:c0 + NB],
                                start=(ec == 0), stop=False)
                            nc.tensor.matmul(
                                ps[:, NB:2 * NB], wk[ec][:, hsl],
                                xt[ec][:, c0 + NB:c0 + 2 * NB],
                                start=False, stop=(ec == EC - 1))
                        nc.vector.tensor_copy(kth[:, c0:c0 + 512], ps[:])
                    kt[h] = kth

                wq = load_w_tiles(wqT_d, grp * 512, f"wq{grp}")
                for hh4 in range(4):
                    h = grp * 4 + hh4
                    qth = qtp.tile([P, S], dt.bfloat16, name=f"qt{h}",
                                   tag="qt")
                    hsl = slice(hh4 * P, (hh4 + 1) * P)
                    for pr in range(2):
                        ps = ppp.tile([P, 512], dt.float32, name="qps",
                                      tag="pp")
                        c0 = pr * 512
                        for ec in range(EC):
                            nc.tensor.matmul(
                                ps[:, 0:NB], wq[ec][:, hsl],
                                xt[ec][:, c0:c0 + NB],
                                start=(ec == 0), stop=False)
                            nc.tensor.matmul(
                                ps[:, NB:2 * NB], wq[ec][:, hsl],
                                xt[ec][:, c0 + NB:c0 + 2 * NB],
                                start=False, stop=(ec == EC - 1))
                        nc.vector.tensor_scalar_add(
                            qth[:, c0:c0 + 512], ps[:], bqt[:, h:h + 1])
                    qt[h] = qth

                fw = grp
                wv = load_w_tiles(wvT_d, fw * 512, f"wv{grp}")
                for sc in range(S // P):
                    ps = ppp.tile([P, 512], dt.float32, name="vps", tag="pp")
                    ssl = slice(sc * P, (sc + 1) * P)
                    for ec in range(EC):
                        nc.tensor.matmul(
                            ps[:, 0:NB], xt[ec][:, ssl], wv[ec][:, 0:NB],
                            start=(ec == 0), stop=False)
                        nc.tensor.matmul(
                            ps[:, NB:2 * NB], xt[ec][:, ssl],
                            wv[ec][:, NB:2 * NB],
                            start=False, stop=(ec == EC - 1))
                    nc.vector.tensor_copy(
                        v_sb[sc][:, fw * 512:(fw + 1) * 512], ps[:])

                for hh4 in range(4 if phase != "proj" else 0):
                    h = grp * 4 + hh4
                    hsl = slice(h * P, (h + 1) * P)
                    for qb in range(S // 512):
                        qA = slice(qb * 512, qb * 512 + NB)
                        qB = slice(qb * 512 + NB, qb * 512 + 2 * NB)
                        cpt = cpp.tile([P, 512], dt.float32,
                                       name=f"ctx{h}_{qb}", tag="cp")
                        ctxA = cpt[:, 0:NB]
                        ctxB = cpt[:, NB:2 * NB]
                        lag = opts.get("pipe_lag", 2)
                        pts = {}

                        def consume(kc):
                            pt = pts[kc]
                            nc.tensor.matmul(
                                ctxA, v_sb[kc][:, hsl], pt[:, 0:NB],
                                start=(kc == 0), stop=False)
                            nc.tensor.matmul(
                                ctxB, v_sb[kc][:, hsl], pt[:, NB:2 * NB],
                                start=False, stop=(kc == KC - 1))

                        for kc in range(KC):
                            ksl = slice(kc * P, (kc + 1) * P)
                            sp = spp.tile([P, 512], dt.float32, name="sp",
                                          tag="sp")
                            nc.tensor.matmul(sp[:, 0:NB], kt[h][:, ksl],
                                             qt[h][:, qA])
                            nc.tensor.matmul(sp[:, NB:2 * NB], kt[h][:, ksl],
                                             qt[h][:, qB])
                            pt = ptp.tile([P, 512], dt.bfloat16, name="pt",
                                          tag="pt")
                            nc.scalar.activation(pt[:], sp[:], AFT.Exp)
                            pts[kc] = pt
                            if kc >= lag:
                                consume(kc - lag)
                        for kc in range(KC - lag, KC):
                            consume(kc)

                        su = upp.tile([1, 512], dt.float32,
                                      name=f"su{h}_{qb}", tag="su")
                        if sums_on == "pe":
                            for kc in range(KC):
                                pt = pts[kc]
                                nc.tensor.matmul(su[:, 0:NB], ones_col[:],
                                                 pt[:, 0:NB],
                                                 start=(kc == 0),
                                                 stop=False)
                                nc.tensor.matmul(su[:, NB:2 * NB],
                                                 ones_col[:],
                                                 pt[:, NB:2 * NB],
                                                 start=False,
                                                 stop=(kc == KC - 1))
                        else:
                            tacc = tp.tile([P, 512], dt.float32, name="tacc",
                                           tag="tacc")
                            nc.vector.tensor_add(tacc[:], pts[0][:],
                                                 pts[1][:])
                            for kc in range(2, KC):
                                nc.vector.tensor_add(tacc[:], tacc[:],
                                                     pts[kc][:])
                            t16 = tp.tile([P, 512], dt.bfloat16, name="t16",
                                          tag="t16")
                            nc.vector.tensor_copy(t16[:], tacc[:])
                            nc.tensor.matmul(su[:, 0:NB], ones_col[:],
                                             t16[:, 0:NB])
                            nc.tensor.matmul(su[:, NB:2 * NB], ones_col[:],
                                             t16[:, NB:2 * NB])
                        r_row = rowp.tile([1, 512], dt.float32, name="r_row",
                                          tag="rrow")
                        nc.vector.reciprocal(r_row[:], su[:])
                        rep = rowp.tile([P, 512], dt.float32, name="rep",
                                        tag="rep")
                        nc.gpsimd.partition_broadcast(rep[:], r_row[:])
                        nc.vector.tensor_mul(
                            ctxT[h][:, qb * 512:(qb + 1) * 512], cpt[:],
                            rep[:])

            for fw in range(4 if phase == "full" else 0):
                wo = load_w_tiles(woT_d, fw * 512, f"wo{fw}", nec=OC)
                for qc in range(S // P):
                    qsl = slice(qc * P, (qc + 1) * P)
                    ps = ppp.tile([P, 512], dt.float32, name="ops", tag="pp")
                    for h in range(HH):
                        nc.tensor.matmul(
                            ps[:, 0:NB], ctxT[h][:, qsl], wo[h][:, 0:NB],
                            start=(h == 0), stop=False)
                        nc.tensor.matmul(
                            ps[:, NB:2 * NB], ctxT[h][:, qsl],
                            wo[h][:, NB:2 * NB],
                            start=False, stop=(h == HH - 1))
                    o0 = outp.tile([P, 512], out_dt, name="o0",
                                   tag="ot")
                    f0 = fw * 512
                    nc.vector.tensor_add(o0[:], ps[:],
                                         bo_rep[:, f0:f0 + 512])
                    nc.sync.dma_start(out_d[qsl, f0:f0 + 512], o0[:])

        emit = emit_v2 if sched == "v2" else emit_v1
        if bench_iters is None:
            emit()
        else:
            with tc.For_i(0, bench_iters, 1):
                emit()

    nc.compile()
    return nc


def _get_program():
    global _PROGRAM
    if _PROGRAM is None:
        _PROGRAM = _build_program()
    return _PROGRAM


def make_in_maps(query, in_proj_weight, in_proj_bias, out_proj_weight,
                 out_proj_bias):
    """Host-side sharding: slice/transpose/cast per core. Pure layout prep."""
    x = np.asarray(query, dtype=np.float32)
    W = np.asarray(in_proj_weight, dtype=np.float32)
    b = np.asarray(in_proj_bias, dtype=np.float32)
    Wo = np.asarray(out_proj_weight, dtype=np.float32)
    bo = np.asarray(out_proj_bias, dtype=np.float32)

    sc = np.float32(1.0 / np.sqrt(D))
    wqT = np.ascontiguousarray((W[:E] * sc).T).astype(BF16)       # [E, E]
    wkT = np.ascontiguousarray(W[E:2 * E].T).astype(BF16)
    wvT = np.ascontiguousarray(W[2 * E:].T).astype(BF16)
    woT = np.ascontiguousarray(Wo.T).astype(BF16)                 # [E, E]
    bq_s = (b[:E] * sc).reshape(H, P)
    bv = b[2 * E:]                                                # [E]

    in_maps = []
    for c in range(NCORES):
        bi, hh = c // 2, c % 2
        esl = slice(hh * EH, (hh + 1) * EH)
        xT = np.ascontiguousarray(x[bi].T).astype(BF16)
        # fold this half's share of Wo@bv into the output bias; add bo
        # itself only on the hh==0 core (partials are summed)
        bo_half = Wo[:, esl] @ bv[esl]
        if hh == 0:
            bo_half = bo_half + bo
        in_maps.append({
            "xT": xT,
            "wqT": np.ascontiguousarray(wqT[:, esl]),
            "wkT": np.ascontiguousarray(wkT[:, esl]),
            "wvT": np.ascontiguousarray(wvT[:, esl]),
            "woT": np.ascontiguousarray(woT[esl, :]),
            "bqT": np.ascontiguousarray(bq_s[hh * HH:(hh + 1) * HH].T),
            "bo": np.ascontiguousarray(bo_half.reshape(1, E)
                                       .astype(np.float32)),
        })
    return in_maps


def assemble_out(results):
    """Gather: sum each batch's two tensor-parallel partial outputs."""
    out = np.empty((B, S, E), dtype=np.float32)
    for bi in range(B):
        out[bi] = (results[2 * bi]["out"].astype(np.float32)
                   + results[2 * bi + 1]["out"].astype(np.float32))
    return out


def kernel(query, in_proj_weight, in_proj_bias, out_proj_weight,
           out_proj_bias):
    from concourse import bass_utils
    nc = _get_program()
    in_maps = make_in_maps(query, in_proj_weight, in_proj_bias,
                           out_proj_weight, out_proj_bias)
    res = bass_utils.run_bass_kernel_spmd(nc, in_maps,
                                          core_ids=list(range(NCORES)))
    return assemble_out(res.results)
